# revision 1
# baseline (speedup 1.0000x reference)
"""ArbSR (moe_routing) Trainium2 kernel, 8-core SPMD.

Structure exploited: with scale=4, the scale-embedding MLP input is periodic
with period 4 in both HR axes, so routing r, offsets off, and the expert-mix
matrices take only 16 distinct values (one per (y%4, x%4) class).  The
offset grid_sample then becomes, per class, a 2x2-tap bilinear filter of the
encoder feature map f at a constant integer shift, and the whole
  fea0 -> expert mixing -> (+fea0) -> 3x3 tail conv
chain collapses to
  pred[:, 4*yl+b, 4*xl+a] = tail_b + sum_delta E[(b,a)][delta] @ f[:, yl+dy, xl+dx]
with host-precomputed [3,64] matrices E (a 3x3 delta neighborhood in
practice).  Tail-conv zero padding at the image border is handled with
per-edge correction streams whose matrices are zeroed on cores that don't
own the edge.

Per core (64 HR rows): encoder conv as one K=28 matmul from a host-built
im2col; the pred computation as ~6 K=128-packed float32r matmul streams (f
stacked with a row-shifted copy); PE transpose into a pixel-major layout;
and an indirect-DMA gather for the nearest-neighbour queries, which the host
routes to cores by row ownership.
"""

import numpy as np
import ml_dtypes

BF16 = ml_dtypes.bfloat16


def _ensure_path():
    import sys
    for p in ('/opt/trn_rl_repo',):
        if p not in sys.path:
            sys.path.append(p)


H = W = 128
S = 4
HH = WH = H * S          # 512
C = 64
NCORES = 8
YLC = H // NCORES        # 16 LR rows per core
HRPC = HH // NCORES      # 64 HR rows per core
NPIX = HRPC * WH         # 32768 HR pixels per core
NQ_COLS = 264
NQP = 128 * NQ_COLS      # 33792 padded queries per core
NCLS = 16                # (b, a) classes
MROWS = NCLS * 3         # 48 stacked pred rows


def _sigmoid(x):
    return 1.0 / (1.0 + np.exp(-x))


def _class_constants(d):
    w1 = np.asarray(d['body_w1'], np.float64)
    b1 = np.asarray(d['body_b1'], np.float64)
    w2 = np.asarray(d['body_w2'], np.float64)
    b2 = np.asarray(d['body_b2'], np.float64)
    rw = np.asarray(d['routing_w'], np.float64)
    rb = np.asarray(d['routing_b'], np.float64)
    ow = np.asarray(d['offset_w'], np.float64)
    ob = np.asarray(d['offset_b'], np.float64)
    wc = np.asarray(d['weight_compress'], np.float64)
    we = np.asarray(d['weight_expand'], np.float64)

    fs = float(S)
    coor = np.array([(i + 0.5) / fs - np.floor((i + 0.5) / fs + 0.001) - 0.5
                     for i in range(S)])
    cls = {}
    for b in range(S):
        for a in range(S):
            inp4 = np.array([1.0 / fs, 1.0 / fs, coor[b], coor[a]])
            emb = np.maximum(w1 @ inp4 + b1, 0.0)
            emb = np.maximum(w2 @ emb + b2, 0.0)
            off = ow @ emb + ob
            r = _sigmoid(rw @ emb + rb)
            A = np.einsum('e,eck->ck', r, we) @ np.einsum('e,ekc->kc', r, wc)
            B = A + np.eye(C)
            cx = (a + 0.5) / fs - 0.5 + off[0]
            cy = (b + 0.5) / fs - 0.5 + off[1]
            ix, iy = int(np.floor(cx)), int(np.floor(cy))
            fx, fy = cx - ix, cy - iy
            wbl = {(0, 0): (1 - fy) * (1 - fx), (0, 1): (1 - fy) * fx,
                   (1, 0): fy * (1 - fx), (1, 1): fy * fx}
            cls[(b, a)] = dict(B=B, ix=ix, iy=iy, wbl=wbl)
    return cls


def _build_E(tail_w, cls, only_ty=None, only_tx=None):
    """E[(b,a)][(dy,dx)] = [3, C] so that pred contribution is E @ f(shift)."""
    Es = {}
    for b in range(S):
        for a in range(S):
            acc = {}
            for ty in range(3):
                if only_ty is not None and ty not in only_ty:
                    continue
                for tx in range(3):
                    if only_tx is not None and tx not in only_tx:
                        continue
                    bp = (b + ty - 1) % S
                    oy = (b + ty - 1 - bp) // S
                    ap_ = (a + tx - 1) % S
                    ox = (a + tx - 1 - ap_) // S
                    c2 = cls[(bp, ap_)]
                    TB = tail_w[:, :, ty, tx] @ c2['B']
                    for (uy, ux), wgt in c2['wbl'].items():
                        if wgt == 0.0:
                            continue
                        key = (oy + c2['iy'] + uy, ox + c2['ix'] + ux)
                        acc[key] = acc.get(key, np.zeros((3, C))) + TB * wgt
            Es[(b, a)] = acc
    return Es


def _stack_E(Es, deltas, classes=None, sign=1.0):
    """Per-delta [MROWS, C] matrices, rows ordered (4b+a)*3 + c."""
    G = {dl: np.zeros((MROWS, C)) for dl in deltas}
    for (b, a), acc in Es.items():
        if classes is not None and (b, a) not in classes:
            continue
        m0 = (4 * b + a) * 3
        for dl, M in acc.items():
            G[dl][m0:m0 + 3, :] += sign * M
    return G


def _pair_streams(deltas):
    """Pair (dy,dx) with (dy+1,dx); unpaired run as K=64 streams."""
    deltas = sorted(deltas)
    dset, used, streams = set(deltas), set(), []
    for dl in deltas:
        if dl in used:
            continue
        hi = (dl[0] + 1, dl[1])
        if hi in dset and hi not in used:
            streams.append((dl, True))
            used.update((dl, hi))
        else:
            streams.append((dl, False))
            used.add(dl)
    return streams


def _stream_tensors(G, streams):
    """lhsT arrays [K, MROWS] per stream (K=128 paired, 64 single)."""
    out = []
    for dl, paired in streams:
        if paired:
            hi = (dl[0] + 1, dl[1])
            lhsT = np.zeros((128, MROWS), np.float32)
            lhsT[0:C, :] = G[dl].T
            lhsT[C:2 * C, :] = G[hi].T
        else:
            lhsT = np.ascontiguousarray(G[dl].T, dtype=np.float32)
        out.append(lhsT)
    return out


def _plan_and_host_data(d):
    """Everything the host precomputes: stream plans, per-core inputs,
    query routing."""
    cls = _class_constants(d)
    tail_w = np.asarray(d['tail_w'], np.float64)
    tail_b = np.asarray(d['tail_b'], np.float64)

    E_main = _build_E(tail_w, cls)
    deltas = sorted({k for acc in E_main.values() for k in acc})
    dys = [dl[0] for dl in deltas]
    dxs = [dl[1] for dl in deltas]
    dy_min, dy_max = min(dys), max(dys)
    dx_min, dx_max = min(dxs), max(dxs)
    NRF = 17 + dy_max - dy_min       # f rows per core (last row upper-only)
    NCF = W + dx_max - dx_min        # f cols
    NF = NRF * NCF
    assert NRF <= 40 and NCF <= 192, (NRF, NCF)

    main_streams = _pair_streams(deltas)
    G_main = _stack_E(E_main, deltas)
    main_T = _stream_tensors(G_main, main_streams)

    # edge corrections: subtract out-of-image tail-tap contributions
    def corr(only_ty, only_tx, classes, sign):
        E = _build_E(tail_w, cls, only_ty=only_ty, only_tx=only_tx)
        dls = sorted({k for (ba, acc) in E.items() if ba in classes
                      for k, M in acc.items()})
        if not dls:
            return [], []
        st = _pair_streams(dls)
        G = _stack_E(E, dls, classes=classes, sign=sign)
        return st, _stream_tensors(G, st)

    top_cls = [(0, a) for a in range(S)]
    bot_cls = [(3, a) for a in range(S)]
    lef_cls = [(b, 0) for b in range(S)]
    rig_cls = [(b, 3) for b in range(S)]
    c_lef = corr(None, (0,), lef_cls, -1.0)
    c_rig = corr(None, (2,), rig_cls, -1.0)

    # top/bottom edge correction M-stacked with the corner add-backs:
    # lhsT cols 0:48 = edge correction; cols 64:96 / 96:128 hold the two
    # corner add-backs at 32-aligned windows (row j of window <-> pred row
    # j + base), so corner merges are 32-partition-aligned DVE adds.
    def corr_merged(only_ty, edge_cls, cornerL, cornerR, cbase):
        E_edge = _build_E(tail_w, cls, only_ty=only_ty)
        E_cl = _build_E(tail_w, cls, only_ty=only_ty, only_tx=(0,))
        E_cr = _build_E(tail_w, cls, only_ty=only_ty, only_tx=(2,))
        dls = sorted({k for ba in edge_cls for k in E_edge[ba]})
        st = _pair_streams(dls)
        G = {dl: np.zeros((128, C)) for dl in dls}
        for ba in edge_cls:
            m0 = (4 * ba[0] + ba[1]) * 3
            for dl, M in E_edge[ba].items():
                G[dl][m0:m0 + 3, :] -= M
        mL = (4 * cornerL[0] + cornerL[1]) * 3 - cbase
        for dl, M in E_cl[cornerL].items():
            G[dl][64 + mL:64 + mL + 3, :] += M
        mR = (4 * cornerR[0] + cornerR[1]) * 3 - cbase
        for dl, M in E_cr[cornerR].items():
            G[dl][96 + mR:96 + mR + 3, :] += M
        out = []
        for dl, paired in st:
            if paired:
                hi = (dl[0] + 1, dl[1])
                lhsT = np.zeros((128, 128), np.float32)
                lhsT[0:C, :] = G[dl].T
                lhsT[C:2 * C, :] = G.get(hi, np.zeros((128, C))).T
            else:
                lhsT = np.ascontiguousarray(G[dl].T, dtype=np.float32)
            out.append(lhsT)
        return st, out

    # top window base 0 (corners in pred rows 0-31), bottom base 32
    c_top = corr_merged((0,), top_cls, (0, 0), (0, 3), 0)
    c_bot = corr_merged((2,), bot_cls, (3, 0), (3, 3), 32)

    zeros_like_T = lambda Ts: [np.zeros_like(t) for t in Ts]

    # encoder weights: K=28 rows = 9 taps x 3 ch + bias row
    enc_w = np.asarray(d['enc_w'], np.float64)
    enc_b = np.asarray(d['enc_b'], np.float64)
    encw = np.zeros((28, C), np.float32)
    for ty in range(3):
        for tx in range(3):
            for ch in range(3):
                encw[(ty * 3 + tx) * 3 + ch, :] = enc_w[:, ch, ty, tx]
    encw[27, :] = enc_b

    # per-core im2col [28, NRF, NCF]
    inp = np.asarray(d['inp'], np.float64)[0]   # [3, H, W]
    PADX = 64
    ippad = np.pad(inp, ((0, 0), (PADX, PADX), (PADX, PADX)))
    ones = np.zeros((H + 2 * PADX, W + 2 * PADX))
    ones[PADX:PADX + H, PADX:PADX + W] = 1.0
    im2cols = []
    for core in range(NCORES):
        y0 = YLC * core + dy_min          # global LR row of f-tile row 0
        x0 = dx_min
        NR1 = NRF + 1
        im = np.zeros((28, NR1, NCF), np.float32)
        for ty in range(3):
            for tx in range(3):
                ys = PADX + y0 + ty - 1
                xs = PADX + x0 + tx - 1
                for ch in range(3):
                    im[(ty * 3 + tx) * 3 + ch] = \
                        ippad[ch, ys:ys + NR1, xs:xs + NCF]
        inside = ones[PADX + y0:PADX + y0 + NR1, PADX + x0:PADX + x0 + NCF]
        im[27] = inside
        # f must be exactly zero at out-of-image positions (grid-sample
        # zero padding), so kill whole columns there, not just oob taps
        im *= inside[None].astype(np.float32)
        im2cols.append(im.reshape(28, NR1 * NCF).astype(BF16))

    # query routing (f32 math matches reference rounding)
    coord = np.asarray(d['coord'], np.float32)[0]
    cell = np.asarray(d['cell'], np.float32)[0]
    cq = np.clip(coord - cell * np.float32(0.5) + np.float32(1e-6),
                 np.float32(-1 + 1e-6), np.float32(1 - 1e-6))
    xi = np.clip(np.round((cq[:, 1] + 1) * np.float32(0.5) * (WH - 1)
                          ).astype(np.int64), 0, WH - 1)
    yi = np.clip(np.round((cq[:, 0] + 1) * np.float32(0.5) * (HH - 1)
                          ).astype(np.int64), 0, HH - 1)
    core_of = yi // HRPC
    ylq = (yi % HRPC) // S
    bq = yi % S
    xlq = xi // S
    aq = xi % S
    cls_q = bq * S + aq
    grow = (ylq * W + xlq) * 4 + cls_q // 4           # D row, [0, NPIX//4)
    sub_q = cls_q % 4                                 # 3-float slot in row
    bank_q = ylq // 4                                 # psum bank / D quarter
    NROWS_D = NPIX // 4
    Q = coord.shape[0]
    # Bank-pipelined sorted block-gather: per (core, bank), sort queries by
    # D row, split into 128 groups; partition p block-fetches its span.
    per_cb = [[np.nonzero((core_of == core) & (bank_q == nb))[0]
               for nb in range(4)] for core in range(NCORES)]
    NQBP = 128 * ((max(s.size for row in per_cb for s in row) + 127) // 128)
    NQBP = max(NQBP, 128)
    idx_arrays, originals, subsels, localoff = [], [], [], []
    max_span = 1
    per = NQBP // 128
    for core in range(NCORES):
        lo4, org4, sub4, loc4 = [], [], [], []
        for nb in range(4):
            sel = per_cb[core][nb]
            rows = np.full(NQBP, 2048 * nb, np.int64)
            rows[:sel.size] = grow[sel]
            if sel.size:
                rows[sel.size:] = rows[:sel.size].max()
            order = np.argsort(rows[:sel.size], kind='stable')
            rows_sorted = np.concatenate([rows[:sel.size][order],
                                          rows[sel.size:]])
            lo = rows_sorted.reshape(128, per)[:, 0].copy()
            span = rows_sorted.reshape(128, per)[:, -1] - lo + 1
            max_span = max(max_span, int(span.max()))
            lo4.append(lo)
            org4.append(sel[order])
            sub4.append(sub_q[sel][order])
            loc4.append(rows_sorted - np.repeat(lo, per))
        idx_arrays.append(lo4)
        originals.append(org4)
        subsels.append(sub4)
        localoff.append(loc4)
    BLK = min(NROWS_D, ((max_span + 3) // 4) * 4)
    for core in range(NCORES):
        lo4 = idx_arrays[core]
        for nb in range(4):
            lo = lo4[nb] - 2048 * nb          # bank-relative
            lo2 = np.clip(np.minimum(lo, 2048 - BLK), 0, None)
            localoff[core][nb] = (localoff[core][nb]
                                  + np.repeat(lo - lo2, per)).astype(np.int64)
            assert (localoff[core][nb] < BLK).all()
            assert (localoff[core][nb] >= 0).all()
            lo4[nb] = lo2
        idx_arrays[core] = np.stack(
            [l.astype(np.int32) for l in lo4], axis=1)   # [128, 4]

    bias48 = np.zeros((1, MROWS), np.float32)
    for b in range(S):
        for a in range(S):
            bias48[0, (4 * b + a) * 3:(4 * b + a) * 3 + 3] = tail_b

    plan = dict(
        dy_min=dy_min, dx_min=dx_min, NRF=NRF, NCF=NCF, NF=NF,
        main_streams=main_streams,
        corr_specs=dict(top=c_top[0], bot=c_bot[0], lef=c_lef[0],
                        rig=c_rig[0]),
    )

    per_core_corr = []
    for core in range(NCORES):
        cc = dict(lef=c_lef[1], rig=c_rig[1])
        cc['top'] = c_top[1] if core == 0 else zeros_like_T(c_top[1])
        cc['bot'] = c_bot[1] if core == NCORES - 1 else zeros_like_T(c_bot[1])
        per_core_corr.append(cc)

    ones512 = np.ones((1, 512), np.float32)
    plan['BLK'] = BLK
    plan['NQBP'] = NQBP

    # ---- pack every small constant into one [128, CW] blob ----
    # segment name -> (p0, c0, K, Mw); K64 tensors pair up vertically
    segs = {}
    state = dict(col=0, half=None)

    def alloc(name, K, Mw):
        c0 = state['col']
        segs[name] = (0, c0, K, Mw)
        state['col'] += Mw
        return segs[name]

    alloc('encw', 28, C)
    alloc('bias48', MROWS, 1)
    for s, t in enumerate(main_T):
        alloc(f'Em{s}', t.shape[0], MROWS)
    for name, streams, mw in [('top', c_top[0], 128), ('bot', c_bot[0], 128),
                              ('lef', c_lef[0], MROWS),
                              ('rig', c_rig[0], MROWS)]:
        for s, (dl, paired) in enumerate(streams):
            alloc(f'Ec_{name}{s}', 128 if paired else C, mw)
    CW = state['col']
    plan['segs'] = segs
    plan['CW'] = CW

    consts_cores = []
    for core in range(NCORES):
        blob = np.zeros((128, CW), np.float32)

        def put(name, arr):
            p0, c0, K, Mw = segs[name]
            blob[p0:p0 + arr.shape[0], c0:c0 + arr.shape[1]] = arr

        put('encw', encw)
        put('bias48', bias48.reshape(MROWS, 1))
        for s, t in enumerate(main_T):
            put(f'Em{s}', t)
        for name, Ts in per_core_corr[core].items():
            for s, t in enumerate(Ts):
                put(f'Ec_{name}{s}', t)
        consts_cores.append(blob.astype(BF16))

    host = dict(consts=consts_cores, im2cols=im2cols,
                idx_arrays=idx_arrays,
                originals=originals, subsels=subsels, localoff=localoff,
                Q=Q)
    return plan, host


def _dma_gather_small_elem(nc, out_ap, in_ap, idxs_ap, num_idxs,
                           elem_size, elem_step, queue_num=0):
    """nc.gpsimd.dma_gather minus the 256-byte *element* restriction.

    The real hardware constraint is that the source ROW STRIDE
    (elem_step * dtype size) is a multiple of 256 bytes; the payload per
    index (elem_size) can be smaller.  Mirrors the non-transpose branch of
    BassGpSimd.dma_gather.
    """
    _ensure_path()
    import concourse.mybir as mybir
    from concourse import ap_utils

    gp = nc.gpsimd
    assert idxs_ap.dtype == mybir.dt.int16
    assert in_ap.dtype == out_ap.dtype
    assert ap_utils.ap_is_contiguous(in_ap.ap[1:])
    assert ap_utils.ap_is_contiguous(out_ap.ap[1:])
    assert ap_utils.ap_is_contiguous(idxs_ap.ap[1:])
    assert in_ap.ap[-1][1] == out_ap.ap[-1][1] == elem_size
    assert out_ap.ap[0][1] * out_ap.ap[1][1] == num_idxs
    assert in_ap.ap[0][0] == elem_step
    stride_bytes = elem_step * mybir.dt.size(in_ap.dtype)
    stride_bytes_256 = stride_bytes // 256
    assert stride_bytes_256 * 256 == stride_bytes and stride_bytes_256 < 256

    _in_ap = gp.lower_ap_dma(in_ap, for_custom_bir_dma=True)
    _idxs_ap = gp.lower_ap(idxs_ap)
    _out_ap = gp.lower_ap(out_ap)
    return gp.add_instruction(
        mybir.InstDMAGatherAnt(
            name=nc.get_next_instruction_name(),
            ins=[*_in_ap, _idxs_ap,
                 gp.lower_val_access(gp.to_reg(num_idxs))],
            outs=[_out_ap],
            transpose=False,
            num_idxs=num_idxs,
            elem_size=elem_size,
            stride_bytes_256=stride_bytes_256,
            gen_mode=0,
            single_packet=True,
            queue_num=queue_num,
            sbuf_tokens_per_rank=0,
            sbuf_free_dim_per_rank=0,
            sbuf_free_dim_pad_per_rank=0,
            sbuf_byte_offset=0,
        ))


def _build_graph(plan, host, debug_outputs=False):
    _ensure_path()
    import concourse.bass as bass
    import concourse.bacc as bacc
    import concourse.mybir as mybir
    import concourse.tile as tile
    from concourse.masks import make_identity

    f32 = mybir.dt.float32
    f32r = mybir.dt.float32r
    bf16 = mybir.dt.bfloat16
    i32 = mybir.dt.int32

    NRF, NCF, NF = plan['NRF'], plan['NCF'], plan['NF']
    dy_min, dx_min = plan['dy_min'], plan['dx_min']
    main_streams = plan['main_streams']
    corr_specs = plan['corr_specs']
    segs, CW = plan['segs'], plan['CW']
    BLK = plan['BLK']

    nc = bacc.Bacc(None, target_bir_lowering=False, debug=False,
                   num_devices=NCORES)

    im2col_d = nc.dram_tensor('im2col', [28, NF + NCF], bf16,
                              kind='ExternalInput')
    consts_d = nc.dram_tensor('consts', [128, CW], bf16,
                              kind='ExternalInput')
    idx_d = nc.dram_tensor('idx', [128, 4], i32, kind='ExternalInput')
    out_d = nc.dram_tensor('out', [128, 4 * BLK * 12], bf16,
                           kind='ExternalOutput')
    if debug_outputs:
        dbg_f2 = nc.dram_tensor('dbg_f2', [128, NF], f32, kind='ExternalOutput')
        dbg_pred = nc.dram_tensor('dbg_pred', [MROWS, YLC * W], f32,
                                  kind='ExternalOutput')
        dbg_D = nc.dram_tensor('dbg_D', [128, YLC * MROWS], f32,
                               kind='ExternalOutput')

    with tile.TileContext(nc) as tc:
        with (
            tc.tile_pool(name='sb', bufs=1) as sb,
            tc.tile_pool(name='sbsmall', bufs=1) as sbs,
            tc.tile_pool(name='pshare', bufs=2, space='PSUM') as pshare,
            tc.tile_pool(name='ppred', bufs=1, space='PSUM') as ppred,
            tc.tile_pool(name='pcorr', bufs=1, space='PSUM') as pcorr,
            tc.tile_pool(name='pcorr2', bufs=1, space='PSUM') as pcorr2,
            tc.tile_pool(name='dram', bufs=1, space='DRAM') as dpool,
        ):
            D_ts = [dpool.tile([NPIX // 16, 12], bf16, tag=f'D{nb}',
                               name=f'Dscr{nb}')
                    for nb in range(4)]
            consts_t = sb.tile([128, CW], bf16)
            # encoder weights first (first 64 cols) so the PE starts early
            nc.sync.dma_start(consts_t[0:28, 0:C], consts_d[0:28, 0:C])
            im2col = sb.tile([28, NF + NCF], bf16)
            nc.scalar.dma_start(im2col[:, 0:512], im2col_d[:, 0:512])
            nc.scalar.dma_start(im2col[:, 512:NF + NCF],
                                im2col_d[:, 512:NF + NCF])
            nc.sync.dma_start(consts_t[:, C:CW], consts_d[:, C:CW])

            idx_t = sbs.tile([128, 4], i32)
            nc.sync.dma_start(idx_t[:], idx_d[:])

            def cseg(name):
                p0, c0, K, Mw = segs[name]
                return consts_t[p0:p0 + K, c0:c0 + Mw]

            encw_t = cseg('encw')
            bias48_t = cseg('bias48')
            biasf_t = sbs.tile([MROWS, 1], f32)
            nc.vector.tensor_copy(biasf_t[:], bias48_t)
            mainT_t = [cseg(f'Em{s}') for s in range(len(main_streams))]
            corrT_t = {name: [cseg(f'Ec_{name}{s}')
                              for s in range(len(streams))]
                       for name, streams in corr_specs.items()}
            ident = sbs.tile([128, 128], bf16)
            make_identity(nc, ident[:])

            # encoder conv: f2 = [f ; f shifted one LR row], both halves
            # computed on the PE (second matmul reads a row-shifted rhs)
            f2 = sb.tile([128, NF], bf16)
            CH = 512
            nchunks = (NF + CH - 1) // CH
            for ci in range(nchunks):
                n0, n1 = ci * CH, min(NF, (ci + 1) * CH)
                pe_lo = pshare.tile([C, CH], f32, tag='pshare')
                nc.tensor.matmul(pe_lo[:, :n1 - n0],
                                 encw_t,
                                 im2col[:, n0:n1],
                                 start=True, stop=True,
                                 skip_group_check=True)
                nc.vector.tensor_copy(f2[0:C, n0:n1], pe_lo[:, :n1 - n0])
                pe_hi = pshare.tile([C, CH], f32, tag='pshare')
                nc.tensor.matmul(pe_hi[:, :n1 - n0],
                                 encw_t,
                                 im2col[:, n0 + NCF:n1 + NCF],
                                 start=True, stop=True,
                                 skip_group_check=True)
                nc.scalar.activation(f2[C:128, n0:n1], pe_hi[:, :n1 - n0],
                                     mybir.ActivationFunctionType.Copy)

            f3 = f2[:].rearrange('p (r c) -> p r c', c=NCF)

            # corrections first: they only need f2, and every bank's
            # merge depends on them
            # corr2_ps [128, top 128 | bot 128] (with corner windows)
            corr_ps = pcorr.tile([MROWS, 512], f32)
            corr2_ps = pcorr2.tile([128, 256], f32)

            def corr_mms(name, col0, row_sel, col_sel, nfree, ps):
                streams = corr_specs[name]
                if not streams:
                    return False
                for s, (dl, paired) in enumerate(streams):
                    K = 128 if paired else C
                    r0 = row_sel + dl[0] - dy_min
                    c0 = col_sel + dl[1] - dx_min
                    if nfree == 128:     # one row, all cols
                        rhs = f3[0:K, r0:r0 + 1, c0:c0 + W]
                    else:                # all rows, one col
                        rhs = f3[0:K, r0:r0 + YLC, c0:c0 + 1]
                    nc.tensor.matmul(
                        ps[:, col0:col0 + nfree],
                        corrT_t[name][s],
                        rhs,
                        start=(s == 0), stop=(s == len(streams) - 1),
                        skip_group_check=True)
                return True

            has = dict()
            has['top'] = corr_mms('top', 0, 0, 0, 128, corr2_ps)
            has['bot'] = corr_mms('bot', 128, YLC - 1, 0, 128, corr2_ps)
            has['lef'] = corr_mms('lef', 256, 0, 0, 16, corr_ps)
            has['rig'] = corr_mms('rig', 272, 0, W - 1, 16, corr_ps)


            # fused per-bank pipeline: main matmuls -> copy+merge ->
            # transpose -> D write -> gather -> out
            pred_ps = ppred.tile([MROWS, YLC * W], f32)
            pred_sb = sb.tile([MROWS, YLC * W], bf16)
            p4 = pred_sb[:].rearrange('p (r c) -> p r c', c=W)
            D_sb = sb.tile([128, YLC * MROWS], bf16)
            D4s = [D_ts[nb][:].rearrange('(yl xl g) k -> xl yl g k',
                                          yl=4, xl=128) for nb in range(4)]
            Ds4 = D_sb[:].rearrange('p (yl g c) -> p yl g c', yl=YLC, g=4)
            gath = sb.tile([128, 4 * BLK * 12], bf16)

            for nb in range(4):
                for s, (dl, paired) in enumerate(main_streams):
                    K = 128 if paired else C
                    r0 = 4 * nb + dl[0] - dy_min
                    c0 = dl[1] - dx_min
                    nc.tensor.matmul(
                        pred_ps[:, nb * 512:(nb + 1) * 512],
                        mainT_t[s],
                        f3[0:K, r0:r0 + 4, c0:c0 + W],
                        start=(s == 0), stop=(s == len(main_streams) - 1),
                        skip_group_check=True)
                nc.vector.tensor_scalar_add(
                    pred_sb[:, nb * 512:(nb + 1) * 512],
                    pred_ps[:, nb * 512:(nb + 1) * 512],
                    biasf_t[:])
                if nb == 0 and has['top']:
                    nc.vector.tensor_add(pred_sb[:, 0:W], pred_sb[:, 0:W],
                                         corr2_ps[0:MROWS, 0:W])
                if nb == 3 and has['bot']:
                    nc.vector.tensor_add(pred_sb[:, (YLC - 1) * W:YLC * W],
                                         pred_sb[:, (YLC - 1) * W:YLC * W],
                                         corr2_ps[0:MROWS, 128:128 + W])
                if has['lef']:
                    nc.vector.tensor_add(
                        p4[:, 4 * nb:4 * nb + 4, 0:1],
                        p4[:, 4 * nb:4 * nb + 4, 0:1],
                        corr_ps[:, 256 + 4 * nb:256 + 4 * nb + 4]
                        .rearrange('p (r c) -> p r c', c=1))
                if has['rig']:
                    nc.vector.tensor_add(
                        p4[:, 4 * nb:4 * nb + 4, W - 1:W],
                        p4[:, 4 * nb:4 * nb + 4, W - 1:W],
                        corr_ps[:, 272 + 4 * nb:272 + 4 * nb + 4]
                        .rearrange('p (r c) -> p r c', c=1))
                # corner add-backs (32-aligned windows; zero rows elsewhere)
                if nb == 0 and has['top']:
                    nc.vector.tensor_add(
                        p4[0:32, 0:1, 0:1], p4[0:32, 0:1, 0:1],
                        corr2_ps[64:96, 0:1]
                        .rearrange('p (r c) -> p r c', c=1))
                    nc.vector.tensor_add(
                        p4[0:32, 0:1, W - 1:W], p4[0:32, 0:1, W - 1:W],
                        corr2_ps[96:128, W - 1:W]
                        .rearrange('p (r c) -> p r c', c=1))
                    pass
                if nb == 3 and has['bot']:
                    nc.vector.tensor_add(
                        p4[32:48, YLC - 1:YLC, 0:1],
                        p4[32:48, YLC - 1:YLC, 0:1],
                        corr2_ps[64:80, 128:129]
                        .rearrange('p (r c) -> p r c', c=1))
                    nc.vector.tensor_add(
                        p4[32:48, YLC - 1:YLC, W - 1:W],
                        p4[32:48, YLC - 1:YLC, W - 1:W],
                        corr2_ps[96:112, 255:256]
                        .rearrange('p (r c) -> p r c', c=1))

                for ch in range(4 * nb, 4 * nb + 4):
                    pt = pshare.tile([128, MROWS], bf16, tag='pshare')
                    nc.tensor.transpose(pt[:],
                                        pred_sb[:, ch * W:(ch + 1) * W],
                                        ident[0:MROWS, 0:MROWS])
                    nc.scalar.activation(
                        D_sb[:, ch * MROWS:(ch + 1) * MROWS], pt[:],
                        mybir.ActivationFunctionType.Copy)
                nc.sync.dma_start(D4s[nb][:, :],
                                  Ds4[:, 4 * nb:4 * nb + 4])
                nc.gpsimd.indirect_dma_start(
                    out=gath[:, nb * BLK * 12:(nb + 1) * BLK * 12],
                    out_offset=None,
                    in_=D_ts[nb][:],
                    in_offset=bass.IndirectOffsetOnAxis(
                        ap=idx_t[:, nb:nb + 1], axis=0))
                nc.scalar.dma_start(
                    out_d[:, nb * BLK * 12:(nb + 1) * BLK * 12],
                    gath[:, nb * BLK * 12:(nb + 1) * BLK * 12])

            if debug_outputs:
                nc.sync.dma_start(dbg_f2[:], f2[:].bitcast(f32))
                nc.sync.dma_start(dbg_pred[:], pred_sb[:])
                nc.sync.dma_start(dbg_D[:], D_sb[:])

    nc.compile()
    return nc


def make_in_maps(host):
    in_maps = []
    for core in range(NCORES):
        m = {
            'im2col': host['im2cols'][core],
            'consts': host['consts'][core],
            'idx': host['idx_arrays'][core],
        }
        in_maps.append(m)
    return in_maps


def kernel(**inputs) -> np.ndarray:
    _ensure_path()
    from concourse.bass_utils import run_bass_kernel_spmd

    scale = inputs.get('scale', S)
    scale = int(np.asarray(scale)) if not isinstance(scale, int) else scale
    assert scale == S, f"kernel hardcodes scale={S}, got {scale}"

    plan, host = _plan_and_host_data(inputs)
    nc = _build_graph(plan, host)

    in_maps = make_in_maps(host)
    res = run_bass_kernel_spmd(nc, in_maps, core_ids=list(range(NCORES)))

    Q = host['Q']
    BLK = plan['BLK']
    NQBP = plan['NQBP']
    q = np.zeros((Q, 3), np.float32)
    per = NQBP // 128
    for core in range(NCORES):
        blocks = np.asarray(res.results[core]['out']).astype(
            np.float32).reshape(128, 4, BLK * 12)
        for nb in range(4):
            sel = host['originals'][core][nb]
            sub = host['subsels'][core][nb]
            loc = host['localoff'][core][nb]
            n = sel.size
            if n == 0:
                continue
            prt = (np.arange(n) // per)
            base = loc[:n] * 12 + sub * 3
            cols = base[:, None] + np.arange(3)[None]
            q[sel] = np.take_along_axis(blocks[prt, nb], cols, axis=1)
    return q[None]



# revision 7
# speedup vs baseline: 1.5353x; 1.5353x over previous
"""ArbSR (moe_routing) Trainium2 kernel, 8-core SPMD.

Structure exploited: with scale=4, the scale-embedding MLP input is periodic
with period 4 in both HR axes, so routing r, offsets off, and the expert-mix
matrices take only 16 distinct values (one per (y%4, x%4) class).  The
offset grid_sample then becomes, per class, a 2x2-tap bilinear filter of the
encoder feature map f at a constant integer shift, and the whole
  encoder conv -> fea0 -> expert mixing -> (+fea0) -> 3x3 tail conv
chain collapses, after folding the encoder INTO the per-delta matrices
(everything is linear in the input image), to
  pred[:, 4*yl+b, 4*xl+a] = tail_b
      + sum_{dy,dx} (E[(b,a)][(dy,dx)] @ encw) @ im2col[:, yl+dy, xl+dx]
with host-precomputed [48, 28] matrices per (dy, dx).  The dy axis is packed
into the contraction dim (K = 28*ndy <= 128) using a row-shift-stacked
im2col, so the device runs ONE matmul per (bank of 4 LR rows, dx):
12 K=84 matmuls of N=512 for the whole main computation.  Tail-conv zero
padding at the image border is handled with per-edge correction matmuls
(folded the same way) whose matrices are zeroed on cores that don't own the
edge.

Per core (64 HR rows): 12 main + ~10 correction matmuls; bias/edge merges on
DVE; PE transpose into a pixel-major layout D (rows keyed xl*16+t*4+g so a
D write is one contiguous 384B descriptor per partition); an indirect-DMA
block-gather for the nearest-neighbour queries, which the host routes to
cores/banks by row ownership and sorts so each of 128 partitions fetches one
contiguous span.
"""

import numpy as np
import ml_dtypes

BF16 = ml_dtypes.bfloat16


def _ensure_path():
    import sys
    for p in ('/opt/trn_rl_repo',):
        if p not in sys.path:
            sys.path.append(p)


H = W = 128
S = 4
HH = WH = H * S          # 512
C = 64
NCORES = 8
YLC = H // NCORES        # 16 LR rows per core
HRPC = HH // NCORES      # 64 HR rows per core
NPIX = HRPC * WH         # 32768 HR pixels per core
NCLS = 16                # (b, a) classes
MROWS = NCLS * 3         # 48 stacked pred rows
KE = 28                  # encoder-folded contraction rows per dy block
NROWS_B = 2048           # D rows per bank (128 xl * 4 t * 4 g)


def _sigmoid(x):
    return 1.0 / (1.0 + np.exp(-x))


def _class_constants(d):
    w1 = np.asarray(d['body_w1'], np.float64)
    b1 = np.asarray(d['body_b1'], np.float64)
    w2 = np.asarray(d['body_w2'], np.float64)
    b2 = np.asarray(d['body_b2'], np.float64)
    rw = np.asarray(d['routing_w'], np.float64)
    rb = np.asarray(d['routing_b'], np.float64)
    ow = np.asarray(d['offset_w'], np.float64)
    ob = np.asarray(d['offset_b'], np.float64)
    wc = np.asarray(d['weight_compress'], np.float64)
    we = np.asarray(d['weight_expand'], np.float64)

    fs = float(S)
    coor = np.array([(i + 0.5) / fs - np.floor((i + 0.5) / fs + 0.001) - 0.5
                     for i in range(S)])
    cls = {}
    for b in range(S):
        for a in range(S):
            inp4 = np.array([1.0 / fs, 1.0 / fs, coor[b], coor[a]])
            emb = np.maximum(w1 @ inp4 + b1, 0.0)
            emb = np.maximum(w2 @ emb + b2, 0.0)
            off = ow @ emb + ob
            r = _sigmoid(rw @ emb + rb)
            A = np.einsum('e,eck->ck', r, we) @ np.einsum('e,ekc->kc', r, wc)
            B = A + np.eye(C)
            cx = (a + 0.5) / fs - 0.5 + off[0]
            cy = (b + 0.5) / fs - 0.5 + off[1]
            ix, iy = int(np.floor(cx)), int(np.floor(cy))
            fx, fy = cx - ix, cy - iy
            wbl = {(0, 0): (1 - fy) * (1 - fx), (0, 1): (1 - fy) * fx,
                   (1, 0): fy * (1 - fx), (1, 1): fy * fx}
            cls[(b, a)] = dict(B=B, ix=ix, iy=iy, wbl=wbl)
    return cls


def _build_E(tail_w, cls, only_ty=None, only_tx=None):
    """E[(b,a)][(dy,dx)] = [3, C] so that pred contribution is E @ f(shift)."""
    Es = {}
    for b in range(S):
        for a in range(S):
            acc = {}
            for ty in range(3):
                if only_ty is not None and ty not in only_ty:
                    continue
                for tx in range(3):
                    if only_tx is not None and tx not in only_tx:
                        continue
                    bp = (b + ty - 1) % S
                    oy = (b + ty - 1 - bp) // S
                    ap_ = (a + tx - 1) % S
                    ox = (a + tx - 1 - ap_) // S
                    c2 = cls[(bp, ap_)]
                    TB = tail_w[:, :, ty, tx] @ c2['B']
                    for (uy, ux), wgt in c2['wbl'].items():
                        if wgt == 0.0:
                            continue
                        key = (oy + c2['iy'] + uy, ox + c2['ix'] + ux)
                        acc[key] = acc.get(key, np.zeros((3, C))) + TB * wgt
            Es[(b, a)] = acc
    return Es


def _build_encw(d):
    """encw [28, C]: rows = 9 taps x 3 ch + inside-mask bias row."""
    enc_w = np.asarray(d['enc_w'], np.float64)
    enc_b = np.asarray(d['enc_b'], np.float64)
    encw = np.zeros((KE, C))
    for ty in range(3):
        for tx in range(3):
            for ch in range(3):
                encw[(ty * 3 + tx) * 3 + ch, :] = enc_w[:, ch, ty, tx]
    encw[27, :] = enc_b
    return encw


def _plan_and_host_data(d):
    """Host precompute: folded lhsT matrices, per-core im2colK, query
    routing."""
    cls = _class_constants(d)
    tail_w = np.asarray(d['tail_w'], np.float64)
    tail_b = np.asarray(d['tail_b'], np.float64)
    encw = _build_encw(d)

    E_main = _build_E(tail_w, cls)
    deltas = sorted({k for acc in E_main.values() for k in acc})
    dys = sorted({dl[0] for dl in deltas})
    dxs = sorted({dl[1] for dl in deltas})
    dy_min, dy_max = min(dys), max(dys)
    dx_min, dx_max = min(dxs), max(dxs)
    ndy = dy_max - dy_min + 1
    ndx = dx_max - dx_min + 1
    KM = KE * ndy                    # main contraction rows
    assert KM <= 128, (ndy, KM)
    NRB = YLC + ndy - 1 + 1          # base im2col rows (1 slack row unused)
    NCF = W + ndx - 1                # im2col cols
    NFK = YLC * NCF                  # free size of the stacked im2colK

    def fold(G):
        """G [M, C] -> encw-folded lhsT block [28, M]."""
        return encw @ G.T

    # ---- main lhsT per dx: [KM, MROWS], block j=dy-dy_min ----
    G_main = {dl: np.zeros((MROWS, C)) for dl in deltas}
    for (b, a), acc in E_main.items():
        m0 = (4 * b + a) * 3
        for dl, M in acc.items():
            G_main[dl][m0:m0 + 3, :] += M
    main_T = []
    for dx in range(dx_min, dx_max + 1):
        lhsT = np.zeros((KM, MROWS))
        for dy in range(dy_min, dy_max + 1):
            if (dy, dx) in G_main:
                j = dy - dy_min
                lhsT[KE * j:KE * j + KE, :] = fold(G_main[(dy, dx)])
        main_T.append(lhsT)

    # ---- top/bot edge corrections (merged with corner add-backs) ----
    # lhsT cols 0:48 = negated edge contribution; cols 64:96 / 96:128 hold
    # the two corner add-backs at 32-aligned windows.
    def corr_merged(only_ty, edge_cls, cornerL, cornerR, cbase):
        E_edge = _build_E(tail_w, cls, only_ty=only_ty)
        E_cl = _build_E(tail_w, cls, only_ty=only_ty, only_tx=(0,))
        E_cr = _build_E(tail_w, cls, only_ty=only_ty, only_tx=(2,))
        dls = sorted({k for ba in edge_cls for k in E_edge[ba]})
        cdys = sorted({dl[0] for dl in dls})
        cdxs = sorted({dl[1] for dl in dls})
        G = {dl: np.zeros((128, C)) for dl in dls}
        for ba in edge_cls:
            m0 = (4 * ba[0] + ba[1]) * 3
            for dl, M in E_edge[ba].items():
                G[dl][m0:m0 + 3, :] -= M
        mL = (4 * cornerL[0] + cornerL[1]) * 3 - cbase
        for dl, M in E_cl[cornerL].items():
            G[dl][64 + mL:64 + mL + 3, :] += M
        mR = (4 * cornerR[0] + cornerR[1]) * 3 - cbase
        for dl, M in E_cr[cornerR].items():
            G[dl][96 + mR:96 + mR + 3, :] += M
        # per dx: lhsT [KM, 128]; blocks at KE*(dy-dy_min) like the mains
        out = []
        for dx in cdxs:
            lhsT = np.zeros((KM, 128))
            for dy in cdys:
                if (dy, dx) in G:
                    j = dy - dy_min
                    lhsT[KE * j:KE * j + KE, :] = fold(G[(dy, dx)])
            out.append(lhsT)
        return dict(T=out, dxs=cdxs)

    top_spec = corr_merged((0,), [(0, a) for a in range(S)], (0, 0), (0, 3), 0)
    bot_spec = corr_merged((2,), [(3, a) for a in range(S)], (3, 0), (3, 3),
                           32)

    # ---- lef/rig corrections: [KM, MROWS] per dx ----
    def corr_side(only_tx, side_cls):
        E_s = _build_E(tail_w, cls, only_tx=only_tx)
        dls = sorted({k for ba in side_cls for k in E_s[ba]})
        cdxs = sorted({dl[1] for dl in dls})
        G = {dl: np.zeros((MROWS, C)) for dl in dls}
        for ba in side_cls:
            m0 = (4 * ba[0] + ba[1]) * 3
            for dl, M in E_s[ba].items():
                G[dl][m0:m0 + 3, :] -= M
        out = []
        for dx in cdxs:
            lhsT = np.zeros((KM, MROWS))
            for dy in range(dy_min, dy_max + 1):
                if (dy, dx) in G:
                    j = dy - dy_min
                    lhsT[KE * j:KE * j + KE, :] = fold(G[(dy, dx)])
            out.append(lhsT)
        return dict(T=out, dxs=cdxs)

    lef_spec = corr_side((0,), [(b, 0) for b in range(S)])
    rig_spec = corr_side((2,), [(b, 3) for b in range(S)])

    # ---- per-core im2colK [KM, YLC*NCF] (dy-shift-stacked) ----
    inp = np.asarray(d['inp'], np.float64)[0]   # [3, H, W]
    PADX = 8
    ippad = np.pad(inp, ((0, 0), (PADX, PADX), (PADX, PADX)))
    ones = np.zeros((H + 2 * PADX, W + 2 * PADX))
    ones[PADX:PADX + H, PADX:PADX + W] = 1.0
    im2cols = []
    for core in range(NCORES):
        y0 = YLC * core + dy_min          # global LR row of base row 0
        x0 = dx_min
        base = np.zeros((KE, NRB, NCF))
        for ty in range(3):
            for tx in range(3):
                ys = PADX + y0 + ty - 1
                xs = PADX + x0 + tx - 1
                for ch in range(3):
                    base[(ty * 3 + tx) * 3 + ch] = \
                        ippad[ch, ys:ys + NRB, xs:xs + NCF]
        inside = ones[PADX + y0:PADX + y0 + NRB, PADX + x0:PADX + x0 + NCF]
        base[27] = inside
        # f must be exactly zero at out-of-image positions (grid-sample
        # zero padding), so kill whole columns there, not just oob taps
        base *= inside[None]
        imk = np.zeros((KM, YLC, NCF), np.float32)
        for j in range(ndy):
            imk[KE * j:KE * j + KE] = base[:, j:j + YLC, :]
        im2cols.append(imk.reshape(KM, NFK).astype(BF16))

    # ---- query routing (f32 math matches reference rounding) ----
    coord = np.asarray(d['coord'], np.float32)[0]
    cell = np.asarray(d['cell'], np.float32)[0]
    cq = np.clip(coord - cell * np.float32(0.5) + np.float32(1e-6),
                 np.float32(-1 + 1e-6), np.float32(1 - 1e-6))
    xi = np.clip(np.round((cq[:, 1] + 1) * np.float32(0.5) * (WH - 1)
                          ).astype(np.int64), 0, WH - 1)
    yi = np.clip(np.round((cq[:, 0] + 1) * np.float32(0.5) * (HH - 1)
                          ).astype(np.int64), 0, HH - 1)
    core_of = yi // HRPC
    ylq = (yi % HRPC) // S
    bq = yi % S
    xlq = xi // S
    aq = xi % S
    cls_q = bq * S + aq
    bank_q = ylq // 4
    # D row within a bank: xl*16 + t*4 + g (t = ylq%4, g = cls//4) so a
    # bank's D write is contiguous per partition xl
    grow = xlq * 16 + (ylq % 4) * 4 + cls_q // 4
    sub_q = cls_q % 4                                 # 3-float slot in row
    Q = coord.shape[0]
    # Bank-pipelined sorted block-gather: per (core, bank), sort queries by
    # D row, split into 128 groups; partition p block-fetches its span.
    per_cb = [[np.nonzero((core_of == core) & (bank_q == nb))[0]
               for nb in range(4)] for core in range(NCORES)]
    NQBP = 128 * ((max(s.size for row in per_cb for s in row) + 127) // 128)
    NQBP = max(NQBP, 128)
    idx_arrays, originals, subsels, localoff = [], [], [], []
    max_span = 1
    per = NQBP // 128
    for core in range(NCORES):
        lo4, org4, sub4, loc4 = [], [], [], []
        for nb in range(4):
            sel = per_cb[core][nb]
            rows = np.zeros(NQBP, np.int64)
            rows[:sel.size] = grow[sel]
            if sel.size:
                rows[sel.size:] = rows[:sel.size].max()
            order = np.argsort(rows[:sel.size], kind='stable')
            rows_sorted = np.concatenate([rows[:sel.size][order],
                                          rows[sel.size:]])
            lo = rows_sorted.reshape(128, per)[:, 0].copy()
            span = rows_sorted.reshape(128, per)[:, -1] - lo + 1
            max_span = max(max_span, int(span.max()))
            lo4.append(lo)
            org4.append(sel[order])
            sub4.append(sub_q[sel][order])
            loc4.append(rows_sorted - np.repeat(lo, per))
        idx_arrays.append(lo4)
        originals.append(org4)
        subsels.append(sub4)
        localoff.append(loc4)
    BLK = min(NROWS_B, ((max_span + 3) // 4) * 4)
    for core in range(NCORES):
        lo4 = idx_arrays[core]
        for nb in range(4):
            lo = lo4[nb]
            lo2 = np.clip(np.minimum(lo, NROWS_B - BLK), 0, None)
            localoff[core][nb] = (localoff[core][nb]
                                  + np.repeat(lo - lo2, per)).astype(np.int64)
            assert (localoff[core][nb] < BLK).all()
            assert (localoff[core][nb] >= 0).all()
            lo4[nb] = lo2
        idx_arrays[core] = np.stack(
            [l.astype(np.int32) for l in lo4], axis=1)   # [128, 4]

    bias48 = np.zeros(MROWS)
    for b in range(S):
        for a in range(S):
            bias48[(4 * b + a) * 3:(4 * b + a) * 3 + 3] = tail_b

    # ---- pack every constant into one [128, CW] blob ----
    # segment name -> (p0, c0, K, Mw)
    segs = {}
    col = [0]

    def alloc(name, K, Mw):
        segs[name] = (0, col[0], K, Mw)
        col[0] += Mw

    for i in range(ndx):
        alloc(f'Em{i}', KM, MROWS)
    for i in range(len(top_spec['dxs'])):
        alloc(f'Etop{i}', KM, 128)
    for i in range(len(bot_spec['dxs'])):
        alloc(f'Ebot{i}', KM, 128)
    for i in range(len(lef_spec['dxs'])):
        alloc(f'Elef{i}', KM, MROWS)
    for i in range(len(rig_spec['dxs'])):
        alloc(f'Erig{i}', KM, MROWS)
    alloc('bias48', MROWS, 1)
    CW = col[0]
    CW_MAIN = ndx * MROWS        # cols holding the main lhsT segs

    consts_cores = []
    for core in range(NCORES):
        blob = np.zeros((128, CW), np.float32)

        def put(name, arr):
            p0, c0, K, Mw = segs[name]
            assert arr.shape == (K, Mw), (name, arr.shape, (K, Mw))
            blob[p0:p0 + K, c0:c0 + Mw] = arr

        for i, t in enumerate(main_T):
            put(f'Em{i}', t)
        zt = np.zeros_like(top_spec['T'][0])
        zb = np.zeros_like(bot_spec['T'][0])
        for i in range(len(top_spec['dxs'])):
            put(f'Etop{i}', top_spec['T'][i] if core == 0 else zt)
        for i in range(len(bot_spec['dxs'])):
            put(f'Ebot{i}', bot_spec['T'][i] if core == NCORES - 1 else zb)
        for i, t in enumerate(lef_spec['T']):
            put(f'Elef{i}', t)
        for i, t in enumerate(rig_spec['T']):
            put(f'Erig{i}', t)
        put('bias48', bias48.reshape(MROWS, 1))
        consts_cores.append(blob.astype(BF16))

    plan = dict(
        dy_min=dy_min, dx_min=dx_min, ndy=ndy, ndx=ndx, KM=KM,
        NCF=NCF, NFK=NFK,
        top=dict(dxs=top_spec['dxs']), bot=dict(dxs=bot_spec['dxs']),
        lef=dict(dxs=lef_spec['dxs']), rig=dict(dxs=rig_spec['dxs']),
        segs=segs, CW=CW, CW_MAIN=CW_MAIN, BLK=BLK, NQBP=NQBP,
    )
    host = dict(consts=consts_cores, im2cols=im2cols,
                idx_arrays=idx_arrays,
                originals=originals, subsels=subsels, localoff=localoff,
                Q=Q)
    return plan, host


def _build_graph(plan, host, debug_outputs=False):
    _ensure_path()
    import concourse.bass as bass
    import concourse.bacc as bacc
    import concourse.mybir as mybir
    import concourse.tile as tile
    from concourse.masks import make_identity

    f32 = mybir.dt.float32
    bf16 = mybir.dt.bfloat16
    i32 = mybir.dt.int32

    KM, NCF, NFK = plan['KM'], plan['NCF'], plan['NFK']
    dy_min, dx_min, ndx = plan['dy_min'], plan['dx_min'], plan['ndx']
    segs, CW, CW_MAIN = plan['segs'], plan['CW'], plan['CW_MAIN']
    BLK = plan['BLK']

    nc = bacc.Bacc(None, target_bir_lowering=False, debug=False,
                   num_devices=NCORES)

    imk_d = nc.dram_tensor('im2col', [KM, NFK], bf16, kind='ExternalInput')
    consts_d = nc.dram_tensor('consts', [128, CW], bf16,
                              kind='ExternalInput')
    idx_d = nc.dram_tensor('idx', [128, 4], i32, kind='ExternalInput')
    out_d = nc.dram_tensor('out', [128, 4 * BLK * 12], bf16,
                           kind='ExternalOutput')
    if debug_outputs:
        dbg_pred = nc.dram_tensor('dbg_pred', [MROWS, YLC * W], f32,
                                  kind='ExternalOutput')
        dbg_D = nc.dram_tensor('dbg_D', [128, YLC * MROWS], f32,
                               kind='ExternalOutput')

    with tile.TileContext(nc) as tc:
        with (
            tc.tile_pool(name='sb', bufs=1) as sb,
            tc.tile_pool(name='sbsmall', bufs=1) as sbs,
            tc.tile_pool(name='pshare', bufs=2, space='PSUM') as pshare,
            tc.tile_pool(name='ppred', bufs=1, space='PSUM') as ppred,
            tc.tile_pool(name='pcorr', bufs=1, space='PSUM') as pcorr,
            tc.tile_pool(name='pcorr2', bufs=1, space='PSUM') as pcorr2,
            tc.tile_pool(name='dram', bufs=1, space='DRAM') as dpool,
        ):
            D_ts = [dpool.tile([NROWS_B, 12], bf16, tag=f'D{nb}',
                               name=f'Dscr{nb}')
                    for nb in range(4)]
            consts_t = sb.tile([128, CW], bf16)
            imk = sb.tile([KM, NFK], bf16)
            # row-group chunks: bank nb needs imk rows 4nb .. 4nb+3+ndy-1
            RA, RB = 6 * NCF, 11 * NCF
            nc.sync.dma_start(consts_t[:, 0:CW_MAIN], consts_d[:, 0:CW_MAIN])
            nc.scalar.dma_start(imk[:, 0:RA], imk_d[:, 0:RA])
            nc.sync.dma_start(consts_t[:, CW_MAIN:CW],
                              consts_d[:, CW_MAIN:CW])
            nc.scalar.dma_start(imk[:, RA:RB], imk_d[:, RA:RB])
            nc.scalar.dma_start(imk[:, RB:NFK], imk_d[:, RB:NFK])
            idx_t = sbs.tile([128, 4], i32)
            nc.sync.dma_start(idx_t[:], idx_d[:])

            def cseg(name):
                p0, c0, K, Mw = segs[name]
                return consts_t[p0:p0 + K, c0:c0 + Mw]

            biasf_t = sbs.tile([MROWS, 1], f32)
            nc.vector.tensor_copy(biasf_t[:], cseg('bias48'))
            ident = sbs.tile([MROWS, MROWS], bf16)
            make_identity(nc, ident[:])

            imk3 = imk[:].rearrange('p (r c) -> p r c', c=NCF)

            # ---- PE stream: mains bank 0, corrections, mains banks 1-3 ----
            pred_ps = ppred.tile([MROWS, YLC * W], f32)
            corr_ps = pcorr.tile([MROWS, 512], f32)
            corr2_ps = pcorr2.tile([128, 256], f32)

            def main_mms(nb):
                for i in range(ndx):
                    nc.tensor.matmul(
                        pred_ps[:, nb * 512:(nb + 1) * 512],
                        cseg(f'Em{i}'),
                        imk3[0:KM, 4 * nb:4 * nb + 4, i:i + W],
                        start=(i == 0), stop=(i == ndx - 1),
                        skip_group_check=True)

            def corr_tb(name, spec, col0, r):
                dxs = spec['dxs']
                for i, dx in enumerate(dxs):
                    c0 = dx - dx_min
                    nc.tensor.matmul(
                        corr2_ps[:, col0:col0 + 128],
                        cseg(f'E{name}{i}'),
                        imk3[0:KM, r:r + 1, c0:c0 + W],
                        start=(i == 0), stop=(i == len(dxs) - 1),
                        skip_group_check=True)

            def corr_side(name, spec, col0, xbase):
                dxs = spec['dxs']
                for i, dx in enumerate(dxs):
                    c0 = xbase + dx - dx_min
                    nc.tensor.matmul(
                        corr_ps[:, col0:col0 + 16],
                        cseg(f'E{name}{i}'),
                        imk3[0:KM, 0:YLC, c0:c0 + 1],
                        start=(i == 0), stop=(i == len(dxs) - 1),
                        skip_group_check=True)

            main_mms(0)
            corr_tb('top', plan['top'], 0, 0)
            corr_side('lef', plan['lef'], 0, 0)
            corr_side('rig', plan['rig'], 16, W - 1)
            corr_tb('bot', plan['bot'], 128, YLC - 1)
            for nb in range(1, 4):
                main_mms(nb)

            # ---- per-bank merge -> transpose -> D -> gather -> out ----
            pred_sb = sb.tile([MROWS, YLC * W], bf16)
            p4 = pred_sb[:].rearrange('p (r c) -> p r c', c=W)
            D_sb = sb.tile([128, YLC * MROWS], bf16)
            D2s = [D_ts[nb][:].rearrange('(xl r) k -> xl (r k)', xl=128)
                   for nb in range(4)]
            gath = sb.tile([128, 4 * BLK * 12], bf16)

            for nb in range(4):
                nc.vector.tensor_scalar_add(
                    pred_sb[:, nb * 512:(nb + 1) * 512],
                    pred_ps[:, nb * 512:(nb + 1) * 512],
                    biasf_t[:])
                if nb == 0:
                    nc.vector.tensor_add(pred_sb[:, 0:W], pred_sb[:, 0:W],
                                         corr2_ps[0:MROWS, 0:W])
                if nb == 3:
                    nc.vector.tensor_add(pred_sb[:, (YLC - 1) * W:YLC * W],
                                         pred_sb[:, (YLC - 1) * W:YLC * W],
                                         corr2_ps[0:MROWS, 128:128 + W])
                nc.vector.tensor_add(
                    p4[:, 4 * nb:4 * nb + 4, 0:1],
                    p4[:, 4 * nb:4 * nb + 4, 0:1],
                    corr_ps[:, 4 * nb:4 * nb + 4]
                    .rearrange('p (r c) -> p r c', c=1))
                nc.vector.tensor_add(
                    p4[:, 4 * nb:4 * nb + 4, W - 1:W],
                    p4[:, 4 * nb:4 * nb + 4, W - 1:W],
                    corr_ps[:, 16 + 4 * nb:16 + 4 * nb + 4]
                    .rearrange('p (r c) -> p r c', c=1))
                # corner add-backs (32-aligned windows; zero rows elsewhere)
                if nb == 0:
                    nc.vector.tensor_add(
                        p4[0:32, 0:1, 0:1], p4[0:32, 0:1, 0:1],
                        corr2_ps[64:96, 0:1]
                        .rearrange('p (r c) -> p r c', c=1))
                    nc.vector.tensor_add(
                        p4[0:32, 0:1, W - 1:W], p4[0:32, 0:1, W - 1:W],
                        corr2_ps[96:128, W - 1:W]
                        .rearrange('p (r c) -> p r c', c=1))
                if nb == 3:
                    nc.vector.tensor_add(
                        p4[32:48, YLC - 1:YLC, 0:1],
                        p4[32:48, YLC - 1:YLC, 0:1],
                        corr2_ps[64:80, 128:129]
                        .rearrange('p (r c) -> p r c', c=1))
                    nc.vector.tensor_add(
                        p4[32:48, YLC - 1:YLC, W - 1:W],
                        p4[32:48, YLC - 1:YLC, W - 1:W],
                        corr2_ps[96:112, 255:256]
                        .rearrange('p (r c) -> p r c', c=1))

                for ch in range(4 * nb, 4 * nb + 4):
                    pt = pshare.tile([128, MROWS], bf16, tag='pshare')
                    nc.tensor.transpose(pt[:],
                                        pred_sb[:, ch * W:(ch + 1) * W],
                                        ident[:])
                    nc.scalar.activation(
                        D_sb[:, ch * MROWS:(ch + 1) * MROWS], pt[:],
                        mybir.ActivationFunctionType.Copy)
                nc.sync.dma_start(D2s[nb][:, :],
                                  D_sb[:, nb * 192:(nb + 1) * 192])
                nc.gpsimd.indirect_dma_start(
                    out=gath[:, nb * BLK * 12:(nb + 1) * BLK * 12],
                    out_offset=None,
                    in_=D_ts[nb][:],
                    in_offset=bass.IndirectOffsetOnAxis(
                        ap=idx_t[:, nb:nb + 1], axis=0))
                nc.scalar.dma_start(
                    out_d[:, nb * BLK * 12:(nb + 1) * BLK * 12],
                    gath[:, nb * BLK * 12:(nb + 1) * BLK * 12])

            if debug_outputs:
                nc.sync.dma_start(dbg_pred[:], pred_sb[:])
                nc.sync.dma_start(dbg_D[:], D_sb[:])

    nc.compile()
    return nc


def make_in_maps(host):
    in_maps = []
    for core in range(NCORES):
        m = {
            'im2col': host['im2cols'][core],
            'consts': host['consts'][core],
            'idx': host['idx_arrays'][core],
        }
        in_maps.append(m)
    return in_maps


def kernel(**inputs) -> np.ndarray:
    _ensure_path()
    from concourse.bass_utils import run_bass_kernel_spmd

    scale = inputs.get('scale', S)
    scale = int(np.asarray(scale)) if not isinstance(scale, int) else scale
    assert scale == S, f"kernel hardcodes scale={S}, got {scale}"

    plan, host = _plan_and_host_data(inputs)
    nc = _build_graph(plan, host)

    in_maps = make_in_maps(host)
    res = run_bass_kernel_spmd(nc, in_maps, core_ids=list(range(NCORES)))

    Q = host['Q']
    BLK = plan['BLK']
    NQBP = plan['NQBP']
    q = np.zeros((Q, 3), np.float32)
    per = NQBP // 128
    for core in range(NCORES):
        blocks = np.asarray(res.results[core]['out']).astype(
            np.float32).reshape(128, 4, BLK * 12)
        for nb in range(4):
            sel = host['originals'][core][nb]
            sub = host['subsels'][core][nb]
            loc = host['localoff'][core][nb]
            n = sel.size
            if n == 0:
                continue
            prt = (np.arange(n) // per)
            base = loc[:n] * 12 + sub * 3
            cols = base[:, None] + np.arange(3)[None]
            q[sel] = np.take_along_axis(blocks[prt, nb], cols, axis=1)
    return q[None]


# revision 12
# speedup vs baseline: 1.7681x; 1.1516x over previous
"""ArbSR (moe_routing) Trainium2 kernel, 8-core SPMD.

Structure exploited: with scale=4, the scale-embedding MLP input is periodic
with period 4 in both HR axes, so routing r, offsets off, and the expert-mix
matrices take only 16 distinct values (one per (y%4, x%4) class).  The
offset grid_sample then becomes, per class, a 2x2-tap bilinear filter of the
encoder feature map f at a constant integer shift, and the whole
  encoder conv -> fea0 -> expert mixing -> (+fea0) -> 3x3 tail conv
chain collapses, after folding the encoder INTO the per-delta matrices
(everything is linear in the input image), to
  pred[:, 4*yl+b, 4*xl+a] = tail_b
      + sum_{dy,dx} (E[(b,a)][(dy,dx)] @ encw) @ im2col[:, yl+dy, xl+dx]
with host-precomputed [48, 28] matrices per (dy, dx).  The dy axis is packed
into the contraction dim (K = 28*ndy) using a row-shift-stacked im2col, so
the device runs ONE matmul per (bank of 4 LR rows, dx): 12 K~117 matmuls of
N=512 for the whole main computation.  The tail bias rides on an all-ones
rhs K row; the left/right tail-conv zero-pad corrections ride on one-hot
rhs K rows whose lhsT coefficients are host-computed exact (true - fold)
values; the top/bot row corrections (cores 0/7) are accumulated into PSUM
by an identity-lhsT matmul against a host-computed [48, W] strip.

Per core (64 HR rows): 14 matmuls + 16 PE transposes into a pixel-major
layout D (rows keyed xl*16+t*4+g so a D write is one contiguous 384B
descriptor per partition); an indirect-DMA block-gather for the
nearest-neighbour queries, which the host routes to cores/banks by row
ownership and sorts so each of 128 partitions fetches one contiguous span.
"""

import numpy as np
import ml_dtypes

BF16 = ml_dtypes.bfloat16


def _ensure_path():
    import sys
    for p in ('/opt/trn_rl_repo',):
        if p not in sys.path:
            sys.path.append(p)


H = W = 128
S = 4
HH = WH = H * S          # 512
C = 64
NCORES = 8
YLC = H // NCORES        # 16 LR rows per core
HRPC = HH // NCORES      # 64 HR rows per core
NPIX = HRPC * WH         # 32768 HR pixels per core
NCLS = 16                # (b, a) classes
MROWS = NCLS * 3         # 48 stacked pred rows
KE = 28                  # encoder-folded contraction rows per dy block
NROWS_B = 2048           # D rows per bank (128 xl * 4 t * 4 g)

GATHER = True            # False: dump D as output, gather on host
EDGE_IN_MAIN = True      # fold top/bot row corr into the main PSUM group
T_SHARED = True          # transposes share one PSUM tile per bank
SPLIT_DRAIN = True       # split PSUM drains between Vector and Scalar


def _sigmoid(x):
    return 1.0 / (1.0 + np.exp(-x))


def _class_constants(d):
    w1 = np.asarray(d['body_w1'], np.float64)
    b1 = np.asarray(d['body_b1'], np.float64)
    w2 = np.asarray(d['body_w2'], np.float64)
    b2 = np.asarray(d['body_b2'], np.float64)
    rw = np.asarray(d['routing_w'], np.float64)
    rb = np.asarray(d['routing_b'], np.float64)
    ow = np.asarray(d['offset_w'], np.float64)
    ob = np.asarray(d['offset_b'], np.float64)
    wc = np.asarray(d['weight_compress'], np.float64)
    we = np.asarray(d['weight_expand'], np.float64)

    fs = float(S)
    coor = np.array([(i + 0.5) / fs - np.floor((i + 0.5) / fs + 0.001) - 0.5
                     for i in range(S)])
    cls = {}
    for b in range(S):
        for a in range(S):
            inp4 = np.array([1.0 / fs, 1.0 / fs, coor[b], coor[a]])
            emb = np.maximum(w1 @ inp4 + b1, 0.0)
            emb = np.maximum(w2 @ emb + b2, 0.0)
            off = ow @ emb + ob
            r = _sigmoid(rw @ emb + rb)
            A = np.einsum('e,eck->ck', r, we) @ np.einsum('e,ekc->kc', r, wc)
            B = A + np.eye(C)
            cx = (a + 0.5) / fs - 0.5 + off[0]
            cy = (b + 0.5) / fs - 0.5 + off[1]
            ix, iy = int(np.floor(cx)), int(np.floor(cy))
            fx, fy = cx - ix, cy - iy
            wbl = {(0, 0): (1 - fy) * (1 - fx), (0, 1): (1 - fy) * fx,
                   (1, 0): fy * (1 - fx), (1, 1): fy * fx}
            cls[(b, a)] = dict(B=B, ix=ix, iy=iy, wbl=wbl)
    return cls


def _build_E(tail_w, cls):
    """E[(b,a)][(dy,dx)] = [3, C] so that pred contribution is E @ f(shift)."""
    Es = {}
    for b in range(S):
        for a in range(S):
            acc = {}
            for ty in range(3):
                for tx in range(3):
                    bp = (b + ty - 1) % S
                    oy = (b + ty - 1 - bp) // S
                    ap_ = (a + tx - 1) % S
                    ox = (a + tx - 1 - ap_) // S
                    c2 = cls[(bp, ap_)]
                    TB = tail_w[:, :, ty, tx] @ c2['B']
                    for (uy, ux), wgt in c2['wbl'].items():
                        if wgt == 0.0:
                            continue
                        key = (oy + c2['iy'] + uy, ox + c2['ix'] + ux)
                        acc[key] = acc.get(key, np.zeros((3, C))) + TB * wgt
            Es[(b, a)] = acc
    return Es


def _build_encw(d):
    """encw [28, C]: rows = 9 taps x 3 ch + inside-mask bias row."""
    enc_w = np.asarray(d['enc_w'], np.float64)
    enc_b = np.asarray(d['enc_b'], np.float64)
    encw = np.zeros((KE, C))
    for ty in range(3):
        for tx in range(3):
            for ch in range(3):
                encw[(ty * 3 + tx) * 3 + ch, :] = enc_w[:, ch, ty, tx]
    encw[27, :] = enc_b
    return encw


PADF = 4   # f64 f-map padding margin (covers all shift indexing)


def _host_f(d):
    """f64 encoder output, zero outside the image, with PADF margin."""
    inp = np.asarray(d['inp'], np.float64)[0]
    ip = np.pad(inp, ((0, 0), (1, 1), (1, 1)))
    enc_w = np.asarray(d['enc_w'], np.float64)
    enc_b = np.asarray(d['enc_b'], np.float64)
    f = np.zeros((C, H, W))
    for ty in range(3):
        for tx in range(3):
            f += np.einsum('oc,chw->ohw', enc_w[:, :, ty, tx],
                           ip[:, ty:ty + H, tx:tx + W])
    f += enc_b[:, None, None]
    fpad = np.zeros((C, H + 2 * PADF, W + 2 * PADF))
    fpad[:, PADF:PADF + H, PADF:PADF + W] = f
    return fpad


def _zgrid(cls, fpad, yHs, xHs):
    """z = out2 + fea0 (zero outside the HR image) on a coordinate grid."""
    yh = np.asarray(yHs)
    xh = np.asarray(xHs)
    out = np.zeros((C, len(yh), len(xh)))
    for b_ in range(S):
        rm = np.nonzero(np.mod(yh, S) == b_)[0]
        if rm.size == 0:
            continue
        ys = yh[rm]
        yl = ys // S
        for a_ in range(S):
            cm = np.nonzero(np.mod(xh, S) == a_)[0]
            if cm.size == 0:
                continue
            xs = xh[cm]
            xl = xs // S
            inside = ((ys[:, None] >= 0) & (ys[:, None] < HH)
                      & (xs[None, :] >= 0) & (xs[None, :] < WH))
            c2 = cls[(b_, a_)]
            fg = np.zeros((C, rm.size, cm.size))
            for (uy, ux), wgt in c2['wbl'].items():
                rr = PADF + yl + c2['iy'] + uy
                cc = PADF + xl + c2['ix'] + ux
                fg += wgt * fpad[:, rr[:, None], cc[None, :]]
            val = np.einsum('oc,cyx->oyx', c2['B'], fg)
            val *= inside[None]
            out[np.ix_(np.arange(C), rm, cm)] = val
    return out


def _true_strip(cls, fpad, tail_w, tail_b, ylgs, xs_lr):
    """Exact pred values [MROWS, len(ylgs), len(xs_lr)] (LR coords)."""
    ylgs = np.asarray(ylgs)
    xs_lr = np.asarray(xs_lr)
    yHs = np.arange(S * ylgs.min() - 1, S * ylgs.max() + S + 1)
    xHs = np.arange(S * xs_lr.min() - 1, S * xs_lr.max() + S + 1)
    z = _zgrid(cls, fpad, yHs, xHs)
    y0, x0 = yHs[0], xHs[0]
    out = np.zeros((MROWS, len(ylgs), len(xs_lr)))
    for b in range(S):
        for a in range(S):
            m0 = (4 * b + a) * 3
            acc = np.zeros((3, len(ylgs), len(xs_lr)))
            for ty in range(3):
                rr = S * ylgs + b + ty - 1 - y0
                for tx in range(3):
                    cc = S * xs_lr + a + tx - 1 - x0
                    acc += np.einsum(
                        'oc,cyx->oyx', tail_w[:, :, ty, tx],
                        z[:, rr[:, None], cc[None, :]])
            out[m0:m0 + 3] = acc + tail_b[:, None, None]
    return out


def _fold_strip(G_main, bias48, fpad, ylgs, xs_lr):
    """What the device mains+bias compute, in f64 (LR coords)."""
    ylgs = np.asarray(ylgs)
    xs_lr = np.asarray(xs_lr)
    out = np.zeros((MROWS, len(ylgs), len(xs_lr)))
    for (dy, dx), G in G_main.items():
        rr = PADF + ylgs + dy
        cc = PADF + xs_lr + dx
        out += np.einsum('mc,cyx->myx', G, fpad[:, rr[:, None], cc[None, :]])
    return out + bias48[:, None, None]


def _plan_and_host_data(d):
    """Host precompute: folded lhsT matrices, per-core im2colK, edge
    corrections, query routing."""
    cls = _class_constants(d)
    tail_w = np.asarray(d['tail_w'], np.float64)
    tail_b = np.asarray(d['tail_b'], np.float64)
    encw = _build_encw(d)

    E_main = _build_E(tail_w, cls)
    deltas = sorted({k for acc in E_main.values() for k in acc})
    dys = sorted({dl[0] for dl in deltas})
    dxs = sorted({dl[1] for dl in deltas})
    dy_min, dy_max = min(dys), max(dys)
    dx_min, dx_max = min(dxs), max(dxs)
    ndy = dy_max - dy_min + 1
    ndx = dx_max - dx_min + 1
    KA = KE * ndy                    # encoder-folded rows
    KM = KA + 1 + 2 * YLC            # + ones row + lef/rig one-hot rows
    assert KM <= 128, (ndy, KM)
    NRB = YLC + ndy                  # base im2col rows
    NCF = W + ndx - 1                # im2col cols
    NFK = YLC * NCF                  # free size of the stacked im2colK

    G_main = {dl: np.zeros((MROWS, C)) for dl in deltas}
    for (b, a), acc in E_main.items():
        m0 = (4 * b + a) * 3
        for dl, M in acc.items():
            G_main[dl][m0:m0 + 3, :] += M

    bias48 = np.zeros(MROWS)
    for b in range(S):
        for a in range(S):
            bias48[(4 * b + a) * 3:(4 * b + a) * 3 + 3] = tail_b

    # ---- exact edge corrections (true - fold), f64 on host ----
    fpad = _host_f(d)
    all_yl = np.arange(H)
    t_lef = _true_strip(cls, fpad, tail_w, tail_b, all_yl, [0])[:, :, 0]
    t_rig = _true_strip(cls, fpad, tail_w, tail_b, all_yl, [W - 1])[:, :, 0]
    f_lef = _fold_strip(G_main, bias48, fpad, all_yl, [0])[:, :, 0]
    f_rig = _fold_strip(G_main, bias48, fpad, all_yl, [W - 1])[:, :, 0]
    corrL = t_lef - f_lef            # [MROWS, H]
    corrR = t_rig - f_rig
    all_x = np.arange(W)
    t_top = _true_strip(cls, fpad, tail_w, tail_b, [0], all_x)[:, 0, :]
    f_top = _fold_strip(G_main, bias48, fpad, [0], all_x)[:, 0, :]
    corrT = t_top - f_top            # [MROWS, W]
    corrT[:, 0] -= corrL[:, 0]
    corrT[:, W - 1] -= corrR[:, 0]
    t_bot = _true_strip(cls, fpad, tail_w, tail_b, [H - 1], all_x)[:, 0, :]
    f_bot = _fold_strip(G_main, bias48, fpad, [H - 1], all_x)[:, 0, :]
    corrB = t_bot - f_bot
    corrB[:, 0] -= corrL[:, H - 1]
    corrB[:, W - 1] -= corrR[:, H - 1]

    # ---- main lhsT per dx: [KM, MROWS] ----
    def fold(G):
        return encw @ G.T

    main_T = []
    for dx in range(dx_min, dx_max + 1):
        lhsT = np.zeros((KM, MROWS))
        for dy in range(dy_min, dy_max + 1):
            if (dy, dx) in G_main:
                j = dy - dy_min
                lhsT[KE * j:KE * j + KE, :] = fold(G_main[(dy, dx)])
        main_T.append(lhsT)
    # bias + per-core lef/rig rows land in the dx=0 seg (c0 = -dx_min)

    # ---- per-core im2colK [KM, YLC*NCF] (dy-shift-stacked) ----
    inp = np.asarray(d['inp'], np.float64)[0]   # [3, H, W]
    PADX = 8
    ippad = np.pad(inp, ((0, 0), (PADX, PADX), (PADX, PADX)))
    ones = np.zeros((H + 2 * PADX, W + 2 * PADX))
    ones[PADX:PADX + H, PADX:PADX + W] = 1.0
    im2cols = []
    for core in range(NCORES):
        y0 = YLC * core + dy_min          # global LR row of base row 0
        x0 = dx_min
        base = np.zeros((KE, NRB, NCF))
        for ty in range(3):
            for tx in range(3):
                ys = PADX + y0 + ty - 1
                xs = PADX + x0 + tx - 1
                for ch in range(3):
                    base[(ty * 3 + tx) * 3 + ch] = \
                        ippad[ch, ys:ys + NRB, xs:xs + NCF]
        inside = ones[PADX + y0:PADX + y0 + NRB, PADX + x0:PADX + x0 + NCF]
        base[27] = inside
        # f must be exactly zero at out-of-image positions (grid-sample
        # zero padding), so kill whole columns there, not just oob taps
        base *= inside[None]
        imk = np.zeros((KM, YLC, NCF), np.float32)
        for j in range(ndy):
            imk[KE * j:KE * j + KE] = base[:, j:j + YLC, :]
        imk[KA] = 1.0                              # bias row
        for yl in range(YLC):                      # lef/rig one-hot rows
            imk[KA + 1 + yl, yl, -dx_min] = 1.0
            imk[KA + 1 + YLC + yl, yl, (W - 1) - dx_min] = 1.0
        im2cols.append(imk.reshape(KM, NFK).astype(BF16))

    # ---- query routing (f32 math matches reference rounding) ----
    coord = np.asarray(d['coord'], np.float32)[0]
    cell = np.asarray(d['cell'], np.float32)[0]
    cq = np.clip(coord - cell * np.float32(0.5) + np.float32(1e-6),
                 np.float32(-1 + 1e-6), np.float32(1 - 1e-6))
    xi = np.clip(np.round((cq[:, 1] + 1) * np.float32(0.5) * (WH - 1)
                          ).astype(np.int64), 0, WH - 1)
    yi = np.clip(np.round((cq[:, 0] + 1) * np.float32(0.5) * (HH - 1)
                          ).astype(np.int64), 0, HH - 1)
    core_of = yi // HRPC
    ylq = (yi % HRPC) // S
    bq = yi % S
    xlq = xi // S
    aq = xi % S
    cls_q = bq * S + aq
    bank_q = ylq // 4
    # D row within a bank: xl*16 + t*4 + g (t = ylq%4, g = cls//4) so a
    # bank's D write is contiguous per partition xl
    grow = xlq * 16 + (ylq % 4) * 4 + cls_q // 4
    sub_q = cls_q % 4                                 # 3-float slot in row
    Q = coord.shape[0]

    host = dict(consts=None, im2cols=im2cols, Q=Q)
    plan = dict(
        dy_min=dy_min, dx_min=dx_min, ndy=ndy, ndx=ndx, KM=KM, KA=KA,
        NCF=NCF, NFK=NFK,
    )

    if GATHER:
        # Bank-pipelined sorted block-gather: per (core, bank), sort
        # queries by D row, split into 128 groups; partition p
        # block-fetches its span.
        per_cb = [[np.nonzero((core_of == core) & (bank_q == nb))[0]
                   for nb in range(4)] for core in range(NCORES)]
        NQBP = 128 * ((max(s.size for row in per_cb for s in row) + 127)
                      // 128)
        NQBP = max(NQBP, 128)
        idx_arrays, originals, subsels, localoff = [], [], [], []
        max_span = 1
        per = NQBP // 128
        for core in range(NCORES):
            lo4, org4, sub4, loc4 = [], [], [], []
            for nb in range(4):
                sel = per_cb[core][nb]
                rows = np.zeros(NQBP, np.int64)
                rows[:sel.size] = grow[sel]
                if sel.size:
                    rows[sel.size:] = rows[:sel.size].max()
                order = np.argsort(rows[:sel.size], kind='stable')
                rows_sorted = np.concatenate([rows[:sel.size][order],
                                              rows[sel.size:]])
                lo = rows_sorted.reshape(128, per)[:, 0].copy()
                span = rows_sorted.reshape(128, per)[:, -1] - lo + 1
                max_span = max(max_span, int(span.max()))
                lo4.append(lo)
                org4.append(sel[order])
                sub4.append(sub_q[sel][order])
                loc4.append(rows_sorted - np.repeat(lo, per))
            idx_arrays.append(lo4)
            originals.append(org4)
            subsels.append(sub4)
            localoff.append(loc4)
        BLK = min(NROWS_B, ((max_span + 3) // 4) * 4)
        for core in range(NCORES):
            lo4 = idx_arrays[core]
            for nb in range(4):
                lo = lo4[nb]
                lo2 = np.clip(np.minimum(lo, NROWS_B - BLK), 0, None)
                localoff[core][nb] = (
                    localoff[core][nb]
                    + np.repeat(lo - lo2, per)).astype(np.int64)
                assert (localoff[core][nb] < BLK).all()
                assert (localoff[core][nb] >= 0).all()
                lo4[nb] = lo2
            idx_arrays[core] = np.stack(
                [l.astype(np.int32) for l in lo4], axis=1)   # [128, 4]
        plan['BLK'] = BLK
        plan['NQBP'] = NQBP
        host.update(idx_arrays=idx_arrays, originals=originals,
                    subsels=subsels, localoff=localoff)
    else:
        host.update(core_of=core_of, bank_q=bank_q, grow=grow, sub_q=sub_q)

    # ---- pack constants into one [128, CW] blob ----
    segs = {}
    col = [0]

    def alloc(name, K, Mw):
        segs[name] = (0, col[0], K, Mw)
        col[0] += Mw

    for i in range(ndx):
        alloc(f'Em{i}', KM, MROWS)
    alloc('corrT', MROWS, W)
    alloc('corrB', MROWS, W)
    CW = col[0]
    plan['segs'] = segs
    plan['CW'] = CW

    i_dx0 = -dx_min
    consts_cores = []
    for core in range(NCORES):
        blob = np.zeros((128, CW), np.float32)
        for i, t in enumerate(main_T):
            t = t.copy()
            if i == i_dx0:
                t[KA, :] = bias48
                yls = YLC * core + np.arange(YLC)
                t[KA + 1:KA + 1 + YLC, :] = corrL[:, yls].T
                t[KA + 1 + YLC:KM, :] = corrR[:, yls].T
            p0, c0, K, Mw = segs[f'Em{i}']
            blob[p0:p0 + K, c0:c0 + Mw] = t
        p0, c0, K, Mw = segs['corrT']
        if core == 0:
            blob[p0:p0 + K, c0:c0 + Mw] = corrT
        p0, c0, K, Mw = segs['corrB']
        if core == NCORES - 1:
            blob[p0:p0 + K, c0:c0 + Mw] = corrB
        consts_cores.append(blob.astype(BF16))
    host['consts'] = consts_cores
    return plan, host


def _build_graph(plan, host, debug_outputs=False):
    _ensure_path()
    import concourse.bass as bass
    import concourse.bacc as bacc
    import concourse.mybir as mybir
    import concourse.tile as tile
    from concourse.masks import make_identity

    f32 = mybir.dt.float32
    bf16 = mybir.dt.bfloat16
    i32 = mybir.dt.int32

    KM, NCF, NFK = plan['KM'], plan['NCF'], plan['NFK']
    dx_min, ndx = plan['dx_min'], plan['ndx']
    segs, CW = plan['segs'], plan['CW']

    nc = bacc.Bacc(None, target_bir_lowering=False, debug=False,
                   num_devices=NCORES)

    imk_d = nc.dram_tensor('im2col', [KM, NFK], bf16, kind='ExternalInput')
    consts_d = nc.dram_tensor('consts', [128, CW], bf16,
                              kind='ExternalInput')
    if GATHER:
        BLK = plan['BLK']
        idx_d = nc.dram_tensor('idx', [128, 4], i32, kind='ExternalInput')
        out_d = nc.dram_tensor('out', [128, 4 * BLK * 12], bf16,
                               kind='ExternalOutput')
    else:
        out_d = nc.dram_tensor('out', [128, YLC * MROWS], bf16,
                               kind='ExternalOutput')
    if debug_outputs:
        dbg_pred = nc.dram_tensor('dbg_pred', [MROWS, YLC * W], bf16,
                                  kind='ExternalOutput')

    with tile.TileContext(nc) as tc:
        with (
            tc.tile_pool(name='sb', bufs=1) as sb,
            tc.tile_pool(name='sbsmall', bufs=1) as sbs,
            tc.tile_pool(name='pshare', bufs=2, space='PSUM') as pshare,
            tc.tile_pool(name='ppred', bufs=1, space='PSUM') as ppred,
            tc.tile_pool(name='dram', bufs=1, space='DRAM') as dpool,
        ):
            consts_t = sb.tile([128, CW], bf16)
            imk = sb.tile([KM, NFK], bf16)
            # consts first (small; first LDWEIGHTS needs it), then imk in
            # 4 row-group chunks: bank nb reads rows 4nb..4nb+4 only
            nc.sync.dma_start(consts_t[:], consts_d[:])
            RC = [0, 4 * NCF, 8 * NCF, 12 * NCF, NFK]
            for ci in range(4):
                nc.sync.dma_start(imk[:, RC[ci]:RC[ci + 1]],
                                  imk_d[:, RC[ci]:RC[ci + 1]])
            if GATHER:
                idx_t = sbs.tile([128, 4], i32)
                nc.scalar.dma_start(idx_t[:], idx_d[:])
                D_ts = [dpool.tile([NROWS_B, 12], bf16, tag=f'D{nb}',
                                   name=f'Dscr{nb}')
                        for nb in range(4)]
                D2s = [D_ts[nb][:].rearrange('(xl r) k -> xl (r k)', xl=128)
                       for nb in range(4)]
                gath = sb.tile([128, 4 * BLK * 12], bf16)

            def cseg(name):
                p0, c0, K, Mw = segs[name]
                return consts_t[p0:p0 + K, c0:c0 + Mw]

            ident = sbs.tile([MROWS, MROWS], bf16)
            make_identity(nc, ident[:])

            imk3 = imk[:].rearrange('p (r c) -> p r c', c=NCF)
            pred_ps = ppred.tile([MROWS, YLC * W], f32)

            # ---- PE stream: 3 mains per bank (+ top/bot rows for banks
            # 0/3) accumulated into one PSUM bank per bank ----
            for nb in range(4):
                edge = EDGE_IN_MAIN and ((nb == 0) or (nb == 3))
                for i in range(ndx):
                    nc.tensor.matmul(
                        pred_ps[:, nb * 512:(nb + 1) * 512],
                        cseg(f'Em{i}'),
                        imk3[0:KM, 4 * nb:4 * nb + 4, i:i + W],
                        start=(i == 0), stop=(i == ndx - 1 and not edge),
                        skip_group_check=True)
                if edge and nb == 0:
                    nc.tensor.matmul(
                        pred_ps[:, 0:W], ident[:], cseg('corrT'),
                        start=False, stop=True, skip_group_check=True)
                if edge and nb == 3:
                    nc.tensor.matmul(
                        pred_ps[:, (YLC - 1) * W:YLC * W], ident[:],
                        cseg('corrB'),
                        start=False, stop=True, skip_group_check=True)

            # ---- per-bank drain -> transpose -> D copy -> D write ->
            # gather -> out ----
            pred_sb = sb.tile([MROWS, YLC * W], bf16)
            D_sb = sb.tile([128, YLC * MROWS], bf16)

            for nb in range(4):
                if SPLIT_DRAIN:
                    nc.vector.tensor_copy(
                        pred_sb[:, nb * 512:nb * 512 + 256],
                        pred_ps[:, nb * 512:nb * 512 + 256])
                    nc.scalar.activation(
                        pred_sb[:, nb * 512 + 256:(nb + 1) * 512],
                        pred_ps[:, nb * 512 + 256:(nb + 1) * 512],
                        mybir.ActivationFunctionType.Copy)
                else:
                    nc.vector.tensor_copy(
                        pred_sb[:, nb * 512:(nb + 1) * 512],
                        pred_ps[:, nb * 512:(nb + 1) * 512])
                if not EDGE_IN_MAIN and nb == 0:
                    nc.vector.tensor_add(pred_sb[:, 0:W], pred_sb[:, 0:W],
                                         cseg('corrT'))
                if not EDGE_IN_MAIN and nb == 3:
                    nc.vector.tensor_add(pred_sb[:, (YLC - 1) * W:YLC * W],
                                         pred_sb[:, (YLC - 1) * W:YLC * W],
                                         cseg('corrB'))
                if T_SHARED:
                    pt = pshare.tile([128, 4 * MROWS], bf16, tag='pshare')
                    for t in range(4):
                        ch = 4 * nb + t
                        nc.tensor.transpose(
                            pt[:, t * MROWS:(t + 1) * MROWS],
                            pred_sb[:, ch * W:(ch + 1) * W], ident[:])
                    nc.vector.tensor_copy(
                        D_sb[:, nb * 192:nb * 192 + 96], pt[:, 0:96])
                    nc.scalar.activation(
                        D_sb[:, nb * 192 + 96:(nb + 1) * 192],
                        pt[:, 96:192],
                        mybir.ActivationFunctionType.Copy)
                else:
                    for t in range(4):
                        ch = 4 * nb + t
                        pt = pshare.tile([128, MROWS], bf16, tag='pshare')
                        nc.tensor.transpose(
                            pt[:], pred_sb[:, ch * W:(ch + 1) * W],
                            ident[:])
                        nc.scalar.activation(
                            D_sb[:, ch * MROWS:(ch + 1) * MROWS], pt[:],
                            mybir.ActivationFunctionType.Copy)
                if GATHER:
                    nc.sync.dma_start(D2s[nb][:, :],
                                      D_sb[:, nb * 192:(nb + 1) * 192])
                    nc.gpsimd.indirect_dma_start(
                        out=gath[:, nb * BLK * 12:(nb + 1) * BLK * 12],
                        out_offset=None,
                        in_=D_ts[nb][:],
                        in_offset=bass.IndirectOffsetOnAxis(
                            ap=idx_t[:, nb:nb + 1], axis=0))
                    nc.scalar.dma_start(
                        out_d[:, nb * BLK * 12:(nb + 1) * BLK * 12],
                        gath[:, nb * BLK * 12:(nb + 1) * BLK * 12])
                else:
                    nc.sync.dma_start(
                        out_d[:, nb * 192:(nb + 1) * 192],
                        D_sb[:, nb * 192:(nb + 1) * 192])

            if debug_outputs:
                nc.sync.dma_start(dbg_pred[:], pred_sb[:])

    nc.compile()
    return nc


def make_in_maps(host):
    in_maps = []
    for core in range(NCORES):
        m = {
            'im2col': host['im2cols'][core],
            'consts': host['consts'][core],
        }
        if GATHER:
            m['idx'] = host['idx_arrays'][core]
        in_maps.append(m)
    return in_maps


def kernel(**inputs) -> np.ndarray:
    _ensure_path()
    from concourse.bass_utils import run_bass_kernel_spmd

    scale = inputs.get('scale', S)
    scale = int(np.asarray(scale)) if not isinstance(scale, int) else scale
    assert scale == S, f"kernel hardcodes scale={S}, got {scale}"

    plan, host = _plan_and_host_data(inputs)
    nc = _build_graph(plan, host)

    in_maps = make_in_maps(host)
    res = run_bass_kernel_spmd(nc, in_maps, core_ids=list(range(NCORES)))

    Q = host['Q']
    q = np.zeros((Q, 3), np.float32)
    if GATHER:
        BLK = plan['BLK']
        NQBP = plan['NQBP']
        per = NQBP // 128
        for core in range(NCORES):
            blocks = np.asarray(res.results[core]['out']).astype(
                np.float32).reshape(128, 4, BLK * 12)
            for nb in range(4):
                sel = host['originals'][core][nb]
                sub = host['subsels'][core][nb]
                loc = host['localoff'][core][nb]
                n = sel.size
                if n == 0:
                    continue
                prt = (np.arange(n) // per)
                base = loc[:n] * 12 + sub * 3
                cols = base[:, None] + np.arange(3)[None]
                q[sel] = np.take_along_axis(blocks[prt, nb], cols, axis=1)
    else:
        core_of, bank_q = host['core_of'], host['bank_q']
        grow, sub_q = host['grow'], host['sub_q']
        outs = np.stack([np.asarray(res.results[core]['out'])
                         for core in range(NCORES)]).astype(np.float32)
        # out[core][xl, nb*192 + (t*4+g)*12 + k]; grow = xl*16 + t*4 + g
        xl = grow // 16
        cols = bank_q * 192 + (grow % 16) * 12 + sub_q * 3
        for c in range(3):
            q[:, c] = outs[core_of, xl, cols + c]
    return q[None]


# revision 14
# speedup vs baseline: 1.8020x; 1.0192x over previous
"""ArbSR (moe_routing) Trainium2 kernel, 8-core SPMD.

Structure exploited: with scale=4, the scale-embedding MLP input is periodic
with period 4 in both HR axes, so routing r, offsets off, and the expert-mix
matrices take only 16 distinct values (one per (y%4, x%4) class).  The
offset grid_sample then becomes, per class, a 2x2-tap bilinear filter of the
encoder feature map f at a constant integer shift, and the whole
  encoder conv -> fea0 -> expert mixing -> (+fea0) -> 3x3 tail conv
chain collapses, after folding the encoder INTO the per-delta matrices
(everything is linear in the input image), to
  pred[:, 4*yl+b, 4*xl+a] = tail_b
      + sum_{dy,dx} (E[(b,a)][(dy,dx)] @ encw) @ im2col[:, yl+dy, xl+dx]
with host-precomputed [48, 28] matrices per (dy, dx).  The dy axis is packed
into the contraction dim (K = 28*ndy) using a row-shift-stacked im2col, so
the device runs ONE matmul per (bank of 4 LR rows, dx): 12 K~117 matmuls of
N=512 for the whole main computation.  The tail bias rides on an all-ones
rhs K row; the left/right tail-conv zero-pad corrections ride on one-hot
rhs K rows whose lhsT coefficients are host-computed exact (true - fold)
values; the top/bot row corrections (cores 0/7) are accumulated into PSUM
by an identity-lhsT matmul against a host-computed [48, W] strip.

Per core (64 HR rows): 14 matmuls + 16 PE transposes into a pixel-major
layout D (rows keyed xl*16+t*4+g so a D write is one contiguous 384B
descriptor per partition); an indirect-DMA block-gather for the
nearest-neighbour queries, which the host routes to cores/banks by row
ownership and sorts so each of 128 partitions fetches one contiguous span.
"""

import numpy as np
import ml_dtypes

BF16 = ml_dtypes.bfloat16


def _ensure_path():
    import sys
    for p in ('/opt/trn_rl_repo',):
        if p not in sys.path:
            sys.path.append(p)


H = W = 128
S = 4
HH = WH = H * S          # 512
C = 64
NCORES = 8
YLC = H // NCORES        # 16 LR rows per core
HRPC = HH // NCORES      # 64 HR rows per core
NPIX = HRPC * WH         # 32768 HR pixels per core
NCLS = 16                # (b, a) classes
MROWS = NCLS * 3         # 48 stacked pred rows
KE = 28                  # encoder-folded contraction rows per dy block
NROWS_B = 2048           # D rows per bank (128 xl * 4 t * 4 g)

GATHER = True            # False: dump D as output, gather on host
EDGE_IN_MAIN = True      # fold top/bot row corr into the main PSUM group
T_SHARED = True          # transposes share one PSUM tile per bank
SPLIT_DRAIN = True       # split PSUM drains between Vector and Scalar


def _sigmoid(x):
    return 1.0 / (1.0 + np.exp(-x))


def _class_constants(d):
    w1 = np.asarray(d['body_w1'], np.float64)
    b1 = np.asarray(d['body_b1'], np.float64)
    w2 = np.asarray(d['body_w2'], np.float64)
    b2 = np.asarray(d['body_b2'], np.float64)
    rw = np.asarray(d['routing_w'], np.float64)
    rb = np.asarray(d['routing_b'], np.float64)
    ow = np.asarray(d['offset_w'], np.float64)
    ob = np.asarray(d['offset_b'], np.float64)
    wc = np.asarray(d['weight_compress'], np.float64)
    we = np.asarray(d['weight_expand'], np.float64)

    fs = float(S)
    coor = np.array([(i + 0.5) / fs - np.floor((i + 0.5) / fs + 0.001) - 0.5
                     for i in range(S)])
    cls = {}
    for b in range(S):
        for a in range(S):
            inp4 = np.array([1.0 / fs, 1.0 / fs, coor[b], coor[a]])
            emb = np.maximum(w1 @ inp4 + b1, 0.0)
            emb = np.maximum(w2 @ emb + b2, 0.0)
            off = ow @ emb + ob
            r = _sigmoid(rw @ emb + rb)
            A = np.einsum('e,eck->ck', r, we) @ np.einsum('e,ekc->kc', r, wc)
            B = A + np.eye(C)
            cx = (a + 0.5) / fs - 0.5 + off[0]
            cy = (b + 0.5) / fs - 0.5 + off[1]
            ix, iy = int(np.floor(cx)), int(np.floor(cy))
            fx, fy = cx - ix, cy - iy
            wbl = {(0, 0): (1 - fy) * (1 - fx), (0, 1): (1 - fy) * fx,
                   (1, 0): fy * (1 - fx), (1, 1): fy * fx}
            cls[(b, a)] = dict(B=B, ix=ix, iy=iy, wbl=wbl)
    return cls


def _build_E(tail_w, cls):
    """E[(b,a)][(dy,dx)] = [3, C] so that pred contribution is E @ f(shift)."""
    Es = {}
    for b in range(S):
        for a in range(S):
            acc = {}
            for ty in range(3):
                for tx in range(3):
                    bp = (b + ty - 1) % S
                    oy = (b + ty - 1 - bp) // S
                    ap_ = (a + tx - 1) % S
                    ox = (a + tx - 1 - ap_) // S
                    c2 = cls[(bp, ap_)]
                    TB = tail_w[:, :, ty, tx] @ c2['B']
                    for (uy, ux), wgt in c2['wbl'].items():
                        if wgt == 0.0:
                            continue
                        key = (oy + c2['iy'] + uy, ox + c2['ix'] + ux)
                        acc[key] = acc.get(key, np.zeros((3, C))) + TB * wgt
            Es[(b, a)] = acc
    return Es


def _build_encw(d):
    """encw [28, C]: rows = 9 taps x 3 ch + inside-mask bias row."""
    enc_w = np.asarray(d['enc_w'], np.float64)
    enc_b = np.asarray(d['enc_b'], np.float64)
    encw = np.zeros((KE, C))
    for ty in range(3):
        for tx in range(3):
            for ch in range(3):
                encw[(ty * 3 + tx) * 3 + ch, :] = enc_w[:, ch, ty, tx]
    encw[27, :] = enc_b
    return encw


PADF = 4   # f64 f-map padding margin (covers all shift indexing)


def _host_f(d):
    """f64 encoder output, zero outside the image, with PADF margin."""
    inp = np.asarray(d['inp'], np.float64)[0]
    ip = np.pad(inp, ((0, 0), (1, 1), (1, 1)))
    enc_w = np.asarray(d['enc_w'], np.float64)
    enc_b = np.asarray(d['enc_b'], np.float64)
    f = np.zeros((C, H, W))
    for ty in range(3):
        for tx in range(3):
            f += np.einsum('oc,chw->ohw', enc_w[:, :, ty, tx],
                           ip[:, ty:ty + H, tx:tx + W])
    f += enc_b[:, None, None]
    fpad = np.zeros((C, H + 2 * PADF, W + 2 * PADF))
    fpad[:, PADF:PADF + H, PADF:PADF + W] = f
    return fpad


def _zgrid(cls, fpad, yHs, xHs):
    """z = out2 + fea0 (zero outside the HR image) on a coordinate grid."""
    yh = np.asarray(yHs)
    xh = np.asarray(xHs)
    out = np.zeros((C, len(yh), len(xh)))
    for b_ in range(S):
        rm = np.nonzero(np.mod(yh, S) == b_)[0]
        if rm.size == 0:
            continue
        ys = yh[rm]
        yl = ys // S
        for a_ in range(S):
            cm = np.nonzero(np.mod(xh, S) == a_)[0]
            if cm.size == 0:
                continue
            xs = xh[cm]
            xl = xs // S
            inside = ((ys[:, None] >= 0) & (ys[:, None] < HH)
                      & (xs[None, :] >= 0) & (xs[None, :] < WH))
            c2 = cls[(b_, a_)]
            fg = np.zeros((C, rm.size, cm.size))
            for (uy, ux), wgt in c2['wbl'].items():
                rr = PADF + yl + c2['iy'] + uy
                cc = PADF + xl + c2['ix'] + ux
                fg += wgt * fpad[:, rr[:, None], cc[None, :]]
            val = np.einsum('oc,cyx->oyx', c2['B'], fg)
            val *= inside[None]
            out[np.ix_(np.arange(C), rm, cm)] = val
    return out


def _true_strip(cls, fpad, tail_w, tail_b, ylgs, xs_lr):
    """Exact pred values [MROWS, len(ylgs), len(xs_lr)] (LR coords)."""
    ylgs = np.asarray(ylgs)
    xs_lr = np.asarray(xs_lr)
    yHs = np.arange(S * ylgs.min() - 1, S * ylgs.max() + S + 1)
    xHs = np.arange(S * xs_lr.min() - 1, S * xs_lr.max() + S + 1)
    z = _zgrid(cls, fpad, yHs, xHs)
    y0, x0 = yHs[0], xHs[0]
    out = np.zeros((MROWS, len(ylgs), len(xs_lr)))
    for b in range(S):
        for a in range(S):
            m0 = (4 * b + a) * 3
            acc = np.zeros((3, len(ylgs), len(xs_lr)))
            for ty in range(3):
                rr = S * ylgs + b + ty - 1 - y0
                for tx in range(3):
                    cc = S * xs_lr + a + tx - 1 - x0
                    acc += np.einsum(
                        'oc,cyx->oyx', tail_w[:, :, ty, tx],
                        z[:, rr[:, None], cc[None, :]])
            out[m0:m0 + 3] = acc + tail_b[:, None, None]
    return out


def _fold_strip(G_main, bias48, fpad, ylgs, xs_lr):
    """What the device mains+bias compute, in f64 (LR coords)."""
    ylgs = np.asarray(ylgs)
    xs_lr = np.asarray(xs_lr)
    out = np.zeros((MROWS, len(ylgs), len(xs_lr)))
    for (dy, dx), G in G_main.items():
        rr = PADF + ylgs + dy
        cc = PADF + xs_lr + dx
        out += np.einsum('mc,cyx->myx', G, fpad[:, rr[:, None], cc[None, :]])
    return out + bias48[:, None, None]


def _plan_and_host_data(d):
    """Host precompute: folded lhsT matrices, per-core im2colK, edge
    corrections, query routing."""
    cls = _class_constants(d)
    tail_w = np.asarray(d['tail_w'], np.float64)
    tail_b = np.asarray(d['tail_b'], np.float64)
    encw = _build_encw(d)

    E_main = _build_E(tail_w, cls)
    deltas = sorted({k for acc in E_main.values() for k in acc})
    dys = sorted({dl[0] for dl in deltas})
    dxs = sorted({dl[1] for dl in deltas})
    dy_min, dy_max = min(dys), max(dys)
    dx_min, dx_max = min(dxs), max(dxs)
    ndy = dy_max - dy_min + 1
    ndx = dx_max - dx_min + 1
    KA = KE * ndy                    # encoder-folded rows
    KM = KA + 1 + 2 * YLC            # + ones row + lef/rig one-hot rows
    assert KM <= 128, (ndy, KM)
    NRB = YLC + ndy                  # base im2col rows
    NCF = W + ndx - 1                # im2col cols
    NFK = YLC * NCF                  # free size of the stacked im2colK

    G_main = {dl: np.zeros((MROWS, C)) for dl in deltas}
    for (b, a), acc in E_main.items():
        m0 = (4 * b + a) * 3
        for dl, M in acc.items():
            G_main[dl][m0:m0 + 3, :] += M

    bias48 = np.zeros(MROWS)
    for b in range(S):
        for a in range(S):
            bias48[(4 * b + a) * 3:(4 * b + a) * 3 + 3] = tail_b

    # ---- exact edge corrections (true - fold), f64 on host ----
    fpad = _host_f(d)
    all_yl = np.arange(H)
    t_lef = _true_strip(cls, fpad, tail_w, tail_b, all_yl, [0])[:, :, 0]
    t_rig = _true_strip(cls, fpad, tail_w, tail_b, all_yl, [W - 1])[:, :, 0]
    f_lef = _fold_strip(G_main, bias48, fpad, all_yl, [0])[:, :, 0]
    f_rig = _fold_strip(G_main, bias48, fpad, all_yl, [W - 1])[:, :, 0]
    corrL = t_lef - f_lef            # [MROWS, H]
    corrR = t_rig - f_rig
    all_x = np.arange(W)
    t_top = _true_strip(cls, fpad, tail_w, tail_b, [0], all_x)[:, 0, :]
    f_top = _fold_strip(G_main, bias48, fpad, [0], all_x)[:, 0, :]
    corrT = t_top - f_top            # [MROWS, W]
    corrT[:, 0] -= corrL[:, 0]
    corrT[:, W - 1] -= corrR[:, 0]
    t_bot = _true_strip(cls, fpad, tail_w, tail_b, [H - 1], all_x)[:, 0, :]
    f_bot = _fold_strip(G_main, bias48, fpad, [H - 1], all_x)[:, 0, :]
    corrB = t_bot - f_bot
    corrB[:, 0] -= corrL[:, H - 1]
    corrB[:, W - 1] -= corrR[:, H - 1]

    # ---- main lhsT per dx: [KM, MROWS] ----
    def fold(G):
        return encw @ G.T

    main_T = []
    for dx in range(dx_min, dx_max + 1):
        lhsT = np.zeros((KM, MROWS))
        for dy in range(dy_min, dy_max + 1):
            if (dy, dx) in G_main:
                j = dy - dy_min
                lhsT[KE * j:KE * j + KE, :] = fold(G_main[(dy, dx)])
        main_T.append(lhsT)
    # bias + per-core lef/rig rows land in the dx=0 seg (c0 = -dx_min)

    # ---- per-core im2colK [KM, YLC*NCF] (dy-shift-stacked) ----
    inp = np.asarray(d['inp'], np.float64)[0]   # [3, H, W]
    PADX = 8
    ippad = np.pad(inp, ((0, 0), (PADX, PADX), (PADX, PADX)))
    ones = np.zeros((H + 2 * PADX, W + 2 * PADX))
    ones[PADX:PADX + H, PADX:PADX + W] = 1.0
    im2cols = []
    for core in range(NCORES):
        y0 = YLC * core + dy_min          # global LR row of base row 0
        x0 = dx_min
        base = np.zeros((KE, NRB, NCF))
        for ty in range(3):
            for tx in range(3):
                ys = PADX + y0 + ty - 1
                xs = PADX + x0 + tx - 1
                for ch in range(3):
                    base[(ty * 3 + tx) * 3 + ch] = \
                        ippad[ch, ys:ys + NRB, xs:xs + NCF]
        inside = ones[PADX + y0:PADX + y0 + NRB, PADX + x0:PADX + x0 + NCF]
        base[27] = inside
        # f must be exactly zero at out-of-image positions (grid-sample
        # zero padding), so kill whole columns there, not just oob taps
        base *= inside[None]
        imk = np.zeros((KM, YLC, NCF), np.float32)
        for j in range(ndy):
            imk[KE * j:KE * j + KE] = base[:, j:j + YLC, :]
        imk[KA] = 1.0                              # bias row
        for yl in range(YLC):                      # lef/rig one-hot rows
            imk[KA + 1 + yl, yl, -dx_min] = 1.0
            imk[KA + 1 + YLC + yl, yl, (W - 1) - dx_min] = 1.0
        im2cols.append(imk.reshape(KM, NFK).astype(BF16))

    # ---- query routing (f32 math matches reference rounding) ----
    coord = np.asarray(d['coord'], np.float32)[0]
    cell = np.asarray(d['cell'], np.float32)[0]
    cq = np.clip(coord - cell * np.float32(0.5) + np.float32(1e-6),
                 np.float32(-1 + 1e-6), np.float32(1 - 1e-6))
    xi = np.clip(np.round((cq[:, 1] + 1) * np.float32(0.5) * (WH - 1)
                          ).astype(np.int64), 0, WH - 1)
    yi = np.clip(np.round((cq[:, 0] + 1) * np.float32(0.5) * (HH - 1)
                          ).astype(np.int64), 0, HH - 1)
    core_of = yi // HRPC
    ylq = (yi % HRPC) // S
    bq = yi % S
    xlq = xi // S
    aq = xi % S
    cls_q = bq * S + aq
    bank_q = ylq // 4
    # D row within a bank: xl*16 + t*4 + g (t = ylq%4, g = cls//4) so a
    # bank's D write is contiguous per partition xl
    grow = xlq * 16 + (ylq % 4) * 4 + cls_q // 4
    sub_q = cls_q % 4                                 # 3-float slot in row
    Q = coord.shape[0]

    host = dict(consts=None, im2cols=im2cols, Q=Q)
    plan = dict(
        dy_min=dy_min, dx_min=dx_min, ndy=ndy, ndx=ndx, KM=KM, KA=KA,
        NCF=NCF, NFK=NFK,
    )

    if GATHER:
        # Bank-pipelined sorted block-gather: per (core, bank), sort
        # queries by D row, split into 128 groups; partition p
        # block-fetches its span.
        per_cb = [[np.nonzero((core_of == core) & (bank_q == nb))[0]
                   for nb in range(4)] for core in range(NCORES)]
        NQBP = 128 * ((max(s.size for row in per_cb for s in row) + 127)
                      // 128)
        NQBP = max(NQBP, 128)
        idx_arrays, originals, subsels, localoff = [], [], [], []
        max_span = 1
        per = NQBP // 128
        for core in range(NCORES):
            lo4, org4, sub4, loc4 = [], [], [], []
            for nb in range(4):
                sel = per_cb[core][nb]
                rows = np.zeros(NQBP, np.int64)
                rows[:sel.size] = grow[sel]
                if sel.size:
                    rows[sel.size:] = rows[:sel.size].max()
                order = np.argsort(rows[:sel.size], kind='stable')
                rows_sorted = np.concatenate([rows[:sel.size][order],
                                              rows[sel.size:]])
                lo = rows_sorted.reshape(128, per)[:, 0].copy()
                span = rows_sorted.reshape(128, per)[:, -1] - lo + 1
                max_span = max(max_span, int(span.max()))
                lo4.append(lo)
                org4.append(sel[order])
                sub4.append(sub_q[sel][order])
                loc4.append(rows_sorted - np.repeat(lo, per))
            idx_arrays.append(lo4)
            originals.append(org4)
            subsels.append(sub4)
            localoff.append(loc4)
        BLK = min(NROWS_B, ((max_span + 3) // 4) * 4)
        for core in range(NCORES):
            lo4 = idx_arrays[core]
            for nb in range(4):
                lo = lo4[nb]
                lo2 = np.clip(np.minimum(lo, NROWS_B - BLK), 0, None)
                localoff[core][nb] = (
                    localoff[core][nb]
                    + np.repeat(lo - lo2, per)).astype(np.int64)
                assert (localoff[core][nb] < BLK).all()
                assert (localoff[core][nb] >= 0).all()
                lo4[nb] = lo2
            idx_arrays[core] = np.stack(
                [l.astype(np.int32) for l in lo4], axis=1)   # [128, 4]
        plan['BLK'] = BLK
        plan['NQBP'] = NQBP
        host.update(idx_arrays=idx_arrays, originals=originals,
                    subsels=subsels, localoff=localoff)
    else:
        host.update(core_of=core_of, bank_q=bank_q, grow=grow, sub_q=sub_q)

    # ---- pack constants into one [128, CW] blob ----
    segs = {}
    col = [0]

    def alloc(name, K, Mw):
        segs[name] = (0, col[0], K, Mw)
        col[0] += Mw

    for i in range(ndx):
        alloc(f'Em{i}', KM, MROWS)
    alloc('corrT', MROWS, W)
    alloc('corrB', MROWS, W)
    CW = col[0]
    plan['segs'] = segs
    plan['CW'] = CW

    i_dx0 = -dx_min
    consts_cores = []
    for core in range(NCORES):
        blob = np.zeros((128, CW), np.float32)
        for i, t in enumerate(main_T):
            t = t.copy()
            if i == i_dx0:
                t[KA, :] = bias48
                yls = YLC * core + np.arange(YLC)
                t[KA + 1:KA + 1 + YLC, :] = corrL[:, yls].T
                t[KA + 1 + YLC:KM, :] = corrR[:, yls].T
            p0, c0, K, Mw = segs[f'Em{i}']
            blob[p0:p0 + K, c0:c0 + Mw] = t
        p0, c0, K, Mw = segs['corrT']
        if core == 0:
            blob[p0:p0 + K, c0:c0 + Mw] = corrT
        p0, c0, K, Mw = segs['corrB']
        if core == NCORES - 1:
            blob[p0:p0 + K, c0:c0 + Mw] = corrB
        consts_cores.append(blob.astype(BF16))
    host['consts'] = consts_cores
    return plan, host


def _build_graph(plan, host, debug_outputs=False):
    _ensure_path()
    import concourse.bass as bass
    import concourse.bacc as bacc
    import concourse.mybir as mybir
    import concourse.tile as tile
    from concourse.masks import make_identity

    f32 = mybir.dt.float32
    bf16 = mybir.dt.bfloat16
    i32 = mybir.dt.int32

    KM, NCF, NFK = plan['KM'], plan['NCF'], plan['NFK']
    dx_min, ndx = plan['dx_min'], plan['ndx']
    segs, CW = plan['segs'], plan['CW']

    nc = bacc.Bacc(None, target_bir_lowering=False, debug=False,
                   num_devices=NCORES)

    imk_d = nc.dram_tensor('im2col', [KM, NFK], bf16, kind='ExternalInput')
    consts_d = nc.dram_tensor('consts', [128, CW], bf16,
                              kind='ExternalInput')
    if GATHER:
        BLK = plan['BLK']
        idx_d = nc.dram_tensor('idx', [128, 4], i32, kind='ExternalInput')
        out_d = nc.dram_tensor('out', [128, 4 * BLK * 12], bf16,
                               kind='ExternalOutput')
    else:
        out_d = nc.dram_tensor('out', [128, YLC * MROWS], bf16,
                               kind='ExternalOutput')
    if debug_outputs:
        dbg_pred = nc.dram_tensor('dbg_pred', [MROWS, YLC * W], bf16,
                                  kind='ExternalOutput')

    with tile.TileContext(nc) as tc:
        with (
            tc.tile_pool(name='sb', bufs=1) as sb,
            tc.tile_pool(name='sbsmall', bufs=1) as sbs,
            tc.tile_pool(name='pshare', bufs=2, space='PSUM') as pshare,
            tc.tile_pool(name='ppred', bufs=1, space='PSUM') as ppred,
            tc.tile_pool(name='dram', bufs=1, space='DRAM') as dpool,
        ):
            consts_t = sb.tile([128, CW], bf16)
            imk = sb.tile([KM, NFK], bf16)
            # consts first (small; first LDWEIGHTS needs it); imk in 4
            # row-group chunks (bank nb reads rows 4nb..4nb+4 only),
            # spread across engine queues so the ~0.8us descriptor
            # generation per DMA runs in parallel
            RC = [0, 4 * NCF, 8 * NCF, 12 * NCF, NFK]
            nc.scalar.dma_start(imk[:, RC[0]:RC[1]], imk_d[:, RC[0]:RC[1]])
            nc.sync.dma_start(consts_t[:], consts_d[:])
            nc.sync.dma_start(imk[:, RC[1]:RC[2]], imk_d[:, RC[1]:RC[2]])
            nc.scalar.dma_start(imk[:, RC[2]:RC[3]], imk_d[:, RC[2]:RC[3]])
            nc.sync.dma_start(imk[:, RC[3]:RC[4]], imk_d[:, RC[3]:RC[4]])
            if GATHER:
                idx_t = sbs.tile([128, 4], i32)
                nc.scalar.dma_start(idx_t[:], idx_d[:])
                D_ts = [dpool.tile([NROWS_B, 12], bf16, tag=f'D{nb}',
                                   name=f'Dscr{nb}')
                        for nb in range(4)]
                D2s = [D_ts[nb][:].rearrange('(xl r) k -> xl (r k)', xl=128)
                       for nb in range(4)]
                gath = sb.tile([128, 4 * BLK * 12], bf16)

            def cseg(name):
                p0, c0, K, Mw = segs[name]
                return consts_t[p0:p0 + K, c0:c0 + Mw]

            ident = sbs.tile([MROWS, MROWS], bf16)
            make_identity(nc, ident[:])

            imk3 = imk[:].rearrange('p (r c) -> p r c', c=NCF)
            pred_ps = ppred.tile([MROWS, YLC * W], f32)

            # ---- PE stream: 3 mains per bank (+ top/bot rows for banks
            # 0/3) accumulated into one PSUM bank per bank ----
            for nb in range(4):
                edge = EDGE_IN_MAIN and ((nb == 0) or (nb == 3))
                for i in range(ndx):
                    nc.tensor.matmul(
                        pred_ps[:, nb * 512:(nb + 1) * 512],
                        cseg(f'Em{i}'),
                        imk3[0:KM, 4 * nb:4 * nb + 4, i:i + W],
                        start=(i == 0), stop=(i == ndx - 1 and not edge),
                        skip_group_check=True)
                if edge and nb == 0:
                    nc.tensor.matmul(
                        pred_ps[:, 0:W], ident[:], cseg('corrT'),
                        start=False, stop=True, skip_group_check=True)
                if edge and nb == 3:
                    nc.tensor.matmul(
                        pred_ps[:, (YLC - 1) * W:YLC * W], ident[:],
                        cseg('corrB'),
                        start=False, stop=True, skip_group_check=True)

            # ---- per-bank drain -> transpose -> D copy -> D write ->
            # gather -> out ----
            pred_sb = sb.tile([MROWS, YLC * W], bf16)
            D_sb = sb.tile([128, YLC * MROWS], bf16)

            for nb in range(4):
                if SPLIT_DRAIN:
                    nc.vector.tensor_copy(
                        pred_sb[:, nb * 512:nb * 512 + 256],
                        pred_ps[:, nb * 512:nb * 512 + 256])
                    nc.scalar.activation(
                        pred_sb[:, nb * 512 + 256:(nb + 1) * 512],
                        pred_ps[:, nb * 512 + 256:(nb + 1) * 512],
                        mybir.ActivationFunctionType.Copy)
                else:
                    nc.vector.tensor_copy(
                        pred_sb[:, nb * 512:(nb + 1) * 512],
                        pred_ps[:, nb * 512:(nb + 1) * 512])
                if not EDGE_IN_MAIN and nb == 0:
                    nc.vector.tensor_add(pred_sb[:, 0:W], pred_sb[:, 0:W],
                                         cseg('corrT'))
                if not EDGE_IN_MAIN and nb == 3:
                    nc.vector.tensor_add(pred_sb[:, (YLC - 1) * W:YLC * W],
                                         pred_sb[:, (YLC - 1) * W:YLC * W],
                                         cseg('corrB'))
                if T_SHARED:
                    pt = pshare.tile([128, 4 * MROWS], bf16, tag='pshare')
                    for t in range(4):
                        ch = 4 * nb + t
                        nc.tensor.transpose(
                            pt[:, t * MROWS:(t + 1) * MROWS],
                            pred_sb[:, ch * W:(ch + 1) * W], ident[:])
                    nc.vector.tensor_copy(
                        D_sb[:, nb * 192:nb * 192 + 96], pt[:, 0:96])
                    nc.scalar.activation(
                        D_sb[:, nb * 192 + 96:(nb + 1) * 192],
                        pt[:, 96:192],
                        mybir.ActivationFunctionType.Copy)
                else:
                    for t in range(4):
                        ch = 4 * nb + t
                        pt = pshare.tile([128, MROWS], bf16, tag='pshare')
                        nc.tensor.transpose(
                            pt[:], pred_sb[:, ch * W:(ch + 1) * W],
                            ident[:])
                        nc.scalar.activation(
                            D_sb[:, ch * MROWS:(ch + 1) * MROWS], pt[:],
                            mybir.ActivationFunctionType.Copy)
                if GATHER:
                    nc.sync.dma_start(D2s[nb][:, :],
                                      D_sb[:, nb * 192:(nb + 1) * 192])
                    nc.gpsimd.indirect_dma_start(
                        out=gath[:, nb * BLK * 12:(nb + 1) * BLK * 12],
                        out_offset=None,
                        in_=D_ts[nb][:],
                        in_offset=bass.IndirectOffsetOnAxis(
                            ap=idx_t[:, nb:nb + 1], axis=0))
                    nc.scalar.dma_start(
                        out_d[:, nb * BLK * 12:(nb + 1) * BLK * 12],
                        gath[:, nb * BLK * 12:(nb + 1) * BLK * 12])
                else:
                    nc.sync.dma_start(
                        out_d[:, nb * 192:(nb + 1) * 192],
                        D_sb[:, nb * 192:(nb + 1) * 192])

            if debug_outputs:
                nc.sync.dma_start(dbg_pred[:], pred_sb[:])

    nc.compile()
    return nc


def make_in_maps(host):
    in_maps = []
    for core in range(NCORES):
        m = {
            'im2col': host['im2cols'][core],
            'consts': host['consts'][core],
        }
        if GATHER:
            m['idx'] = host['idx_arrays'][core]
        in_maps.append(m)
    return in_maps


def kernel(**inputs) -> np.ndarray:
    _ensure_path()
    from concourse.bass_utils import run_bass_kernel_spmd

    scale = inputs.get('scale', S)
    scale = int(np.asarray(scale)) if not isinstance(scale, int) else scale
    assert scale == S, f"kernel hardcodes scale={S}, got {scale}"

    plan, host = _plan_and_host_data(inputs)
    nc = _build_graph(plan, host)

    in_maps = make_in_maps(host)
    res = run_bass_kernel_spmd(nc, in_maps, core_ids=list(range(NCORES)))

    Q = host['Q']
    q = np.zeros((Q, 3), np.float32)
    if GATHER:
        BLK = plan['BLK']
        NQBP = plan['NQBP']
        per = NQBP // 128
        for core in range(NCORES):
            blocks = np.asarray(res.results[core]['out']).astype(
                np.float32).reshape(128, 4, BLK * 12)
            for nb in range(4):
                sel = host['originals'][core][nb]
                sub = host['subsels'][core][nb]
                loc = host['localoff'][core][nb]
                n = sel.size
                if n == 0:
                    continue
                prt = (np.arange(n) // per)
                base = loc[:n] * 12 + sub * 3
                cols = base[:, None] + np.arange(3)[None]
                q[sel] = np.take_along_axis(blocks[prt, nb], cols, axis=1)
    else:
        core_of, bank_q = host['core_of'], host['bank_q']
        grow, sub_q = host['grow'], host['sub_q']
        outs = np.stack([np.asarray(res.results[core]['out'])
                         for core in range(NCORES)]).astype(np.float32)
        # out[core][xl, nb*192 + (t*4+g)*12 + k]; grow = xl*16 + t*4 + g
        xl = grow // 16
        cols = bank_q * 192 + (grow % 16) * 12 + sub_q * 3
        for c in range(3):
            q[:, c] = outs[core_of, xl, cols + c]
    return q[None]


# revision 15
# speedup vs baseline: 2.2991x; 1.2759x over previous
"""ArbSR (moe_routing) Trainium2 kernel, 8-core SPMD.

Structure exploited: with scale=4, the scale-embedding MLP input is periodic
with period 4 in both HR axes, so routing r, offsets off, and the expert-mix
matrices take only 16 distinct values (one per (y%4, x%4) class).  The
offset grid_sample then becomes, per class, a 2x2-tap bilinear filter of the
encoder feature map f at a constant integer shift, and the whole
  encoder conv -> fea0 -> expert mixing -> (+fea0) -> 3x3 tail conv
chain collapses, after folding the encoder INTO the per-delta matrices
(everything is linear in the input image), to
  pred[:, 4*yl+b, 4*xl+a] = tail_b
      + sum_{dy,dx} (E[(b,a)][(dy,dx)] @ encw) @ im2col[:, yl+dy, xl+dx]
with host-precomputed [48, 28] matrices per (dy, dx).  The dy axis is packed
into the contraction dim (K = 28*ndy) using a row-shift-stacked im2col, so
the device runs ONE matmul per (bank of 4 LR rows, dx): 12 K~117 matmuls of
N=512 for the whole main computation.  The tail bias rides on an all-ones
rhs K row; the left/right tail-conv zero-pad corrections ride on one-hot
rhs K rows whose lhsT coefficients are host-computed exact (true - fold)
values; the top/bot row corrections (cores 0/7) are accumulated into PSUM
by an identity-lhsT matmul against a host-computed [48, W] strip.

Per core (64 HR rows): 14 matmuls + 16 PE transposes into a pixel-major
layout D (rows keyed xl*16+t*4+g so a D write is one contiguous 384B
descriptor per partition); an indirect-DMA block-gather for the
nearest-neighbour queries, which the host routes to cores/banks by row
ownership and sorts so each of 128 partitions fetches one contiguous span.
"""

import numpy as np
import ml_dtypes

BF16 = ml_dtypes.bfloat16


def _ensure_path():
    import sys
    for p in ('/opt/trn_rl_repo',):
        if p not in sys.path:
            sys.path.append(p)


H = W = 128
S = 4
HH = WH = H * S          # 512
C = 64
NCORES = 8
YLC = H // NCORES        # 16 LR rows per core
HRPC = HH // NCORES      # 64 HR rows per core
NPIX = HRPC * WH         # 32768 HR pixels per core
NCLS = 16                # (b, a) classes
MROWS = NCLS * 3         # 48 stacked pred rows
KE = 28                  # encoder-folded contraction rows per dy block
NROWS_B = 2048           # D rows per bank (128 xl * 4 t * 4 g)

GATHER = False           # False: dump D as output, gather on host
EDGE_IN_MAIN = True      # fold top/bot row corr into the main PSUM group
T_SHARED = True          # transposes share one PSUM tile per bank
SPLIT_DRAIN = True       # split PSUM drains between Vector and Scalar


def _sigmoid(x):
    return 1.0 / (1.0 + np.exp(-x))


def _class_constants(d):
    w1 = np.asarray(d['body_w1'], np.float64)
    b1 = np.asarray(d['body_b1'], np.float64)
    w2 = np.asarray(d['body_w2'], np.float64)
    b2 = np.asarray(d['body_b2'], np.float64)
    rw = np.asarray(d['routing_w'], np.float64)
    rb = np.asarray(d['routing_b'], np.float64)
    ow = np.asarray(d['offset_w'], np.float64)
    ob = np.asarray(d['offset_b'], np.float64)
    wc = np.asarray(d['weight_compress'], np.float64)
    we = np.asarray(d['weight_expand'], np.float64)

    fs = float(S)
    coor = np.array([(i + 0.5) / fs - np.floor((i + 0.5) / fs + 0.001) - 0.5
                     for i in range(S)])
    cls = {}
    for b in range(S):
        for a in range(S):
            inp4 = np.array([1.0 / fs, 1.0 / fs, coor[b], coor[a]])
            emb = np.maximum(w1 @ inp4 + b1, 0.0)
            emb = np.maximum(w2 @ emb + b2, 0.0)
            off = ow @ emb + ob
            r = _sigmoid(rw @ emb + rb)
            A = np.einsum('e,eck->ck', r, we) @ np.einsum('e,ekc->kc', r, wc)
            B = A + np.eye(C)
            cx = (a + 0.5) / fs - 0.5 + off[0]
            cy = (b + 0.5) / fs - 0.5 + off[1]
            ix, iy = int(np.floor(cx)), int(np.floor(cy))
            fx, fy = cx - ix, cy - iy
            wbl = {(0, 0): (1 - fy) * (1 - fx), (0, 1): (1 - fy) * fx,
                   (1, 0): fy * (1 - fx), (1, 1): fy * fx}
            cls[(b, a)] = dict(B=B, ix=ix, iy=iy, wbl=wbl)
    return cls


def _build_E(tail_w, cls):
    """E[(b,a)][(dy,dx)] = [3, C] so that pred contribution is E @ f(shift)."""
    Es = {}
    for b in range(S):
        for a in range(S):
            acc = {}
            for ty in range(3):
                for tx in range(3):
                    bp = (b + ty - 1) % S
                    oy = (b + ty - 1 - bp) // S
                    ap_ = (a + tx - 1) % S
                    ox = (a + tx - 1 - ap_) // S
                    c2 = cls[(bp, ap_)]
                    TB = tail_w[:, :, ty, tx] @ c2['B']
                    for (uy, ux), wgt in c2['wbl'].items():
                        if wgt == 0.0:
                            continue
                        key = (oy + c2['iy'] + uy, ox + c2['ix'] + ux)
                        acc[key] = acc.get(key, np.zeros((3, C))) + TB * wgt
            Es[(b, a)] = acc
    return Es


def _build_encw(d):
    """encw [28, C]: rows = 9 taps x 3 ch + inside-mask bias row."""
    enc_w = np.asarray(d['enc_w'], np.float64)
    enc_b = np.asarray(d['enc_b'], np.float64)
    encw = np.zeros((KE, C))
    for ty in range(3):
        for tx in range(3):
            for ch in range(3):
                encw[(ty * 3 + tx) * 3 + ch, :] = enc_w[:, ch, ty, tx]
    encw[27, :] = enc_b
    return encw


PADF = 4   # f64 f-map padding margin (covers all shift indexing)


def _host_f(d):
    """f64 encoder output, zero outside the image, with PADF margin."""
    inp = np.asarray(d['inp'], np.float64)[0]
    ip = np.pad(inp, ((0, 0), (1, 1), (1, 1)))
    enc_w = np.asarray(d['enc_w'], np.float64)
    enc_b = np.asarray(d['enc_b'], np.float64)
    f = np.zeros((C, H, W))
    for ty in range(3):
        for tx in range(3):
            f += np.einsum('oc,chw->ohw', enc_w[:, :, ty, tx],
                           ip[:, ty:ty + H, tx:tx + W])
    f += enc_b[:, None, None]
    fpad = np.zeros((C, H + 2 * PADF, W + 2 * PADF))
    fpad[:, PADF:PADF + H, PADF:PADF + W] = f
    return fpad


def _zgrid(cls, fpad, yHs, xHs):
    """z = out2 + fea0 (zero outside the HR image) on a coordinate grid."""
    yh = np.asarray(yHs)
    xh = np.asarray(xHs)
    out = np.zeros((C, len(yh), len(xh)))
    for b_ in range(S):
        rm = np.nonzero(np.mod(yh, S) == b_)[0]
        if rm.size == 0:
            continue
        ys = yh[rm]
        yl = ys // S
        for a_ in range(S):
            cm = np.nonzero(np.mod(xh, S) == a_)[0]
            if cm.size == 0:
                continue
            xs = xh[cm]
            xl = xs // S
            inside = ((ys[:, None] >= 0) & (ys[:, None] < HH)
                      & (xs[None, :] >= 0) & (xs[None, :] < WH))
            c2 = cls[(b_, a_)]
            fg = np.zeros((C, rm.size, cm.size))
            for (uy, ux), wgt in c2['wbl'].items():
                rr = PADF + yl + c2['iy'] + uy
                cc = PADF + xl + c2['ix'] + ux
                fg += wgt * fpad[:, rr[:, None], cc[None, :]]
            val = np.einsum('oc,cyx->oyx', c2['B'], fg)
            val *= inside[None]
            out[np.ix_(np.arange(C), rm, cm)] = val
    return out


def _true_strip(cls, fpad, tail_w, tail_b, ylgs, xs_lr):
    """Exact pred values [MROWS, len(ylgs), len(xs_lr)] (LR coords)."""
    ylgs = np.asarray(ylgs)
    xs_lr = np.asarray(xs_lr)
    yHs = np.arange(S * ylgs.min() - 1, S * ylgs.max() + S + 1)
    xHs = np.arange(S * xs_lr.min() - 1, S * xs_lr.max() + S + 1)
    z = _zgrid(cls, fpad, yHs, xHs)
    y0, x0 = yHs[0], xHs[0]
    out = np.zeros((MROWS, len(ylgs), len(xs_lr)))
    for b in range(S):
        for a in range(S):
            m0 = (4 * b + a) * 3
            acc = np.zeros((3, len(ylgs), len(xs_lr)))
            for ty in range(3):
                rr = S * ylgs + b + ty - 1 - y0
                for tx in range(3):
                    cc = S * xs_lr + a + tx - 1 - x0
                    acc += np.einsum(
                        'oc,cyx->oyx', tail_w[:, :, ty, tx],
                        z[:, rr[:, None], cc[None, :]])
            out[m0:m0 + 3] = acc + tail_b[:, None, None]
    return out


def _fold_strip(G_main, bias48, fpad, ylgs, xs_lr):
    """What the device mains+bias compute, in f64 (LR coords)."""
    ylgs = np.asarray(ylgs)
    xs_lr = np.asarray(xs_lr)
    out = np.zeros((MROWS, len(ylgs), len(xs_lr)))
    for (dy, dx), G in G_main.items():
        rr = PADF + ylgs + dy
        cc = PADF + xs_lr + dx
        out += np.einsum('mc,cyx->myx', G, fpad[:, rr[:, None], cc[None, :]])
    return out + bias48[:, None, None]


def _plan_and_host_data(d):
    """Host precompute: folded lhsT matrices, per-core im2colK, edge
    corrections, query routing."""
    cls = _class_constants(d)
    tail_w = np.asarray(d['tail_w'], np.float64)
    tail_b = np.asarray(d['tail_b'], np.float64)
    encw = _build_encw(d)

    E_main = _build_E(tail_w, cls)
    deltas = sorted({k for acc in E_main.values() for k in acc})
    dys = sorted({dl[0] for dl in deltas})
    dxs = sorted({dl[1] for dl in deltas})
    dy_min, dy_max = min(dys), max(dys)
    dx_min, dx_max = min(dxs), max(dxs)
    ndy = dy_max - dy_min + 1
    ndx = dx_max - dx_min + 1
    KA = KE * ndy                    # encoder-folded rows
    KM = KA + 1 + 2 * YLC            # + ones row + lef/rig one-hot rows
    assert KM <= 128, (ndy, KM)
    NRB = YLC + ndy                  # base im2col rows
    NCF = W + ndx - 1                # im2col cols
    NFK = YLC * NCF                  # free size of the stacked im2colK

    G_main = {dl: np.zeros((MROWS, C)) for dl in deltas}
    for (b, a), acc in E_main.items():
        m0 = (4 * b + a) * 3
        for dl, M in acc.items():
            G_main[dl][m0:m0 + 3, :] += M

    bias48 = np.zeros(MROWS)
    for b in range(S):
        for a in range(S):
            bias48[(4 * b + a) * 3:(4 * b + a) * 3 + 3] = tail_b

    # ---- exact edge corrections (true - fold), f64 on host ----
    fpad = _host_f(d)
    all_yl = np.arange(H)
    t_lef = _true_strip(cls, fpad, tail_w, tail_b, all_yl, [0])[:, :, 0]
    t_rig = _true_strip(cls, fpad, tail_w, tail_b, all_yl, [W - 1])[:, :, 0]
    f_lef = _fold_strip(G_main, bias48, fpad, all_yl, [0])[:, :, 0]
    f_rig = _fold_strip(G_main, bias48, fpad, all_yl, [W - 1])[:, :, 0]
    corrL = t_lef - f_lef            # [MROWS, H]
    corrR = t_rig - f_rig
    all_x = np.arange(W)
    t_top = _true_strip(cls, fpad, tail_w, tail_b, [0], all_x)[:, 0, :]
    f_top = _fold_strip(G_main, bias48, fpad, [0], all_x)[:, 0, :]
    corrT = t_top - f_top            # [MROWS, W]
    corrT[:, 0] -= corrL[:, 0]
    corrT[:, W - 1] -= corrR[:, 0]
    t_bot = _true_strip(cls, fpad, tail_w, tail_b, [H - 1], all_x)[:, 0, :]
    f_bot = _fold_strip(G_main, bias48, fpad, [H - 1], all_x)[:, 0, :]
    corrB = t_bot - f_bot
    corrB[:, 0] -= corrL[:, H - 1]
    corrB[:, W - 1] -= corrR[:, H - 1]

    # ---- main lhsT per dx: [KM, MROWS] ----
    def fold(G):
        return encw @ G.T

    main_T = []
    for dx in range(dx_min, dx_max + 1):
        lhsT = np.zeros((KM, MROWS))
        for dy in range(dy_min, dy_max + 1):
            if (dy, dx) in G_main:
                j = dy - dy_min
                lhsT[KE * j:KE * j + KE, :] = fold(G_main[(dy, dx)])
        main_T.append(lhsT)
    # bias + per-core lef/rig rows land in the dx=0 seg (c0 = -dx_min)

    # ---- per-core im2colK [KM, YLC*NCF] (dy-shift-stacked) ----
    inp = np.asarray(d['inp'], np.float64)[0]   # [3, H, W]
    PADX = 8
    ippad = np.pad(inp, ((0, 0), (PADX, PADX), (PADX, PADX)))
    ones = np.zeros((H + 2 * PADX, W + 2 * PADX))
    ones[PADX:PADX + H, PADX:PADX + W] = 1.0
    im2cols = []
    for core in range(NCORES):
        y0 = YLC * core + dy_min          # global LR row of base row 0
        x0 = dx_min
        base = np.zeros((KE, NRB, NCF))
        for ty in range(3):
            for tx in range(3):
                ys = PADX + y0 + ty - 1
                xs = PADX + x0 + tx - 1
                for ch in range(3):
                    base[(ty * 3 + tx) * 3 + ch] = \
                        ippad[ch, ys:ys + NRB, xs:xs + NCF]
        inside = ones[PADX + y0:PADX + y0 + NRB, PADX + x0:PADX + x0 + NCF]
        base[27] = inside
        # f must be exactly zero at out-of-image positions (grid-sample
        # zero padding), so kill whole columns there, not just oob taps
        base *= inside[None]
        imk = np.zeros((KM, YLC, NCF), np.float32)
        for j in range(ndy):
            imk[KE * j:KE * j + KE] = base[:, j:j + YLC, :]
        imk[KA] = 1.0                              # bias row
        for yl in range(YLC):                      # lef/rig one-hot rows
            imk[KA + 1 + yl, yl, -dx_min] = 1.0
            imk[KA + 1 + YLC + yl, yl, (W - 1) - dx_min] = 1.0
        im2cols.append(imk.reshape(KM, NFK).astype(BF16))

    # ---- query routing (f32 math matches reference rounding) ----
    coord = np.asarray(d['coord'], np.float32)[0]
    cell = np.asarray(d['cell'], np.float32)[0]
    cq = np.clip(coord - cell * np.float32(0.5) + np.float32(1e-6),
                 np.float32(-1 + 1e-6), np.float32(1 - 1e-6))
    xi = np.clip(np.round((cq[:, 1] + 1) * np.float32(0.5) * (WH - 1)
                          ).astype(np.int64), 0, WH - 1)
    yi = np.clip(np.round((cq[:, 0] + 1) * np.float32(0.5) * (HH - 1)
                          ).astype(np.int64), 0, HH - 1)
    core_of = yi // HRPC
    ylq = (yi % HRPC) // S
    bq = yi % S
    xlq = xi // S
    aq = xi % S
    cls_q = bq * S + aq
    bank_q = ylq // 4
    # D row within a bank: xl*16 + t*4 + g (t = ylq%4, g = cls//4) so a
    # bank's D write is contiguous per partition xl
    grow = xlq * 16 + (ylq % 4) * 4 + cls_q // 4
    sub_q = cls_q % 4                                 # 3-float slot in row
    Q = coord.shape[0]

    host = dict(consts=None, im2cols=im2cols, Q=Q)
    plan = dict(
        dy_min=dy_min, dx_min=dx_min, ndy=ndy, ndx=ndx, KM=KM, KA=KA,
        NCF=NCF, NFK=NFK,
    )

    if GATHER:
        # Bank-pipelined sorted block-gather: per (core, bank), sort
        # queries by D row, split into 128 groups; partition p
        # block-fetches its span.
        per_cb = [[np.nonzero((core_of == core) & (bank_q == nb))[0]
                   for nb in range(4)] for core in range(NCORES)]
        NQBP = 128 * ((max(s.size for row in per_cb for s in row) + 127)
                      // 128)
        NQBP = max(NQBP, 128)
        idx_arrays, originals, subsels, localoff = [], [], [], []
        max_span = 1
        per = NQBP // 128
        for core in range(NCORES):
            lo4, org4, sub4, loc4 = [], [], [], []
            for nb in range(4):
                sel = per_cb[core][nb]
                rows = np.zeros(NQBP, np.int64)
                rows[:sel.size] = grow[sel]
                if sel.size:
                    rows[sel.size:] = rows[:sel.size].max()
                order = np.argsort(rows[:sel.size], kind='stable')
                rows_sorted = np.concatenate([rows[:sel.size][order],
                                              rows[sel.size:]])
                lo = rows_sorted.reshape(128, per)[:, 0].copy()
                span = rows_sorted.reshape(128, per)[:, -1] - lo + 1
                max_span = max(max_span, int(span.max()))
                lo4.append(lo)
                org4.append(sel[order])
                sub4.append(sub_q[sel][order])
                loc4.append(rows_sorted - np.repeat(lo, per))
            idx_arrays.append(lo4)
            originals.append(org4)
            subsels.append(sub4)
            localoff.append(loc4)
        BLK = min(NROWS_B, ((max_span + 3) // 4) * 4)
        for core in range(NCORES):
            lo4 = idx_arrays[core]
            for nb in range(4):
                lo = lo4[nb]
                lo2 = np.clip(np.minimum(lo, NROWS_B - BLK), 0, None)
                localoff[core][nb] = (
                    localoff[core][nb]
                    + np.repeat(lo - lo2, per)).astype(np.int64)
                assert (localoff[core][nb] < BLK).all()
                assert (localoff[core][nb] >= 0).all()
                lo4[nb] = lo2
            idx_arrays[core] = np.stack(
                [l.astype(np.int32) for l in lo4], axis=1)   # [128, 4]
        plan['BLK'] = BLK
        plan['NQBP'] = NQBP
        host.update(idx_arrays=idx_arrays, originals=originals,
                    subsels=subsels, localoff=localoff)
    else:
        host.update(core_of=core_of, bank_q=bank_q, grow=grow, sub_q=sub_q)

    # ---- pack constants into one [128, CW] blob ----
    segs = {}
    col = [0]

    def alloc(name, K, Mw):
        segs[name] = (0, col[0], K, Mw)
        col[0] += Mw

    for i in range(ndx):
        alloc(f'Em{i}', KM, MROWS)
    alloc('corrT', MROWS, W)
    alloc('corrB', MROWS, W)
    CW = col[0]
    plan['segs'] = segs
    plan['CW'] = CW

    i_dx0 = -dx_min
    consts_cores = []
    for core in range(NCORES):
        blob = np.zeros((128, CW), np.float32)
        for i, t in enumerate(main_T):
            t = t.copy()
            if i == i_dx0:
                t[KA, :] = bias48
                yls = YLC * core + np.arange(YLC)
                t[KA + 1:KA + 1 + YLC, :] = corrL[:, yls].T
                t[KA + 1 + YLC:KM, :] = corrR[:, yls].T
            p0, c0, K, Mw = segs[f'Em{i}']
            blob[p0:p0 + K, c0:c0 + Mw] = t
        p0, c0, K, Mw = segs['corrT']
        if core == 0:
            blob[p0:p0 + K, c0:c0 + Mw] = corrT
        p0, c0, K, Mw = segs['corrB']
        if core == NCORES - 1:
            blob[p0:p0 + K, c0:c0 + Mw] = corrB
        consts_cores.append(blob.astype(BF16))
    host['consts'] = consts_cores
    return plan, host


def _build_graph(plan, host, debug_outputs=False):
    _ensure_path()
    import concourse.bass as bass
    import concourse.bacc as bacc
    import concourse.mybir as mybir
    import concourse.tile as tile
    from concourse.masks import make_identity

    f32 = mybir.dt.float32
    bf16 = mybir.dt.bfloat16
    i32 = mybir.dt.int32

    KM, NCF, NFK = plan['KM'], plan['NCF'], plan['NFK']
    dx_min, ndx = plan['dx_min'], plan['ndx']
    segs, CW = plan['segs'], plan['CW']

    nc = bacc.Bacc(None, target_bir_lowering=False, debug=False,
                   num_devices=NCORES)

    imk_d = nc.dram_tensor('im2col', [KM, NFK], bf16, kind='ExternalInput')
    consts_d = nc.dram_tensor('consts', [128, CW], bf16,
                              kind='ExternalInput')
    if GATHER:
        BLK = plan['BLK']
        idx_d = nc.dram_tensor('idx', [128, 4], i32, kind='ExternalInput')
        out_d = nc.dram_tensor('out', [128, 4 * BLK * 12], bf16,
                               kind='ExternalOutput')
    else:
        out_d = nc.dram_tensor('out', [128, YLC * MROWS], bf16,
                               kind='ExternalOutput')
    if debug_outputs:
        dbg_pred = nc.dram_tensor('dbg_pred', [MROWS, YLC * W], bf16,
                                  kind='ExternalOutput')

    with tile.TileContext(nc) as tc:
        with (
            tc.tile_pool(name='sb', bufs=1) as sb,
            tc.tile_pool(name='sbsmall', bufs=1) as sbs,
            tc.tile_pool(name='pshare', bufs=2, space='PSUM') as pshare,
            tc.tile_pool(name='ppred', bufs=1, space='PSUM') as ppred,
            tc.tile_pool(name='dram', bufs=1, space='DRAM') as dpool,
        ):
            consts_t = sb.tile([128, CW], bf16)
            imk = sb.tile([KM, NFK], bf16)
            # consts first (small; first LDWEIGHTS needs it); imk in 4
            # row-group chunks (bank nb reads rows 4nb..4nb+4 only),
            # spread across engine queues so the ~0.8us descriptor
            # generation per DMA runs in parallel
            RC = [0, 4 * NCF, 8 * NCF, 12 * NCF, NFK]
            nc.scalar.dma_start(imk[:, RC[0]:RC[1]], imk_d[:, RC[0]:RC[1]])
            nc.sync.dma_start(consts_t[:], consts_d[:])
            nc.sync.dma_start(imk[:, RC[1]:RC[2]], imk_d[:, RC[1]:RC[2]])
            nc.scalar.dma_start(imk[:, RC[2]:RC[3]], imk_d[:, RC[2]:RC[3]])
            nc.sync.dma_start(imk[:, RC[3]:RC[4]], imk_d[:, RC[3]:RC[4]])
            if GATHER:
                idx_t = sbs.tile([128, 4], i32)
                nc.scalar.dma_start(idx_t[:], idx_d[:])
                D_ts = [dpool.tile([NROWS_B, 12], bf16, tag=f'D{nb}',
                                   name=f'Dscr{nb}')
                        for nb in range(4)]
                D2s = [D_ts[nb][:].rearrange('(xl r) k -> xl (r k)', xl=128)
                       for nb in range(4)]
                gath = sb.tile([128, 4 * BLK * 12], bf16)

            def cseg(name):
                p0, c0, K, Mw = segs[name]
                return consts_t[p0:p0 + K, c0:c0 + Mw]

            ident = sbs.tile([MROWS, MROWS], bf16)
            make_identity(nc, ident[:])

            imk3 = imk[:].rearrange('p (r c) -> p r c', c=NCF)
            pred_ps = ppred.tile([MROWS, YLC * W], f32)

            # ---- PE stream: 3 mains per bank (+ top/bot rows for banks
            # 0/3) accumulated into one PSUM bank per bank ----
            for nb in range(4):
                edge = EDGE_IN_MAIN and ((nb == 0) or (nb == 3))
                for i in range(ndx):
                    nc.tensor.matmul(
                        pred_ps[:, nb * 512:(nb + 1) * 512],
                        cseg(f'Em{i}'),
                        imk3[0:KM, 4 * nb:4 * nb + 4, i:i + W],
                        start=(i == 0), stop=(i == ndx - 1 and not edge),
                        skip_group_check=True)
                if edge and nb == 0:
                    nc.tensor.matmul(
                        pred_ps[:, 0:W], ident[:], cseg('corrT'),
                        start=False, stop=True, skip_group_check=True)
                if edge and nb == 3:
                    nc.tensor.matmul(
                        pred_ps[:, (YLC - 1) * W:YLC * W], ident[:],
                        cseg('corrB'),
                        start=False, stop=True, skip_group_check=True)

            # ---- per-bank drain -> transpose -> D copy -> D write ->
            # gather -> out ----
            pred_sb = sb.tile([MROWS, YLC * W], bf16)
            D_sb = sb.tile([128, YLC * MROWS], bf16)

            for nb in range(4):
                if SPLIT_DRAIN:
                    nc.vector.tensor_copy(
                        pred_sb[:, nb * 512:nb * 512 + 256],
                        pred_ps[:, nb * 512:nb * 512 + 256])
                    nc.scalar.activation(
                        pred_sb[:, nb * 512 + 256:(nb + 1) * 512],
                        pred_ps[:, nb * 512 + 256:(nb + 1) * 512],
                        mybir.ActivationFunctionType.Copy)
                else:
                    nc.vector.tensor_copy(
                        pred_sb[:, nb * 512:(nb + 1) * 512],
                        pred_ps[:, nb * 512:(nb + 1) * 512])
                if not EDGE_IN_MAIN and nb == 0:
                    nc.vector.tensor_add(pred_sb[:, 0:W], pred_sb[:, 0:W],
                                         cseg('corrT'))
                if not EDGE_IN_MAIN and nb == 3:
                    nc.vector.tensor_add(pred_sb[:, (YLC - 1) * W:YLC * W],
                                         pred_sb[:, (YLC - 1) * W:YLC * W],
                                         cseg('corrB'))
                if T_SHARED:
                    pt = pshare.tile([128, 4 * MROWS], bf16, tag='pshare')
                    for t in range(4):
                        ch = 4 * nb + t
                        nc.tensor.transpose(
                            pt[:, t * MROWS:(t + 1) * MROWS],
                            pred_sb[:, ch * W:(ch + 1) * W], ident[:])
                    nc.vector.tensor_copy(
                        D_sb[:, nb * 192:nb * 192 + 96], pt[:, 0:96])
                    nc.scalar.activation(
                        D_sb[:, nb * 192 + 96:(nb + 1) * 192],
                        pt[:, 96:192],
                        mybir.ActivationFunctionType.Copy)
                else:
                    for t in range(4):
                        ch = 4 * nb + t
                        pt = pshare.tile([128, MROWS], bf16, tag='pshare')
                        nc.tensor.transpose(
                            pt[:], pred_sb[:, ch * W:(ch + 1) * W],
                            ident[:])
                        nc.scalar.activation(
                            D_sb[:, ch * MROWS:(ch + 1) * MROWS], pt[:],
                            mybir.ActivationFunctionType.Copy)
                if GATHER:
                    nc.sync.dma_start(D2s[nb][:, :],
                                      D_sb[:, nb * 192:(nb + 1) * 192])
                    nc.gpsimd.indirect_dma_start(
                        out=gath[:, nb * BLK * 12:(nb + 1) * BLK * 12],
                        out_offset=None,
                        in_=D_ts[nb][:],
                        in_offset=bass.IndirectOffsetOnAxis(
                            ap=idx_t[:, nb:nb + 1], axis=0))
                    nc.scalar.dma_start(
                        out_d[:, nb * BLK * 12:(nb + 1) * BLK * 12],
                        gath[:, nb * BLK * 12:(nb + 1) * BLK * 12])
                else:
                    nc.sync.dma_start(
                        out_d[:, nb * 192:(nb + 1) * 192],
                        D_sb[:, nb * 192:(nb + 1) * 192])

            if debug_outputs:
                nc.sync.dma_start(dbg_pred[:], pred_sb[:])

    nc.compile()
    return nc


def make_in_maps(host):
    in_maps = []
    for core in range(NCORES):
        m = {
            'im2col': host['im2cols'][core],
            'consts': host['consts'][core],
        }
        if GATHER:
            m['idx'] = host['idx_arrays'][core]
        in_maps.append(m)
    return in_maps


def kernel(**inputs) -> np.ndarray:
    _ensure_path()
    from concourse.bass_utils import run_bass_kernel_spmd

    scale = inputs.get('scale', S)
    scale = int(np.asarray(scale)) if not isinstance(scale, int) else scale
    assert scale == S, f"kernel hardcodes scale={S}, got {scale}"

    plan, host = _plan_and_host_data(inputs)
    nc = _build_graph(plan, host)

    in_maps = make_in_maps(host)
    res = run_bass_kernel_spmd(nc, in_maps, core_ids=list(range(NCORES)))

    Q = host['Q']
    q = np.zeros((Q, 3), np.float32)
    if GATHER:
        BLK = plan['BLK']
        NQBP = plan['NQBP']
        per = NQBP // 128
        for core in range(NCORES):
            blocks = np.asarray(res.results[core]['out']).astype(
                np.float32).reshape(128, 4, BLK * 12)
            for nb in range(4):
                sel = host['originals'][core][nb]
                sub = host['subsels'][core][nb]
                loc = host['localoff'][core][nb]
                n = sel.size
                if n == 0:
                    continue
                prt = (np.arange(n) // per)
                base = loc[:n] * 12 + sub * 3
                cols = base[:, None] + np.arange(3)[None]
                q[sel] = np.take_along_axis(blocks[prt, nb], cols, axis=1)
    else:
        core_of, bank_q = host['core_of'], host['bank_q']
        grow, sub_q = host['grow'], host['sub_q']
        outs = np.stack([np.asarray(res.results[core]['out'])
                         for core in range(NCORES)]).astype(np.float32)
        # out[core][xl, nb*192 + (t*4+g)*12 + k]; grow = xl*16 + t*4 + g
        xl = grow // 16
        cols = bank_q * 192 + (grow % 16) * 12 + sub_q * 3
        for c in range(3):
            q[:, c] = outs[core_of, xl, cols + c]
    return q[None]


# revision 17
# speedup vs baseline: 2.5135x; 1.0933x over previous
"""ArbSR (moe_routing) Trainium2 kernel, 8-core SPMD.

Structure exploited: with scale=4, the scale-embedding MLP input is periodic
with period 4 in both HR axes, so routing r, offsets off, and the expert-mix
matrices take only 16 distinct values (one per (y%4, x%4) class).  The
offset grid_sample then becomes, per class, a 2x2-tap bilinear filter of the
encoder feature map f at a constant integer shift, and the whole
  encoder conv -> fea0 -> expert mixing -> (+fea0) -> 3x3 tail conv
chain collapses, after folding the encoder INTO the per-delta matrices
(everything is linear in the input image), to
  pred[:, 4*yl+b, 4*xl+a] = tail_b
      + sum_{dy,dx} (E[(b,a)][(dy,dx)] @ encw) @ im2col[:, yl+dy, xl+dx]
with host-precomputed [48, 28] matrices per (dy, dx).  The dy axis is packed
into the contraction dim (K = 28*ndy) using a row-shift-stacked im2col, so
the device runs ONE matmul per (bank of 4 LR rows, dx): 12 K~117 matmuls of
N=512 for the whole main computation.  The tail bias rides on an all-ones
rhs K row; the left/right tail-conv zero-pad corrections ride on one-hot
rhs K rows whose lhsT coefficients are host-computed exact (true - fold)
values; the top/bot row corrections (cores 0/7) are accumulated into PSUM
by an identity-lhsT matmul against a host-computed [48, W] strip.

Per core (64 HR rows): 14 matmuls + 16 PE transposes into a pixel-major
layout D (rows keyed xl*16+t*4+g so a D write is one contiguous 384B
descriptor per partition); an indirect-DMA block-gather for the
nearest-neighbour queries, which the host routes to cores/banks by row
ownership and sorts so each of 128 partitions fetches one contiguous span.
"""

import numpy as np
import ml_dtypes

BF16 = ml_dtypes.bfloat16


def _ensure_path():
    import sys
    for p in ('/opt/trn_rl_repo',):
        if p not in sys.path:
            sys.path.append(p)


H = W = 128
S = 4
HH = WH = H * S          # 512
C = 64
NCORES = 8
YLC = H // NCORES        # 16 LR rows per core
HRPC = HH // NCORES      # 64 HR rows per core
NPIX = HRPC * WH         # 32768 HR pixels per core
NCLS = 16                # (b, a) classes
MROWS = NCLS * 3         # 48 stacked pred rows
KE = 28                  # encoder-folded contraction rows per dy block
NROWS_B = 2048           # D rows per bank (128 xl * 4 t * 4 g)

GATHER = False           # False: dump D as output, gather on host
EDGE_IN_MAIN = True      # fold top/bot row corr into the main PSUM group
T_SHARED = True          # transposes share one PSUM tile per bank
SPLIT_DRAIN = True       # split PSUM drains between Vector and Scalar


def _sigmoid(x):
    return 1.0 / (1.0 + np.exp(-x))


def _class_constants(d):
    w1 = np.asarray(d['body_w1'], np.float64)
    b1 = np.asarray(d['body_b1'], np.float64)
    w2 = np.asarray(d['body_w2'], np.float64)
    b2 = np.asarray(d['body_b2'], np.float64)
    rw = np.asarray(d['routing_w'], np.float64)
    rb = np.asarray(d['routing_b'], np.float64)
    ow = np.asarray(d['offset_w'], np.float64)
    ob = np.asarray(d['offset_b'], np.float64)
    wc = np.asarray(d['weight_compress'], np.float64)
    we = np.asarray(d['weight_expand'], np.float64)

    fs = float(S)
    coor = np.array([(i + 0.5) / fs - np.floor((i + 0.5) / fs + 0.001) - 0.5
                     for i in range(S)])
    cls = {}
    for b in range(S):
        for a in range(S):
            inp4 = np.array([1.0 / fs, 1.0 / fs, coor[b], coor[a]])
            emb = np.maximum(w1 @ inp4 + b1, 0.0)
            emb = np.maximum(w2 @ emb + b2, 0.0)
            off = ow @ emb + ob
            r = _sigmoid(rw @ emb + rb)
            A = np.einsum('e,eck->ck', r, we) @ np.einsum('e,ekc->kc', r, wc)
            B = A + np.eye(C)
            cx = (a + 0.5) / fs - 0.5 + off[0]
            cy = (b + 0.5) / fs - 0.5 + off[1]
            ix, iy = int(np.floor(cx)), int(np.floor(cy))
            fx, fy = cx - ix, cy - iy
            wbl = {(0, 0): (1 - fy) * (1 - fx), (0, 1): (1 - fy) * fx,
                   (1, 0): fy * (1 - fx), (1, 1): fy * fx}
            cls[(b, a)] = dict(B=B, ix=ix, iy=iy, wbl=wbl)
    return cls


def _build_E(tail_w, cls):
    """E[(b,a)][(dy,dx)] = [3, C] so that pred contribution is E @ f(shift)."""
    Es = {}
    for b in range(S):
        for a in range(S):
            acc = {}
            for ty in range(3):
                for tx in range(3):
                    bp = (b + ty - 1) % S
                    oy = (b + ty - 1 - bp) // S
                    ap_ = (a + tx - 1) % S
                    ox = (a + tx - 1 - ap_) // S
                    c2 = cls[(bp, ap_)]
                    TB = tail_w[:, :, ty, tx] @ c2['B']
                    for (uy, ux), wgt in c2['wbl'].items():
                        if wgt == 0.0:
                            continue
                        key = (oy + c2['iy'] + uy, ox + c2['ix'] + ux)
                        acc[key] = acc.get(key, np.zeros((3, C))) + TB * wgt
            Es[(b, a)] = acc
    return Es


def _build_encw(d):
    """encw [28, C]: rows = 9 taps x 3 ch + inside-mask bias row."""
    enc_w = np.asarray(d['enc_w'], np.float64)
    enc_b = np.asarray(d['enc_b'], np.float64)
    encw = np.zeros((KE, C))
    for ty in range(3):
        for tx in range(3):
            for ch in range(3):
                encw[(ty * 3 + tx) * 3 + ch, :] = enc_w[:, ch, ty, tx]
    encw[27, :] = enc_b
    return encw


PADF = 4   # f64 f-map padding margin (covers all shift indexing)


def _host_f(d):
    """f64 encoder output, zero outside the image, with PADF margin."""
    inp = np.asarray(d['inp'], np.float64)[0]
    ip = np.pad(inp, ((0, 0), (1, 1), (1, 1)))
    enc_w = np.asarray(d['enc_w'], np.float64)
    enc_b = np.asarray(d['enc_b'], np.float64)
    f = np.zeros((C, H, W))
    for ty in range(3):
        for tx in range(3):
            f += np.einsum('oc,chw->ohw', enc_w[:, :, ty, tx],
                           ip[:, ty:ty + H, tx:tx + W])
    f += enc_b[:, None, None]
    fpad = np.zeros((C, H + 2 * PADF, W + 2 * PADF))
    fpad[:, PADF:PADF + H, PADF:PADF + W] = f
    return fpad


def _zgrid(cls, fpad, yHs, xHs):
    """z = out2 + fea0 (zero outside the HR image) on a coordinate grid."""
    yh = np.asarray(yHs)
    xh = np.asarray(xHs)
    out = np.zeros((C, len(yh), len(xh)))
    for b_ in range(S):
        rm = np.nonzero(np.mod(yh, S) == b_)[0]
        if rm.size == 0:
            continue
        ys = yh[rm]
        yl = ys // S
        for a_ in range(S):
            cm = np.nonzero(np.mod(xh, S) == a_)[0]
            if cm.size == 0:
                continue
            xs = xh[cm]
            xl = xs // S
            inside = ((ys[:, None] >= 0) & (ys[:, None] < HH)
                      & (xs[None, :] >= 0) & (xs[None, :] < WH))
            c2 = cls[(b_, a_)]
            fg = np.zeros((C, rm.size, cm.size))
            for (uy, ux), wgt in c2['wbl'].items():
                rr = PADF + yl + c2['iy'] + uy
                cc = PADF + xl + c2['ix'] + ux
                fg += wgt * fpad[:, rr[:, None], cc[None, :]]
            val = np.einsum('oc,cyx->oyx', c2['B'], fg)
            val *= inside[None]
            out[np.ix_(np.arange(C), rm, cm)] = val
    return out


def _true_strip(cls, fpad, tail_w, tail_b, ylgs, xs_lr):
    """Exact pred values [MROWS, len(ylgs), len(xs_lr)] (LR coords)."""
    ylgs = np.asarray(ylgs)
    xs_lr = np.asarray(xs_lr)
    yHs = np.arange(S * ylgs.min() - 1, S * ylgs.max() + S + 1)
    xHs = np.arange(S * xs_lr.min() - 1, S * xs_lr.max() + S + 1)
    z = _zgrid(cls, fpad, yHs, xHs)
    y0, x0 = yHs[0], xHs[0]
    out = np.zeros((MROWS, len(ylgs), len(xs_lr)))
    for b in range(S):
        for a in range(S):
            m0 = (4 * b + a) * 3
            acc = np.zeros((3, len(ylgs), len(xs_lr)))
            for ty in range(3):
                rr = S * ylgs + b + ty - 1 - y0
                for tx in range(3):
                    cc = S * xs_lr + a + tx - 1 - x0
                    acc += np.einsum(
                        'oc,cyx->oyx', tail_w[:, :, ty, tx],
                        z[:, rr[:, None], cc[None, :]])
            out[m0:m0 + 3] = acc + tail_b[:, None, None]
    return out


def _fold_strip(G_main, bias48, fpad, ylgs, xs_lr):
    """What the device mains+bias compute, in f64 (LR coords)."""
    ylgs = np.asarray(ylgs)
    xs_lr = np.asarray(xs_lr)
    out = np.zeros((MROWS, len(ylgs), len(xs_lr)))
    for (dy, dx), G in G_main.items():
        rr = PADF + ylgs + dy
        cc = PADF + xs_lr + dx
        out += np.einsum('mc,cyx->myx', G, fpad[:, rr[:, None], cc[None, :]])
    return out + bias48[:, None, None]


def _plan_and_host_data(d):
    """Host precompute: folded lhsT matrices, per-core im2colK, edge
    corrections, query routing."""
    cls = _class_constants(d)
    tail_w = np.asarray(d['tail_w'], np.float64)
    tail_b = np.asarray(d['tail_b'], np.float64)
    encw = _build_encw(d)

    E_main = _build_E(tail_w, cls)
    deltas = sorted({k for acc in E_main.values() for k in acc})
    dys = sorted({dl[0] for dl in deltas})
    dxs = sorted({dl[1] for dl in deltas})
    dy_min, dy_max = min(dys), max(dys)
    dx_min, dx_max = min(dxs), max(dxs)
    ndy = dy_max - dy_min + 1
    ndx = dx_max - dx_min + 1
    KA = KE * ndy                    # encoder-folded rows
    KM = KA + 1 + 2 * YLC            # + ones row + lef/rig one-hot rows
    assert KM <= 128, (ndy, KM)
    NRB = YLC + ndy                  # base im2col rows
    NCF = W + ndx - 1                # im2col cols
    NFK = YLC * NCF                  # free size of the stacked im2colK

    G_main = {dl: np.zeros((MROWS, C)) for dl in deltas}
    for (b, a), acc in E_main.items():
        m0 = (4 * b + a) * 3
        for dl, M in acc.items():
            G_main[dl][m0:m0 + 3, :] += M

    bias48 = np.zeros(MROWS)
    for b in range(S):
        for a in range(S):
            bias48[(4 * b + a) * 3:(4 * b + a) * 3 + 3] = tail_b

    # ---- exact edge corrections (true - fold), f64 on host ----
    fpad = _host_f(d)
    all_yl = np.arange(H)
    t_lef = _true_strip(cls, fpad, tail_w, tail_b, all_yl, [0])[:, :, 0]
    t_rig = _true_strip(cls, fpad, tail_w, tail_b, all_yl, [W - 1])[:, :, 0]
    f_lef = _fold_strip(G_main, bias48, fpad, all_yl, [0])[:, :, 0]
    f_rig = _fold_strip(G_main, bias48, fpad, all_yl, [W - 1])[:, :, 0]
    corrL = t_lef - f_lef            # [MROWS, H]
    corrR = t_rig - f_rig
    all_x = np.arange(W)
    t_top = _true_strip(cls, fpad, tail_w, tail_b, [0], all_x)[:, 0, :]
    f_top = _fold_strip(G_main, bias48, fpad, [0], all_x)[:, 0, :]
    corrT = t_top - f_top            # [MROWS, W]
    corrT[:, 0] -= corrL[:, 0]
    corrT[:, W - 1] -= corrR[:, 0]
    t_bot = _true_strip(cls, fpad, tail_w, tail_b, [H - 1], all_x)[:, 0, :]
    f_bot = _fold_strip(G_main, bias48, fpad, [H - 1], all_x)[:, 0, :]
    corrB = t_bot - f_bot
    corrB[:, 0] -= corrL[:, H - 1]
    corrB[:, W - 1] -= corrR[:, H - 1]

    # ---- main lhsT per dx: [KM, MROWS] ----
    def fold(G):
        return encw @ G.T

    main_T = []
    for dx in range(dx_min, dx_max + 1):
        lhsT = np.zeros((KM, MROWS))
        for dy in range(dy_min, dy_max + 1):
            if (dy, dx) in G_main:
                j = dy - dy_min
                lhsT[KE * j:KE * j + KE, :] = fold(G_main[(dy, dx)])
        main_T.append(lhsT)
    # bias + per-core lef/rig rows land in the dx=0 seg (c0 = -dx_min)

    # ---- per-core im2colK [KM, YLC*NCF] (dy-shift-stacked) ----
    inp = np.asarray(d['inp'], np.float64)[0]   # [3, H, W]
    PADX = 8
    ippad = np.pad(inp, ((0, 0), (PADX, PADX), (PADX, PADX)))
    ones = np.zeros((H + 2 * PADX, W + 2 * PADX))
    ones[PADX:PADX + H, PADX:PADX + W] = 1.0
    im2cols = []
    for core in range(NCORES):
        y0 = YLC * core + dy_min          # global LR row of base row 0
        x0 = dx_min
        base = np.zeros((KE, NRB, NCF))
        for ty in range(3):
            for tx in range(3):
                ys = PADX + y0 + ty - 1
                xs = PADX + x0 + tx - 1
                for ch in range(3):
                    base[(ty * 3 + tx) * 3 + ch] = \
                        ippad[ch, ys:ys + NRB, xs:xs + NCF]
        inside = ones[PADX + y0:PADX + y0 + NRB, PADX + x0:PADX + x0 + NCF]
        base[27] = inside
        # f must be exactly zero at out-of-image positions (grid-sample
        # zero padding), so kill whole columns there, not just oob taps
        base *= inside[None]
        imk = np.zeros((KM, YLC, NCF), np.float32)
        for j in range(ndy):
            imk[KE * j:KE * j + KE] = base[:, j:j + YLC, :]
        imk[KA] = 1.0                              # bias row
        for yl in range(YLC):                      # lef/rig one-hot rows
            imk[KA + 1 + yl, yl, -dx_min] = 1.0
            imk[KA + 1 + YLC + yl, yl, (W - 1) - dx_min] = 1.0
        im2cols.append(imk.reshape(KM, NFK).astype(BF16))

    # ---- query routing (f32 math matches reference rounding) ----
    coord = np.asarray(d['coord'], np.float32)[0]
    cell = np.asarray(d['cell'], np.float32)[0]
    cq = np.clip(coord - cell * np.float32(0.5) + np.float32(1e-6),
                 np.float32(-1 + 1e-6), np.float32(1 - 1e-6))
    xi = np.clip(np.round((cq[:, 1] + 1) * np.float32(0.5) * (WH - 1)
                          ).astype(np.int64), 0, WH - 1)
    yi = np.clip(np.round((cq[:, 0] + 1) * np.float32(0.5) * (HH - 1)
                          ).astype(np.int64), 0, HH - 1)
    core_of = yi // HRPC
    ylq = (yi % HRPC) // S
    bq = yi % S
    xlq = xi // S
    aq = xi % S
    cls_q = bq * S + aq
    bank_q = ylq // 4
    # D row within a bank: xl*16 + t*4 + g (t = ylq%4, g = cls//4) so a
    # bank's D write is contiguous per partition xl
    grow = xlq * 16 + (ylq % 4) * 4 + cls_q // 4
    sub_q = cls_q % 4                                 # 3-float slot in row
    Q = coord.shape[0]

    host = dict(consts=None, im2cols=im2cols, Q=Q)
    plan = dict(
        dy_min=dy_min, dx_min=dx_min, ndy=ndy, ndx=ndx, KM=KM, KA=KA,
        NCF=NCF, NFK=NFK,
    )

    if GATHER:
        # Bank-pipelined sorted block-gather: per (core, bank), sort
        # queries by D row, split into 128 groups; partition p
        # block-fetches its span.
        per_cb = [[np.nonzero((core_of == core) & (bank_q == nb))[0]
                   for nb in range(4)] for core in range(NCORES)]
        NQBP = 128 * ((max(s.size for row in per_cb for s in row) + 127)
                      // 128)
        NQBP = max(NQBP, 128)
        idx_arrays, originals, subsels, localoff = [], [], [], []
        max_span = 1
        per = NQBP // 128
        for core in range(NCORES):
            lo4, org4, sub4, loc4 = [], [], [], []
            for nb in range(4):
                sel = per_cb[core][nb]
                rows = np.zeros(NQBP, np.int64)
                rows[:sel.size] = grow[sel]
                if sel.size:
                    rows[sel.size:] = rows[:sel.size].max()
                order = np.argsort(rows[:sel.size], kind='stable')
                rows_sorted = np.concatenate([rows[:sel.size][order],
                                              rows[sel.size:]])
                lo = rows_sorted.reshape(128, per)[:, 0].copy()
                span = rows_sorted.reshape(128, per)[:, -1] - lo + 1
                max_span = max(max_span, int(span.max()))
                lo4.append(lo)
                org4.append(sel[order])
                sub4.append(sub_q[sel][order])
                loc4.append(rows_sorted - np.repeat(lo, per))
            idx_arrays.append(lo4)
            originals.append(org4)
            subsels.append(sub4)
            localoff.append(loc4)
        BLK = min(NROWS_B, ((max_span + 3) // 4) * 4)
        for core in range(NCORES):
            lo4 = idx_arrays[core]
            for nb in range(4):
                lo = lo4[nb]
                lo2 = np.clip(np.minimum(lo, NROWS_B - BLK), 0, None)
                localoff[core][nb] = (
                    localoff[core][nb]
                    + np.repeat(lo - lo2, per)).astype(np.int64)
                assert (localoff[core][nb] < BLK).all()
                assert (localoff[core][nb] >= 0).all()
                lo4[nb] = lo2
            idx_arrays[core] = np.stack(
                [l.astype(np.int32) for l in lo4], axis=1)   # [128, 4]
        plan['BLK'] = BLK
        plan['NQBP'] = NQBP
        host.update(idx_arrays=idx_arrays, originals=originals,
                    subsels=subsels, localoff=localoff)
    else:
        host.update(core_of=core_of, bank_q=bank_q, grow=grow, sub_q=sub_q)

    # ---- pack constants into one [128, CW] blob ----
    segs = {}
    col = [0]

    def alloc(name, K, Mw):
        segs[name] = (0, col[0], K, Mw)
        col[0] += Mw

    for i in range(ndx):
        alloc(f'Em{i}', KM, MROWS)
    alloc('corrT', MROWS, W)
    alloc('corrB', MROWS, W)
    CW = col[0]
    plan['segs'] = segs
    plan['CW'] = CW

    i_dx0 = -dx_min
    consts_cores = []
    for core in range(NCORES):
        blob = np.zeros((128, CW), np.float32)
        for i, t in enumerate(main_T):
            t = t.copy()
            if i == i_dx0:
                t[KA, :] = bias48
                yls = YLC * core + np.arange(YLC)
                t[KA + 1:KA + 1 + YLC, :] = corrL[:, yls].T
                t[KA + 1 + YLC:KM, :] = corrR[:, yls].T
            p0, c0, K, Mw = segs[f'Em{i}']
            blob[p0:p0 + K, c0:c0 + Mw] = t
        p0, c0, K, Mw = segs['corrT']
        if core == 0:
            blob[p0:p0 + K, c0:c0 + Mw] = corrT
        p0, c0, K, Mw = segs['corrB']
        if core == NCORES - 1:
            blob[p0:p0 + K, c0:c0 + Mw] = corrB
        consts_cores.append(blob.astype(BF16))
    host['consts'] = consts_cores
    return plan, host


def _build_graph(plan, host, debug_outputs=False):
    _ensure_path()
    import concourse.bass as bass
    import concourse.bacc as bacc
    import concourse.mybir as mybir
    import concourse.tile as tile
    from concourse.masks import make_identity

    f32 = mybir.dt.float32
    bf16 = mybir.dt.bfloat16
    i32 = mybir.dt.int32

    KM, NCF, NFK = plan['KM'], plan['NCF'], plan['NFK']
    dx_min, ndx = plan['dx_min'], plan['ndx']
    segs, CW = plan['segs'], plan['CW']

    nc = bacc.Bacc(None, target_bir_lowering=False, debug=False,
                   num_devices=NCORES)

    imk_d = nc.dram_tensor('im2col', [KM, NFK], bf16, kind='ExternalInput')
    consts_d = nc.dram_tensor('consts', [128, CW], bf16,
                              kind='ExternalInput')
    if GATHER:
        BLK = plan['BLK']
        idx_d = nc.dram_tensor('idx', [128, 4], i32, kind='ExternalInput')
        out_d = nc.dram_tensor('out', [128, 4 * BLK * 12], bf16,
                               kind='ExternalOutput')
    else:
        out_d = nc.dram_tensor('out', [128, YLC * MROWS], bf16,
                               kind='ExternalOutput')
    if debug_outputs:
        dbg_pred = nc.dram_tensor('dbg_pred', [MROWS, YLC * W], bf16,
                                  kind='ExternalOutput')

    with tile.TileContext(nc) as tc:
        with (
            tc.tile_pool(name='sb', bufs=1) as sb,
            tc.tile_pool(name='sbsmall', bufs=1) as sbs,
            tc.tile_pool(name='pshare', bufs=2, space='PSUM') as pshare,
            tc.tile_pool(name='ppred', bufs=1, space='PSUM') as ppred,
            tc.tile_pool(name='dram', bufs=1, space='DRAM') as dpool,
        ):
            consts_t = sb.tile([128, CW], bf16)
            imk = sb.tile([KM, NFK], bf16)
            # consts first (small; first LDWEIGHTS needs it); imk in 4
            # row-group chunks (bank nb reads rows 4nb..4nb+4 only),
            # spread across engine queues so the ~0.8us descriptor
            # generation per DMA runs in parallel
            # scalar's queue is blocked ~1.3us by the auto ACT_TABLE_LOAD,
            # so everything the first matmuls need goes on sync
            RC = [0, 4 * NCF, 8 * NCF, 12 * NCF, NFK]
            nc.sync.dma_start(consts_t[:], consts_d[:])
            nc.sync.dma_start(imk[:, RC[0]:RC[1]], imk_d[:, RC[0]:RC[1]])
            nc.sync.dma_start(imk[:, RC[1]:RC[2]], imk_d[:, RC[1]:RC[2]])
            nc.scalar.dma_start(imk[:, RC[2]:RC[3]], imk_d[:, RC[2]:RC[3]])
            nc.sync.dma_start(imk[:, RC[3]:RC[4]], imk_d[:, RC[3]:RC[4]])
            if GATHER:
                idx_t = sbs.tile([128, 4], i32)
                nc.scalar.dma_start(idx_t[:], idx_d[:])
                D_ts = [dpool.tile([NROWS_B, 12], bf16, tag=f'D{nb}',
                                   name=f'Dscr{nb}')
                        for nb in range(4)]
                D2s = [D_ts[nb][:].rearrange('(xl r) k -> xl (r k)', xl=128)
                       for nb in range(4)]
                gath = sb.tile([128, 4 * BLK * 12], bf16)

            def cseg(name):
                p0, c0, K, Mw = segs[name]
                return consts_t[p0:p0 + K, c0:c0 + Mw]

            ident = sbs.tile([MROWS, MROWS], bf16)
            make_identity(nc, ident[:])

            imk3 = imk[:].rearrange('p (r c) -> p r c', c=NCF)
            # one PSUM tile per bank so drains don't wait on later
            # banks' matmuls (Tile tracks deps at tile granularity)
            pred_ps = [ppred.tile([MROWS, 512], f32, tag=f'pp{nb}',
                                  name=f'pp{nb}')
                       for nb in range(4)]

            # ---- PE stream: 3 mains per bank (+ top/bot rows for banks
            # 0/3) accumulated into one PSUM bank per bank ----
            for nb in range(4):
                edge = EDGE_IN_MAIN and ((nb == 0) or (nb == 3))
                for i in range(ndx):
                    nc.tensor.matmul(
                        pred_ps[nb][:],
                        cseg(f'Em{i}'),
                        imk3[0:KM, 4 * nb:4 * nb + 4, i:i + W],
                        start=(i == 0), stop=(i == ndx - 1 and not edge),
                        skip_group_check=True)
                if edge and nb == 0:
                    nc.tensor.matmul(
                        pred_ps[0][:, 0:W], ident[:], cseg('corrT'),
                        start=False, stop=True, skip_group_check=True)
                if edge and nb == 3:
                    nc.tensor.matmul(
                        pred_ps[3][:, 3 * W:4 * W], ident[:],
                        cseg('corrB'),
                        start=False, stop=True, skip_group_check=True)

            # ---- per-bank drain -> transpose -> D copy -> D write ->
            # gather -> out ----
            pred_sb = sb.tile([MROWS, YLC * W], bf16)
            D_sb = sb.tile([128, YLC * MROWS], bf16)

            for nb in range(4):
                if SPLIT_DRAIN:
                    nc.vector.tensor_copy(
                        pred_sb[:, nb * 512:nb * 512 + 256],
                        pred_ps[nb][:, 0:256])
                    nc.scalar.activation(
                        pred_sb[:, nb * 512 + 256:(nb + 1) * 512],
                        pred_ps[nb][:, 256:512],
                        mybir.ActivationFunctionType.Copy)
                else:
                    nc.vector.tensor_copy(
                        pred_sb[:, nb * 512:(nb + 1) * 512],
                        pred_ps[nb][:])
                if not EDGE_IN_MAIN and nb == 0:
                    nc.vector.tensor_add(pred_sb[:, 0:W], pred_sb[:, 0:W],
                                         cseg('corrT'))
                if not EDGE_IN_MAIN and nb == 3:
                    nc.vector.tensor_add(pred_sb[:, (YLC - 1) * W:YLC * W],
                                         pred_sb[:, (YLC - 1) * W:YLC * W],
                                         cseg('corrB'))
                if T_SHARED:
                    pt = pshare.tile([128, 4 * MROWS], bf16, tag='pshare')
                    for t in range(4):
                        ch = 4 * nb + t
                        nc.tensor.transpose(
                            pt[:, t * MROWS:(t + 1) * MROWS],
                            pred_sb[:, ch * W:(ch + 1) * W], ident[:])
                    nc.vector.tensor_copy(
                        D_sb[:, nb * 192:nb * 192 + 96], pt[:, 0:96])
                    nc.scalar.activation(
                        D_sb[:, nb * 192 + 96:(nb + 1) * 192],
                        pt[:, 96:192],
                        mybir.ActivationFunctionType.Copy)
                else:
                    for t in range(4):
                        ch = 4 * nb + t
                        pt = pshare.tile([128, MROWS], bf16, tag='pshare')
                        nc.tensor.transpose(
                            pt[:], pred_sb[:, ch * W:(ch + 1) * W],
                            ident[:])
                        nc.scalar.activation(
                            D_sb[:, ch * MROWS:(ch + 1) * MROWS], pt[:],
                            mybir.ActivationFunctionType.Copy)
                if GATHER:
                    nc.sync.dma_start(D2s[nb][:, :],
                                      D_sb[:, nb * 192:(nb + 1) * 192])
                    nc.gpsimd.indirect_dma_start(
                        out=gath[:, nb * BLK * 12:(nb + 1) * BLK * 12],
                        out_offset=None,
                        in_=D_ts[nb][:],
                        in_offset=bass.IndirectOffsetOnAxis(
                            ap=idx_t[:, nb:nb + 1], axis=0))
                    nc.scalar.dma_start(
                        out_d[:, nb * BLK * 12:(nb + 1) * BLK * 12],
                        gath[:, nb * BLK * 12:(nb + 1) * BLK * 12])
                else:
                    nc.sync.dma_start(
                        out_d[:, nb * 192:(nb + 1) * 192],
                        D_sb[:, nb * 192:(nb + 1) * 192])

            if debug_outputs:
                nc.sync.dma_start(dbg_pred[:], pred_sb[:])

    nc.compile()
    return nc


def make_in_maps(host):
    in_maps = []
    for core in range(NCORES):
        m = {
            'im2col': host['im2cols'][core],
            'consts': host['consts'][core],
        }
        if GATHER:
            m['idx'] = host['idx_arrays'][core]
        in_maps.append(m)
    return in_maps


def kernel(**inputs) -> np.ndarray:
    _ensure_path()
    from concourse.bass_utils import run_bass_kernel_spmd

    scale = inputs.get('scale', S)
    scale = int(np.asarray(scale)) if not isinstance(scale, int) else scale
    assert scale == S, f"kernel hardcodes scale={S}, got {scale}"

    plan, host = _plan_and_host_data(inputs)
    nc = _build_graph(plan, host)

    in_maps = make_in_maps(host)
    res = run_bass_kernel_spmd(nc, in_maps, core_ids=list(range(NCORES)))

    Q = host['Q']
    q = np.zeros((Q, 3), np.float32)
    if GATHER:
        BLK = plan['BLK']
        NQBP = plan['NQBP']
        per = NQBP // 128
        for core in range(NCORES):
            blocks = np.asarray(res.results[core]['out']).astype(
                np.float32).reshape(128, 4, BLK * 12)
            for nb in range(4):
                sel = host['originals'][core][nb]
                sub = host['subsels'][core][nb]
                loc = host['localoff'][core][nb]
                n = sel.size
                if n == 0:
                    continue
                prt = (np.arange(n) // per)
                base = loc[:n] * 12 + sub * 3
                cols = base[:, None] + np.arange(3)[None]
                q[sel] = np.take_along_axis(blocks[prt, nb], cols, axis=1)
    else:
        core_of, bank_q = host['core_of'], host['bank_q']
        grow, sub_q = host['grow'], host['sub_q']
        outs = np.stack([np.asarray(res.results[core]['out'])
                         for core in range(NCORES)]).astype(np.float32)
        # out[core][xl, nb*192 + (t*4+g)*12 + k]; grow = xl*16 + t*4 + g
        xl = grow // 16
        cols = bank_q * 192 + (grow % 16) * 12 + sub_q * 3
        for c in range(3):
            q[:, c] = outs[core_of, xl, cols + c]
    return q[None]


# revision 18
# speedup vs baseline: 2.5163x; 1.0011x over previous
"""ArbSR (moe_routing) Trainium2 kernel, 8-core SPMD.

Structure exploited: with scale=4, the scale-embedding MLP input is periodic
with period 4 in both HR axes, so routing r, offsets off, and the expert-mix
matrices take only 16 distinct values (one per (y%4, x%4) class).  The
offset grid_sample then becomes, per class, a 2x2-tap bilinear filter of the
encoder feature map f at a constant integer shift, and the whole
  encoder conv -> fea0 -> expert mixing -> (+fea0) -> 3x3 tail conv
chain collapses, after folding the encoder INTO the per-delta matrices
(everything is linear in the input image), to
  pred[:, 4*yl+b, 4*xl+a] = tail_b
      + sum_{dy,dx} (E[(b,a)][(dy,dx)] @ encw) @ im2col[:, yl+dy, xl+dx]
with host-precomputed [48, 28] matrices per (dy, dx).  The dy axis is packed
into the contraction dim (K = 28*ndy) using a row-shift-stacked im2col, so
the device runs ONE matmul per (bank of 4 LR rows, dx): 12 K~117 matmuls of
N=512 for the whole main computation.  The tail bias rides on an all-ones
rhs K row; the left/right tail-conv zero-pad corrections ride on one-hot
rhs K rows whose lhsT coefficients are host-computed exact (true - fold)
values; the top/bot row corrections (cores 0/7) are accumulated into PSUM
by an identity-lhsT matmul against a host-computed [48, W] strip.

Per core (64 HR rows): 14 matmuls + 16 PE transposes into a pixel-major
layout D (rows keyed xl*16+t*4+g so a D write is one contiguous 384B
descriptor per partition); an indirect-DMA block-gather for the
nearest-neighbour queries, which the host routes to cores/banks by row
ownership and sorts so each of 128 partitions fetches one contiguous span.
"""

import numpy as np
import ml_dtypes

BF16 = ml_dtypes.bfloat16


def _ensure_path():
    import sys
    for p in ('/opt/trn_rl_repo',):
        if p not in sys.path:
            sys.path.append(p)


H = W = 128
S = 4
HH = WH = H * S          # 512
C = 64
NCORES = 8
YLC = H // NCORES        # 16 LR rows per core
HRPC = HH // NCORES      # 64 HR rows per core
NPIX = HRPC * WH         # 32768 HR pixels per core
NCLS = 16                # (b, a) classes
MROWS = NCLS * 3         # 48 stacked pred rows
KE = 28                  # encoder-folded contraction rows per dy block
NROWS_B = 2048           # D rows per bank (128 xl * 4 t * 4 g)

GATHER = False           # False: dump D as output, gather on host
EDGE_IN_MAIN = True      # fold top/bot row corr into the main PSUM group
T_SHARED = True          # transposes share one PSUM tile per bank
SPLIT_DRAIN = True       # split PSUM drains between Vector and Scalar
NO_SCALAR = True         # no scalar.activation at all: kills the 1.3us
                         # ACT_TABLE_LOAD that blocks scalar's DMA queue


def _sigmoid(x):
    return 1.0 / (1.0 + np.exp(-x))


def _class_constants(d):
    w1 = np.asarray(d['body_w1'], np.float64)
    b1 = np.asarray(d['body_b1'], np.float64)
    w2 = np.asarray(d['body_w2'], np.float64)
    b2 = np.asarray(d['body_b2'], np.float64)
    rw = np.asarray(d['routing_w'], np.float64)
    rb = np.asarray(d['routing_b'], np.float64)
    ow = np.asarray(d['offset_w'], np.float64)
    ob = np.asarray(d['offset_b'], np.float64)
    wc = np.asarray(d['weight_compress'], np.float64)
    we = np.asarray(d['weight_expand'], np.float64)

    fs = float(S)
    coor = np.array([(i + 0.5) / fs - np.floor((i + 0.5) / fs + 0.001) - 0.5
                     for i in range(S)])
    cls = {}
    for b in range(S):
        for a in range(S):
            inp4 = np.array([1.0 / fs, 1.0 / fs, coor[b], coor[a]])
            emb = np.maximum(w1 @ inp4 + b1, 0.0)
            emb = np.maximum(w2 @ emb + b2, 0.0)
            off = ow @ emb + ob
            r = _sigmoid(rw @ emb + rb)
            A = np.einsum('e,eck->ck', r, we) @ np.einsum('e,ekc->kc', r, wc)
            B = A + np.eye(C)
            cx = (a + 0.5) / fs - 0.5 + off[0]
            cy = (b + 0.5) / fs - 0.5 + off[1]
            ix, iy = int(np.floor(cx)), int(np.floor(cy))
            fx, fy = cx - ix, cy - iy
            wbl = {(0, 0): (1 - fy) * (1 - fx), (0, 1): (1 - fy) * fx,
                   (1, 0): fy * (1 - fx), (1, 1): fy * fx}
            cls[(b, a)] = dict(B=B, ix=ix, iy=iy, wbl=wbl)
    return cls


def _build_E(tail_w, cls):
    """E[(b,a)][(dy,dx)] = [3, C] so that pred contribution is E @ f(shift)."""
    Es = {}
    for b in range(S):
        for a in range(S):
            acc = {}
            for ty in range(3):
                for tx in range(3):
                    bp = (b + ty - 1) % S
                    oy = (b + ty - 1 - bp) // S
                    ap_ = (a + tx - 1) % S
                    ox = (a + tx - 1 - ap_) // S
                    c2 = cls[(bp, ap_)]
                    TB = tail_w[:, :, ty, tx] @ c2['B']
                    for (uy, ux), wgt in c2['wbl'].items():
                        if wgt == 0.0:
                            continue
                        key = (oy + c2['iy'] + uy, ox + c2['ix'] + ux)
                        acc[key] = acc.get(key, np.zeros((3, C))) + TB * wgt
            Es[(b, a)] = acc
    return Es


def _build_encw(d):
    """encw [28, C]: rows = 9 taps x 3 ch + inside-mask bias row."""
    enc_w = np.asarray(d['enc_w'], np.float64)
    enc_b = np.asarray(d['enc_b'], np.float64)
    encw = np.zeros((KE, C))
    for ty in range(3):
        for tx in range(3):
            for ch in range(3):
                encw[(ty * 3 + tx) * 3 + ch, :] = enc_w[:, ch, ty, tx]
    encw[27, :] = enc_b
    return encw


PADF = 4   # f64 f-map padding margin (covers all shift indexing)


def _host_f(d):
    """f64 encoder output, zero outside the image, with PADF margin."""
    inp = np.asarray(d['inp'], np.float64)[0]
    ip = np.pad(inp, ((0, 0), (1, 1), (1, 1)))
    enc_w = np.asarray(d['enc_w'], np.float64)
    enc_b = np.asarray(d['enc_b'], np.float64)
    f = np.zeros((C, H, W))
    for ty in range(3):
        for tx in range(3):
            f += np.einsum('oc,chw->ohw', enc_w[:, :, ty, tx],
                           ip[:, ty:ty + H, tx:tx + W])
    f += enc_b[:, None, None]
    fpad = np.zeros((C, H + 2 * PADF, W + 2 * PADF))
    fpad[:, PADF:PADF + H, PADF:PADF + W] = f
    return fpad


def _zgrid(cls, fpad, yHs, xHs):
    """z = out2 + fea0 (zero outside the HR image) on a coordinate grid."""
    yh = np.asarray(yHs)
    xh = np.asarray(xHs)
    out = np.zeros((C, len(yh), len(xh)))
    for b_ in range(S):
        rm = np.nonzero(np.mod(yh, S) == b_)[0]
        if rm.size == 0:
            continue
        ys = yh[rm]
        yl = ys // S
        for a_ in range(S):
            cm = np.nonzero(np.mod(xh, S) == a_)[0]
            if cm.size == 0:
                continue
            xs = xh[cm]
            xl = xs // S
            inside = ((ys[:, None] >= 0) & (ys[:, None] < HH)
                      & (xs[None, :] >= 0) & (xs[None, :] < WH))
            c2 = cls[(b_, a_)]
            fg = np.zeros((C, rm.size, cm.size))
            for (uy, ux), wgt in c2['wbl'].items():
                rr = PADF + yl + c2['iy'] + uy
                cc = PADF + xl + c2['ix'] + ux
                fg += wgt * fpad[:, rr[:, None], cc[None, :]]
            val = np.einsum('oc,cyx->oyx', c2['B'], fg)
            val *= inside[None]
            out[np.ix_(np.arange(C), rm, cm)] = val
    return out


def _true_strip(cls, fpad, tail_w, tail_b, ylgs, xs_lr):
    """Exact pred values [MROWS, len(ylgs), len(xs_lr)] (LR coords)."""
    ylgs = np.asarray(ylgs)
    xs_lr = np.asarray(xs_lr)
    yHs = np.arange(S * ylgs.min() - 1, S * ylgs.max() + S + 1)
    xHs = np.arange(S * xs_lr.min() - 1, S * xs_lr.max() + S + 1)
    z = _zgrid(cls, fpad, yHs, xHs)
    y0, x0 = yHs[0], xHs[0]
    out = np.zeros((MROWS, len(ylgs), len(xs_lr)))
    for b in range(S):
        for a in range(S):
            m0 = (4 * b + a) * 3
            acc = np.zeros((3, len(ylgs), len(xs_lr)))
            for ty in range(3):
                rr = S * ylgs + b + ty - 1 - y0
                for tx in range(3):
                    cc = S * xs_lr + a + tx - 1 - x0
                    acc += np.einsum(
                        'oc,cyx->oyx', tail_w[:, :, ty, tx],
                        z[:, rr[:, None], cc[None, :]])
            out[m0:m0 + 3] = acc + tail_b[:, None, None]
    return out


def _fold_strip(G_main, bias48, fpad, ylgs, xs_lr):
    """What the device mains+bias compute, in f64 (LR coords)."""
    ylgs = np.asarray(ylgs)
    xs_lr = np.asarray(xs_lr)
    out = np.zeros((MROWS, len(ylgs), len(xs_lr)))
    for (dy, dx), G in G_main.items():
        rr = PADF + ylgs + dy
        cc = PADF + xs_lr + dx
        out += np.einsum('mc,cyx->myx', G, fpad[:, rr[:, None], cc[None, :]])
    return out + bias48[:, None, None]


def _plan_and_host_data(d):
    """Host precompute: folded lhsT matrices, per-core im2colK, edge
    corrections, query routing."""
    cls = _class_constants(d)
    tail_w = np.asarray(d['tail_w'], np.float64)
    tail_b = np.asarray(d['tail_b'], np.float64)
    encw = _build_encw(d)

    E_main = _build_E(tail_w, cls)
    deltas = sorted({k for acc in E_main.values() for k in acc})
    dys = sorted({dl[0] for dl in deltas})
    dxs = sorted({dl[1] for dl in deltas})
    dy_min, dy_max = min(dys), max(dys)
    dx_min, dx_max = min(dxs), max(dxs)
    ndy = dy_max - dy_min + 1
    ndx = dx_max - dx_min + 1
    KA = KE * ndy                    # encoder-folded rows
    KM = KA + 1 + 2 * YLC            # + ones row + lef/rig one-hot rows
    assert KM <= 128, (ndy, KM)
    NRB = YLC + ndy                  # base im2col rows
    NCF = W + ndx - 1                # im2col cols
    NFK = YLC * NCF                  # free size of the stacked im2colK

    G_main = {dl: np.zeros((MROWS, C)) for dl in deltas}
    for (b, a), acc in E_main.items():
        m0 = (4 * b + a) * 3
        for dl, M in acc.items():
            G_main[dl][m0:m0 + 3, :] += M

    bias48 = np.zeros(MROWS)
    for b in range(S):
        for a in range(S):
            bias48[(4 * b + a) * 3:(4 * b + a) * 3 + 3] = tail_b

    # ---- exact edge corrections (true - fold), f64 on host ----
    fpad = _host_f(d)
    all_yl = np.arange(H)
    t_lef = _true_strip(cls, fpad, tail_w, tail_b, all_yl, [0])[:, :, 0]
    t_rig = _true_strip(cls, fpad, tail_w, tail_b, all_yl, [W - 1])[:, :, 0]
    f_lef = _fold_strip(G_main, bias48, fpad, all_yl, [0])[:, :, 0]
    f_rig = _fold_strip(G_main, bias48, fpad, all_yl, [W - 1])[:, :, 0]
    corrL = t_lef - f_lef            # [MROWS, H]
    corrR = t_rig - f_rig
    all_x = np.arange(W)
    t_top = _true_strip(cls, fpad, tail_w, tail_b, [0], all_x)[:, 0, :]
    f_top = _fold_strip(G_main, bias48, fpad, [0], all_x)[:, 0, :]
    corrT = t_top - f_top            # [MROWS, W]
    corrT[:, 0] -= corrL[:, 0]
    corrT[:, W - 1] -= corrR[:, 0]
    t_bot = _true_strip(cls, fpad, tail_w, tail_b, [H - 1], all_x)[:, 0, :]
    f_bot = _fold_strip(G_main, bias48, fpad, [H - 1], all_x)[:, 0, :]
    corrB = t_bot - f_bot
    corrB[:, 0] -= corrL[:, H - 1]
    corrB[:, W - 1] -= corrR[:, H - 1]

    # ---- main lhsT per dx: [KM, MROWS] ----
    def fold(G):
        return encw @ G.T

    main_T = []
    for dx in range(dx_min, dx_max + 1):
        lhsT = np.zeros((KM, MROWS))
        for dy in range(dy_min, dy_max + 1):
            if (dy, dx) in G_main:
                j = dy - dy_min
                lhsT[KE * j:KE * j + KE, :] = fold(G_main[(dy, dx)])
        main_T.append(lhsT)
    # bias + per-core lef/rig rows land in the dx=0 seg (c0 = -dx_min)

    # ---- per-core im2colK [KM, YLC*NCF] (dy-shift-stacked) ----
    inp = np.asarray(d['inp'], np.float64)[0]   # [3, H, W]
    PADX = 8
    ippad = np.pad(inp, ((0, 0), (PADX, PADX), (PADX, PADX)))
    ones = np.zeros((H + 2 * PADX, W + 2 * PADX))
    ones[PADX:PADX + H, PADX:PADX + W] = 1.0
    im2cols = []
    for core in range(NCORES):
        y0 = YLC * core + dy_min          # global LR row of base row 0
        x0 = dx_min
        base = np.zeros((KE, NRB, NCF))
        for ty in range(3):
            for tx in range(3):
                ys = PADX + y0 + ty - 1
                xs = PADX + x0 + tx - 1
                for ch in range(3):
                    base[(ty * 3 + tx) * 3 + ch] = \
                        ippad[ch, ys:ys + NRB, xs:xs + NCF]
        inside = ones[PADX + y0:PADX + y0 + NRB, PADX + x0:PADX + x0 + NCF]
        base[27] = inside
        # f must be exactly zero at out-of-image positions (grid-sample
        # zero padding), so kill whole columns there, not just oob taps
        base *= inside[None]
        imk = np.zeros((KM, YLC, NCF), np.float32)
        for j in range(ndy):
            imk[KE * j:KE * j + KE] = base[:, j:j + YLC, :]
        imk[KA] = 1.0                              # bias row
        for yl in range(YLC):                      # lef/rig one-hot rows
            imk[KA + 1 + yl, yl, -dx_min] = 1.0
            imk[KA + 1 + YLC + yl, yl, (W - 1) - dx_min] = 1.0
        im2cols.append(imk.reshape(KM, NFK).astype(BF16))

    # ---- query routing (f32 math matches reference rounding) ----
    coord = np.asarray(d['coord'], np.float32)[0]
    cell = np.asarray(d['cell'], np.float32)[0]
    cq = np.clip(coord - cell * np.float32(0.5) + np.float32(1e-6),
                 np.float32(-1 + 1e-6), np.float32(1 - 1e-6))
    xi = np.clip(np.round((cq[:, 1] + 1) * np.float32(0.5) * (WH - 1)
                          ).astype(np.int64), 0, WH - 1)
    yi = np.clip(np.round((cq[:, 0] + 1) * np.float32(0.5) * (HH - 1)
                          ).astype(np.int64), 0, HH - 1)
    core_of = yi // HRPC
    ylq = (yi % HRPC) // S
    bq = yi % S
    xlq = xi // S
    aq = xi % S
    cls_q = bq * S + aq
    bank_q = ylq // 4
    # D row within a bank: xl*16 + t*4 + g (t = ylq%4, g = cls//4) so a
    # bank's D write is contiguous per partition xl
    grow = xlq * 16 + (ylq % 4) * 4 + cls_q // 4
    sub_q = cls_q % 4                                 # 3-float slot in row
    Q = coord.shape[0]

    host = dict(consts=None, im2cols=im2cols, Q=Q)
    plan = dict(
        dy_min=dy_min, dx_min=dx_min, ndy=ndy, ndx=ndx, KM=KM, KA=KA,
        NCF=NCF, NFK=NFK,
    )

    if GATHER:
        # Bank-pipelined sorted block-gather: per (core, bank), sort
        # queries by D row, split into 128 groups; partition p
        # block-fetches its span.
        per_cb = [[np.nonzero((core_of == core) & (bank_q == nb))[0]
                   for nb in range(4)] for core in range(NCORES)]
        NQBP = 128 * ((max(s.size for row in per_cb for s in row) + 127)
                      // 128)
        NQBP = max(NQBP, 128)
        idx_arrays, originals, subsels, localoff = [], [], [], []
        max_span = 1
        per = NQBP // 128
        for core in range(NCORES):
            lo4, org4, sub4, loc4 = [], [], [], []
            for nb in range(4):
                sel = per_cb[core][nb]
                rows = np.zeros(NQBP, np.int64)
                rows[:sel.size] = grow[sel]
                if sel.size:
                    rows[sel.size:] = rows[:sel.size].max()
                order = np.argsort(rows[:sel.size], kind='stable')
                rows_sorted = np.concatenate([rows[:sel.size][order],
                                              rows[sel.size:]])
                lo = rows_sorted.reshape(128, per)[:, 0].copy()
                span = rows_sorted.reshape(128, per)[:, -1] - lo + 1
                max_span = max(max_span, int(span.max()))
                lo4.append(lo)
                org4.append(sel[order])
                sub4.append(sub_q[sel][order])
                loc4.append(rows_sorted - np.repeat(lo, per))
            idx_arrays.append(lo4)
            originals.append(org4)
            subsels.append(sub4)
            localoff.append(loc4)
        BLK = min(NROWS_B, ((max_span + 3) // 4) * 4)
        for core in range(NCORES):
            lo4 = idx_arrays[core]
            for nb in range(4):
                lo = lo4[nb]
                lo2 = np.clip(np.minimum(lo, NROWS_B - BLK), 0, None)
                localoff[core][nb] = (
                    localoff[core][nb]
                    + np.repeat(lo - lo2, per)).astype(np.int64)
                assert (localoff[core][nb] < BLK).all()
                assert (localoff[core][nb] >= 0).all()
                lo4[nb] = lo2
            idx_arrays[core] = np.stack(
                [l.astype(np.int32) for l in lo4], axis=1)   # [128, 4]
        plan['BLK'] = BLK
        plan['NQBP'] = NQBP
        host.update(idx_arrays=idx_arrays, originals=originals,
                    subsels=subsels, localoff=localoff)
    else:
        host.update(core_of=core_of, bank_q=bank_q, grow=grow, sub_q=sub_q)

    # ---- pack constants into one [128, CW] blob ----
    segs = {}
    col = [0]

    def alloc(name, K, Mw):
        segs[name] = (0, col[0], K, Mw)
        col[0] += Mw

    for i in range(ndx):
        alloc(f'Em{i}', KM, MROWS)
    alloc('corrT', MROWS, W)
    alloc('corrB', MROWS, W)
    CW = col[0]
    plan['segs'] = segs
    plan['CW'] = CW

    i_dx0 = -dx_min
    consts_cores = []
    for core in range(NCORES):
        blob = np.zeros((128, CW), np.float32)
        for i, t in enumerate(main_T):
            t = t.copy()
            if i == i_dx0:
                t[KA, :] = bias48
                yls = YLC * core + np.arange(YLC)
                t[KA + 1:KA + 1 + YLC, :] = corrL[:, yls].T
                t[KA + 1 + YLC:KM, :] = corrR[:, yls].T
            p0, c0, K, Mw = segs[f'Em{i}']
            blob[p0:p0 + K, c0:c0 + Mw] = t
        p0, c0, K, Mw = segs['corrT']
        if core == 0:
            blob[p0:p0 + K, c0:c0 + Mw] = corrT
        p0, c0, K, Mw = segs['corrB']
        if core == NCORES - 1:
            blob[p0:p0 + K, c0:c0 + Mw] = corrB
        consts_cores.append(blob.astype(BF16))
    host['consts'] = consts_cores
    return plan, host


def _build_graph(plan, host, debug_outputs=False):
    _ensure_path()
    import concourse.bass as bass
    import concourse.bacc as bacc
    import concourse.mybir as mybir
    import concourse.tile as tile
    from concourse.masks import make_identity

    f32 = mybir.dt.float32
    bf16 = mybir.dt.bfloat16
    i32 = mybir.dt.int32

    KM, NCF, NFK = plan['KM'], plan['NCF'], plan['NFK']
    dx_min, ndx = plan['dx_min'], plan['ndx']
    segs, CW = plan['segs'], plan['CW']

    nc = bacc.Bacc(None, target_bir_lowering=False, debug=False,
                   num_devices=NCORES)

    imk_d = nc.dram_tensor('im2col', [KM, NFK], bf16, kind='ExternalInput')
    consts_d = nc.dram_tensor('consts', [128, CW], bf16,
                              kind='ExternalInput')
    if GATHER:
        BLK = plan['BLK']
        idx_d = nc.dram_tensor('idx', [128, 4], i32, kind='ExternalInput')
        out_d = nc.dram_tensor('out', [128, 4 * BLK * 12], bf16,
                               kind='ExternalOutput')
    else:
        out_d = nc.dram_tensor('out', [128, YLC * MROWS], bf16,
                               kind='ExternalOutput')
    if debug_outputs:
        dbg_pred = nc.dram_tensor('dbg_pred', [MROWS, YLC * W], bf16,
                                  kind='ExternalOutput')

    with tile.TileContext(nc) as tc:
        with (
            tc.tile_pool(name='sb', bufs=1) as sb,
            tc.tile_pool(name='sbsmall', bufs=1) as sbs,
            tc.tile_pool(name='pshare', bufs=2, space='PSUM') as pshare,
            tc.tile_pool(name='ppred', bufs=1, space='PSUM') as ppred,
            tc.tile_pool(name='dram', bufs=1, space='DRAM') as dpool,
        ):
            consts_t = sb.tile([128, CW], bf16)
            imk = sb.tile([KM, NFK], bf16)
            # consts first (small; first LDWEIGHTS needs it); imk in 4
            # row-group chunks (bank nb reads rows 4nb..4nb+4 only),
            # spread across engine queues so the ~0.8us descriptor
            # generation per DMA runs in parallel
            # scalar's queue is blocked ~1.3us by the auto ACT_TABLE_LOAD,
            # so everything the first matmuls need goes on sync
            RC = [0, 4 * NCF, 8 * NCF, 12 * NCF, NFK]
            if NO_SCALAR:
                # scalar's queue is free (no ACT_TABLE_LOAD): desc-gen of
                # consts and imk-A runs in parallel
                nc.scalar.dma_start(consts_t[:], consts_d[:])
                nc.sync.dma_start(imk[:, RC[0]:RC[1]], imk_d[:, RC[0]:RC[1]])
                nc.sync.dma_start(imk[:, RC[1]:RC[2]], imk_d[:, RC[1]:RC[2]])
                nc.scalar.dma_start(imk[:, RC[2]:RC[3]], imk_d[:, RC[2]:RC[3]])
                nc.sync.dma_start(imk[:, RC[3]:RC[4]], imk_d[:, RC[3]:RC[4]])
            else:
                nc.sync.dma_start(consts_t[:], consts_d[:])
                nc.sync.dma_start(imk[:, RC[0]:RC[1]], imk_d[:, RC[0]:RC[1]])
                nc.sync.dma_start(imk[:, RC[1]:RC[2]], imk_d[:, RC[1]:RC[2]])
                nc.scalar.dma_start(imk[:, RC[2]:RC[3]], imk_d[:, RC[2]:RC[3]])
                nc.sync.dma_start(imk[:, RC[3]:RC[4]], imk_d[:, RC[3]:RC[4]])
            if GATHER:
                idx_t = sbs.tile([128, 4], i32)
                nc.scalar.dma_start(idx_t[:], idx_d[:])
                D_ts = [dpool.tile([NROWS_B, 12], bf16, tag=f'D{nb}',
                                   name=f'Dscr{nb}')
                        for nb in range(4)]
                D2s = [D_ts[nb][:].rearrange('(xl r) k -> xl (r k)', xl=128)
                       for nb in range(4)]
                gath = sb.tile([128, 4 * BLK * 12], bf16)

            def cseg(name):
                p0, c0, K, Mw = segs[name]
                return consts_t[p0:p0 + K, c0:c0 + Mw]

            ident = sbs.tile([MROWS, MROWS], bf16)
            make_identity(nc, ident[:])

            imk3 = imk[:].rearrange('p (r c) -> p r c', c=NCF)
            # one PSUM tile per bank so drains don't wait on later
            # banks' matmuls (Tile tracks deps at tile granularity)
            pred_ps = [ppred.tile([MROWS, 512], f32, tag=f'pp{nb}',
                                  name=f'pp{nb}')
                       for nb in range(4)]

            # ---- PE stream: 3 mains per bank (+ top/bot rows for banks
            # 0/3) accumulated into one PSUM bank per bank ----
            for nb in range(4):
                edge = EDGE_IN_MAIN and ((nb == 0) or (nb == 3))
                for i in range(ndx):
                    nc.tensor.matmul(
                        pred_ps[nb][:],
                        cseg(f'Em{i}'),
                        imk3[0:KM, 4 * nb:4 * nb + 4, i:i + W],
                        start=(i == 0), stop=(i == ndx - 1 and not edge),
                        skip_group_check=True)
                if edge and nb == 0:
                    nc.tensor.matmul(
                        pred_ps[0][:, 0:W], ident[:], cseg('corrT'),
                        start=False, stop=True, skip_group_check=True)
                if edge and nb == 3:
                    nc.tensor.matmul(
                        pred_ps[3][:, 3 * W:4 * W], ident[:],
                        cseg('corrB'),
                        start=False, stop=True, skip_group_check=True)

            # ---- per-bank drain -> transpose -> D copy -> D write ->
            # gather -> out ----
            pred_sb = sb.tile([MROWS, YLC * W], bf16)
            D_sb = sb.tile([128, YLC * MROWS], bf16)

            for nb in range(4):
                if NO_SCALAR:
                    nc.vector.tensor_copy(
                        pred_sb[:, nb * 512:(nb + 1) * 512],
                        pred_ps[nb][:])
                elif SPLIT_DRAIN:
                    nc.vector.tensor_copy(
                        pred_sb[:, nb * 512:nb * 512 + 256],
                        pred_ps[nb][:, 0:256])
                    nc.scalar.activation(
                        pred_sb[:, nb * 512 + 256:(nb + 1) * 512],
                        pred_ps[nb][:, 256:512],
                        mybir.ActivationFunctionType.Copy)
                else:
                    nc.vector.tensor_copy(
                        pred_sb[:, nb * 512:(nb + 1) * 512],
                        pred_ps[nb][:])
                if not EDGE_IN_MAIN and nb == 0:
                    nc.vector.tensor_add(pred_sb[:, 0:W], pred_sb[:, 0:W],
                                         cseg('corrT'))
                if not EDGE_IN_MAIN and nb == 3:
                    nc.vector.tensor_add(pred_sb[:, (YLC - 1) * W:YLC * W],
                                         pred_sb[:, (YLC - 1) * W:YLC * W],
                                         cseg('corrB'))
                if T_SHARED:
                    pt = pshare.tile([128, 4 * MROWS], bf16, tag='pshare')
                    for t in range(4):
                        ch = 4 * nb + t
                        nc.tensor.transpose(
                            pt[:, t * MROWS:(t + 1) * MROWS],
                            pred_sb[:, ch * W:(ch + 1) * W], ident[:])
                    if NO_SCALAR:
                        nc.vector.tensor_copy(
                            D_sb[:, nb * 192:(nb + 1) * 192], pt[:, 0:192])
                    else:
                        nc.vector.tensor_copy(
                            D_sb[:, nb * 192:nb * 192 + 96], pt[:, 0:96])
                        nc.scalar.activation(
                            D_sb[:, nb * 192 + 96:(nb + 1) * 192],
                            pt[:, 96:192],
                            mybir.ActivationFunctionType.Copy)
                else:
                    for t in range(4):
                        ch = 4 * nb + t
                        pt = pshare.tile([128, MROWS], bf16, tag='pshare')
                        nc.tensor.transpose(
                            pt[:], pred_sb[:, ch * W:(ch + 1) * W],
                            ident[:])
                        nc.scalar.activation(
                            D_sb[:, ch * MROWS:(ch + 1) * MROWS], pt[:],
                            mybir.ActivationFunctionType.Copy)
                if GATHER:
                    nc.sync.dma_start(D2s[nb][:, :],
                                      D_sb[:, nb * 192:(nb + 1) * 192])
                    nc.gpsimd.indirect_dma_start(
                        out=gath[:, nb * BLK * 12:(nb + 1) * BLK * 12],
                        out_offset=None,
                        in_=D_ts[nb][:],
                        in_offset=bass.IndirectOffsetOnAxis(
                            ap=idx_t[:, nb:nb + 1], axis=0))
                    nc.scalar.dma_start(
                        out_d[:, nb * BLK * 12:(nb + 1) * BLK * 12],
                        gath[:, nb * BLK * 12:(nb + 1) * BLK * 12])
                else:
                    nc.sync.dma_start(
                        out_d[:, nb * 192:(nb + 1) * 192],
                        D_sb[:, nb * 192:(nb + 1) * 192])

            if debug_outputs:
                nc.sync.dma_start(dbg_pred[:], pred_sb[:])

    nc.compile()
    return nc


def make_in_maps(host):
    in_maps = []
    for core in range(NCORES):
        m = {
            'im2col': host['im2cols'][core],
            'consts': host['consts'][core],
        }
        if GATHER:
            m['idx'] = host['idx_arrays'][core]
        in_maps.append(m)
    return in_maps


def kernel(**inputs) -> np.ndarray:
    _ensure_path()
    from concourse.bass_utils import run_bass_kernel_spmd

    scale = inputs.get('scale', S)
    scale = int(np.asarray(scale)) if not isinstance(scale, int) else scale
    assert scale == S, f"kernel hardcodes scale={S}, got {scale}"

    plan, host = _plan_and_host_data(inputs)
    nc = _build_graph(plan, host)

    in_maps = make_in_maps(host)
    res = run_bass_kernel_spmd(nc, in_maps, core_ids=list(range(NCORES)))

    Q = host['Q']
    q = np.zeros((Q, 3), np.float32)
    if GATHER:
        BLK = plan['BLK']
        NQBP = plan['NQBP']
        per = NQBP // 128
        for core in range(NCORES):
            blocks = np.asarray(res.results[core]['out']).astype(
                np.float32).reshape(128, 4, BLK * 12)
            for nb in range(4):
                sel = host['originals'][core][nb]
                sub = host['subsels'][core][nb]
                loc = host['localoff'][core][nb]
                n = sel.size
                if n == 0:
                    continue
                prt = (np.arange(n) // per)
                base = loc[:n] * 12 + sub * 3
                cols = base[:, None] + np.arange(3)[None]
                q[sel] = np.take_along_axis(blocks[prt, nb], cols, axis=1)
    else:
        core_of, bank_q = host['core_of'], host['bank_q']
        grow, sub_q = host['grow'], host['sub_q']
        outs = np.stack([np.asarray(res.results[core]['out'])
                         for core in range(NCORES)]).astype(np.float32)
        # out[core][xl, nb*192 + (t*4+g)*12 + k]; grow = xl*16 + t*4 + g
        xl = grow // 16
        cols = bank_q * 192 + (grow % 16) * 12 + sub_q * 3
        for c in range(3):
            q[:, c] = outs[core_of, xl, cols + c]
    return q[None]
